# revision 2
# baseline (speedup 1.0000x reference)
"""Multi-graph 2-layer GCN on 8 Trainium2 NeuronCores — fused single launch, v3.

v3 over v2:
- Per-core dst blocks are sorted by edge count and packed into chunk
  positions with per-position tile caps (max over cores), cutting gather
  padding from ~19% to ~3%.
- The t2-half AllGather is split into 4 pieces interleaved into the L1
  chunk stream so most of the exchange hides behind L1 gathers.
- Layer-2 gathers use a second index table (idx2) addressing the
  piecewise/sorted t2 table layout; layer-1 indices stay natural.

See kernel_v2 docstring for the base design (bf16 pair-row gather tables,
parity tiles, one-hot S' matmuls with packed-bf16 DVE builds, dinv folded
into PE transposes, ELU = max(x+1, exp(min(x,0))) - 1).
"""

import sys

try:
    import concourse.bass as bass  # noqa: F401
except ImportError:
    sys.path.insert(0, "/opt/trn_rl_repo")
    import concourse.bass as bass

import numpy as np
import ml_dtypes

import concourse.tile as tile_mod  # noqa: F401
from concourse import bacc
import concourse.mybir as mybir
from concourse.bass_utils import run_bass_kernel_spmd
from concourse.tile import TileContext, add_dep_helper
from concourse.masks import make_identity

AF = mybir.ActivationFunctionType
ALU = mybir.AluOpType
F32 = mybir.dt.float32
BF16 = mybir.dt.bfloat16
I16 = mybir.dt.int16

BF_NP = ml_dtypes.bfloat16


def _patched_drain_and_barrier(self, tick_clock, wait_clock):
    from bass_rust import ScopedClock

    probe = self.nc.sync.nop()
    wait_clock.add_sem_waits(probe.ins, ScopedClock({None: tick_clock.global_clock}))
    si = probe.ins.sync_info
    waits = list(si.on_wait) if si and si.on_wait else []
    if si is not None:
        si.on_wait = waits[:1]
    for w in waits[1:]:
        n = self.nc.sync.nop()
        nsi = n.ins.sync_info
        if nsi is None:
            n.ins.sync_info = mybir.SyncInfo(on_wait=[w], on_update=[])
        else:
            nsi.on_wait = [w]
    self.nc.sync.drain()
    self.nc.all_engine_barrier()
    popped = self.nc._tile_sem_poison_stack.pop()
    assert popped is self._sem_poison
    self.nc.clear_and_free_semaphores(list(self.sems.allocated().values()))
    self.nc.all_engine_barrier()


TileContext._drain_and_barrier = _patched_drain_and_barrier

_orig_add_instruction = TileContext._add_instruction
_waitsplit_counter = [0]


def _patched_add_instruction(self, inst):
    """walrus rejects instructions carrying >1 sem wait; hoist excess waits
    onto same-engine nops inserted immediately before the instruction."""
    si = inst.sync_info
    if (si is not None and si.on_wait and len(si.on_wait) > 1
            and inst.engine != mybir.EngineType.Unassigned):
        waits = list(si.on_wait)
        si.on_wait = waits[-1:]
        for w in waits[:-1]:
            _waitsplit_counter[0] += 1
            nop = mybir.InstNoOp(
                name=f"I-wsplit-{_waitsplit_counter[0]}", ins=[], outs=[])
            nop.engine = inst.engine
            nop.sync_info = mybir.SyncInfo(on_wait=[w], on_update=[])
            _orig_add_instruction(self, nop)
    _orig_add_instruction(self, inst)


TileContext._add_instruction = _patched_add_instruction


# ---------------------------------------------------------------------------
# Config
# ---------------------------------------------------------------------------
class Cfg:
    def __init__(self, G, N, E, F_IN, HID, OUT, chunk=4):
        self.G, self.N, self.E = G, N, E
        self.F_IN, self.HID, self.OUT = F_IN, HID, OUT
        assert F_IN == OUT == 64 and HID == 128
        self.NB = (N + 255) // 256 * 2
        self.NPAD = self.NB * 128
        self.NBH = self.NB // 2
        self.HALF = self.NBH * 128
        self.PAIRS = self.NPAD // 2
        self.PHALF = self.HALF // 2
        self.CHUNK = chunk
        assert self.NBH % chunk == 0
        self.NCHUNK = self.NBH // chunk


CFG = Cfg(G=4, N=50000, E=800000, F_IN=64, HID=128, OUT=64, chunk=4)
NPIECE = 4


def _piece_bounds(cfg):
    n = cfg.NCHUNK
    step = n // NPIECE
    return [p * step for p in range(NPIECE)] + [n]


# ---------------------------------------------------------------------------
# Layout derivation shared by host packing and device program
# ---------------------------------------------------------------------------
def derive_layout(cfg, capsE, capsO):
    """capsE/capsO: [NCHUNK][CHUNK] ints. Returns static layout tables."""
    NCH, CH = cfg.NCHUNK, cfg.CHUNK
    cap2 = [[capsE[c][b] + capsO[c][b] for b in range(CH)] for c in range(NCH)]
    T = [sum(cap2[c]) for c in range(NCH)]
    tbase = [[0] * CH for _ in range(NCH)]
    for c in range(NCH):
        for b in range(1, CH):
            tbase[c][b] = tbase[c][b - 1] + cap2[c][b - 1]
    colbase = [[0] * CH for _ in range(NCH)]
    acc = 0
    for c in range(NCH):
        for b in range(CH):
            colbase[c][b] = acc
            acc += cap2[c][b]
    totcols = acc
    slotoff = [0] * NCH
    for c in range(1, NCH):
        slotoff[c] = slotoff[c - 1] + T[c - 1] * 128
    total_slots = slotoff[-1] + T[-1] * 128
    variants = sorted({cap2[c][b] for c in range(NCH) for b in range(CH)})
    iota_off = {}
    acc = 0
    for v in variants:
        iota_off[v] = acc
        acc += 128 * v
    iota_cols = acc
    bounds = _piece_bounds(cfg)
    pieces = []
    outb = 0
    for p in range(NPIECE):
        c0, c1 = bounds[p], bounds[p + 1]
        rows = (c1 - c0) * CH * 64
        pieces.append(dict(c0=c0, c1=c1, inb=c0 * CH * 64, rows=rows,
                           outb=outb))
        outb += 2 * rows
    return dict(cap2=cap2, T=T, tbase=tbase, colbase=colbase,
                totcols=totcols, slotoff=slotoff, total_slots=total_slots,
                variants=variants, iota_off=iota_off, iota_cols=iota_cols,
                pieces=pieces)


# ---------------------------------------------------------------------------
# Host-side preprocessing
# ---------------------------------------------------------------------------
def _wrap16(flat_i16):
    s = flat_i16.shape[0]
    assert s % 16 == 0
    w = flat_i16.reshape(s // 16, 16).T
    return np.tile(w, (8, 1))


def preprocess(cfg, edge_index):
    cores = []
    for g in range(cfg.G):
        src_g = np.asarray(edge_index[g, 0], np.int64)
        dst_g = np.asarray(edge_index[g, 1], np.int64)
        deg = np.bincount(dst_g, minlength=cfg.NPAD).astype(np.float64) + 1.0
        dinv = (1.0 / np.sqrt(deg)).astype(np.float32)
        for h in range(2):
            lo, hi = h * cfg.HALF, (h + 1) * cfg.HALF
            sel = (dst_g >= lo) & (dst_g < hi)
            s = src_g[sel]
            d = dst_g[sel] - lo
            blk = d >> 7
            dloc = d & 127
            par = s & 1
            prow = s >> 1
            order = np.lexsort((par, blk))
            s, blk, dloc, par, prow = (a[order] for a in
                                       (s, blk, dloc, par, prow))
            key = blk * 2 + par
            counts = np.bincount(key, minlength=cfg.NBH * 2)
            starts = np.concatenate([[0], np.cumsum(counts)[:-1]])
            rank = np.arange(len(s)) - starts[key]
            cores.append({
                "g": g, "h": h, "dinv": dinv, "src": s,
                "blk": blk, "dloc": dloc, "par": par, "prow": prow,
                "rank": rank, "countsE": counts[0::2], "countsO": counts[1::2],
            })
    borders = []
    NCH, CH = cfg.NCHUNK, cfg.CHUNK
    capsE = np.zeros((NCH, CH), np.int64)
    capsO = np.zeros((NCH, CH), np.int64)
    for core in cores:
        tot = core["countsE"] + core["countsO"]
        border = np.argsort(-tot, kind="stable")
        borders.append(border)
        nE = core["countsE"][border].reshape(NCH, CH)
        nO = core["countsO"][border].reshape(NCH, CH)
        capsE = np.maximum(capsE, (nE + 127) // 128)
        capsO = np.maximum(capsO, (nO + 127) // 128)
    capsE = np.maximum(capsE, 1)
    capsO = np.maximum(capsO, 1)
    capsE_t = tuple(tuple(int(x) for x in r) for r in capsE)
    capsO_t = tuple(tuple(int(x) for x in r) for r in capsO)
    return cores, borders, capsE_t, capsO_t


def build_core_arrays(cfg, cores, borders, i, capsE, capsO, lay):
    """idx1/idx2 (wrapped int16) + dstl (bf16) for core i."""
    core = cores[i]
    g = core["g"]
    border = borders[i]
    inv = np.empty(cfg.NBH, np.int64)
    inv[border] = np.arange(cfg.NBH)
    capsE_a = np.asarray(capsE)
    blk, dloc, par, prow, rank = (core[k] for k in
                                  ("blk", "dloc", "par", "prow", "rank"))
    pos = inv[blk]
    c = pos >> 2
    b = pos & 3
    capE_cb = capsE_a[c, b]
    t = np.where(par == 0, rank >> 7, capE_cb + (rank >> 7))
    tbase = np.asarray(lay["tbase"])
    colbase = np.asarray(lay["colbase"])
    slotoff = np.asarray(lay["slotoff"])
    slot = slotoff[c] + (tbase[c, b] + t) * 128 + (rank & 127)
    idx1 = np.zeros(lay["total_slots"], np.int16)
    idx1[slot] = prow.astype(np.int16)
    # idx2: position of src's pair row in the piecewise/sorted t2 layout
    inv_of = []
    for hs in range(2):
        bo = borders[2 * g + hs]
        io = np.empty(cfg.NBH, np.int64)
        io[bo] = np.arange(cfg.NBH)
        inv_of.append(io)
    s = core["src"]
    hs = s // cfg.HALF
    srel = s - hs * cfg.HALF
    nb_nat = srel >> 7
    pos_o = np.where(hs == 0, inv_of[0][nb_nat], inv_of[1][nb_nat])
    c2 = pos_o >> 2
    rowo = pos_o * 64 + ((srel & 127) >> 1)
    bounds = _piece_bounds(cfg)
    piece_of_chunk = np.zeros(cfg.NCHUNK, np.int64)
    for p in range(NPIECE):
        piece_of_chunk[bounds[p]: bounds[p + 1]] = p
    pieces = lay["pieces"]
    inb = np.array([pc["inb"] for pc in pieces])
    rows = np.array([pc["rows"] for pc in pieces])
    outb = np.array([pc["outb"] for pc in pieces])
    pc = piece_of_chunk[c2]
    row2 = outb[pc] + hs * rows[pc] + (rowo - inb[pc])
    idx2 = np.zeros(lay["total_slots"], np.int16)
    idx2[slot] = row2.astype(np.int16)
    dstl = np.full((128, lay["totcols"]), -1.0, np.float32)
    dstl[rank & 127, colbase[c, b] + t] = dloc
    return _wrap16(idx1), _wrap16(idx2), dstl.astype(BF_NP)


def _iota_arr(lay):
    cols = np.empty(lay["iota_cols"], np.float32)
    for v in lay["variants"]:
        o = lay["iota_off"][v]
        cols[o: o + 128 * v] = np.repeat(np.arange(128, dtype=np.float32), v)
    return np.tile(cols, (128, 1)).astype(BF_NP)


# ---------------------------------------------------------------------------
# Device kernel
# ---------------------------------------------------------------------------
def build_kernel(cfg, capsE, capsO):
    lay = derive_layout(cfg, capsE, capsO)
    NCH, CH = cfg.NCHUNK, cfg.CHUNK
    cap2, T, tbase, colbase = (lay[k] for k in
                               ("cap2", "T", "tbase", "colbase"))
    slotoff, totcols = lay["slotoff"], lay["totcols"]
    TOT = lay["total_slots"]
    J16 = TOT // 16
    iota_off, iota_cols = lay["iota_off"], lay["iota_cols"]
    pieces = lay["pieces"]
    bounds = _piece_bounds(cfg)
    nc = bacc.Bacc(target_bir_lowering=False)

    t1p_in = nc.dram_tensor("t1p", [cfg.PAIRS, 128], BF16, kind="ExternalInput")
    t1own_in = nc.dram_tensor("t1own", [128, cfg.NBH * 64], BF16,
                              kind="ExternalInput")
    idx1_in = nc.dram_tensor("idx1", [128, J16], I16, kind="ExternalInput")
    idx2_in = nc.dram_tensor("idx2", [128, J16], I16, kind="ExternalInput")
    dstl_in = nc.dram_tensor("dstl", [128, totcols], BF16,
                             kind="ExternalInput")
    iota_in = nc.dram_tensor("iota", [128, iota_cols], BF16,
                             kind="ExternalInput")
    dinv_in = nc.dram_tensor("dinv", [128, cfg.NBH], F32, kind="ExternalInput")
    w1_in = nc.dram_tensor("w1", [64, 128], F32, kind="ExternalInput")
    b1_in = nc.dram_tensor("b1", [128, 1], F32, kind="ExternalInput")
    b1p1_in = nc.dram_tensor("b1p1", [128, 1], F32, kind="ExternalInput")
    w2_in = nc.dram_tensor("w2", [128, 64], F32, kind="ExternalInput")
    b2b_in = nc.dram_tensor("b2b", [128, CH * 64], F32, kind="ExternalInput")
    oh_out = nc.dram_tensor("oh", [cfg.HALF, 64], F32, kind="ExternalOutput")
    t2pin = nc.dram_tensor("t2pin", [cfg.PHALF, 128], BF16)
    t2pout = nc.dram_tensor("t2pout", [cfg.PAIRS, 128], BF16)

    with (
        nc.sbuf_tensor("dstl_sb", [128, totcols], BF16) as dstl_sb,
        nc.sbuf_tensor("iota_sb", [128, iota_cols], BF16) as iota_sb,
        nc.sbuf_tensor("dinv_sb", [128, cfg.NBH], F32) as dinv_sb,
        nc.sbuf_tensor("t1own_sb", [128, cfg.NBH * 64], BF16) as t1own_sb,
        nc.sbuf_tensor("t2own_sb", [128, cfg.NBH * 64], BF16) as t2own_sb,
        nc.sbuf_tensor("identb", [128, 128], BF16) as identb,
        nc.sbuf_tensor("w1bf", [64, 128], BF16) as w1bf,
        nc.sbuf_tensor("w2bf", [128, 64], BF16) as w2bf,
        nc.sbuf_tensor("b1sb", [128, 1], F32) as b1sb,
        nc.sbuf_tensor("b1p1sb", [128, 1], F32) as b1p1sb,
        nc.sbuf_tensor("b2sb", [128, CH * 64], F32) as b2sb,
        nc.semaphore("wsem") as wsem,
        nc.semaphore("ccsem") as ccsem,
    ):
        with TileContext(nc) as tc:
            with tc.tile_pool(name="pre", bufs=2) as pre:
                make_identity(nc, identb[:])
                nc.sync.dma_start(out=dstl_sb[:], in_=dstl_in[:])
                nc.sync.dma_start(out=iota_sb[:], in_=iota_in[:])
                nc.sync.dma_start(out=dinv_sb[:], in_=dinv_in[:])
                nc.sync.dma_start(out=t1own_sb[:], in_=t1own_in[:])
                nc.sync.dma_start(out=b1sb[:], in_=b1_in[:])
                nc.sync.dma_start(out=b1p1sb[:], in_=b1p1_in[:])
                nc.sync.dma_start(out=b2sb[:], in_=b2b_in[:])
                wt = pre.tile([64, 128], F32, tag="w1")
                nc.sync.dma_start(out=wt[:], in_=w1_in[:])
                nc.vector.tensor_copy(out=w1bf[:], in_=wt[:])
                wt2 = pre.tile([128, 64], F32, tag="w2")
                nc.sync.dma_start(out=wt2[:], in_=w2_in[:])
                nc.vector.tensor_copy(out=w2bf[:], in_=wt2[:])

        from contextlib import ExitStack
        with TileContext(nc) as tc:
            with ExitStack() as stack:
                idxp = stack.enter_context(tc.tile_pool(name="idxp", bufs=3))
                msgp = stack.enter_context(tc.tile_pool(name="msgp", bufs=2))
                spool = stack.enter_context(tc.tile_pool(name="sp", bufs=3))
                aggpool = stack.enter_context(
                    tc.tile_pool(name="aggp", bufs=2, space="PSUM"))
                tppool = stack.enter_context(
                    tc.tile_pool(name="tpp", bufs=1, space="PSUM"))
                h1pool = stack.enter_context(
                    tc.tile_pool(name="h1p", bufs=2, space="PSUM"))
                zpool = stack.enter_context(
                    tc.tile_pool(name="zpp", bufs=1, space="PSUM"))
                t2ppool = stack.enter_context(
                    tc.tile_pool(name="t2pp", bufs=2, space="PSUM"))
                finp = stack.enter_context(tc.tile_pool(name="fin", bufs=2))
                finp2 = stack.enter_context(tc.tile_pool(name="fin2", bufs=2))
                stgp = stack.enter_context(tc.tile_pool(name="stg", bufs=2))

                regs = {}
                for v in sorted(set(T)):
                    regs[v] = nc.gpsimd.to_reg(v * 128)

                def chunk_agg(c, table, idx_dram):
                    Tc = T[c]
                    idx_t = idxp.tile([128, Tc * 128 // 16], I16)
                    nc.sync.dma_start(
                        out=idx_t[:],
                        in_=idx_dram[:, slotoff[c] // 16:
                                     slotoff[c] // 16 + Tc * 128 // 16])
                    msg = msgp.tile([128, Tc * 128], BF16)
                    nc.gpsimd.dma_gather(
                        out_ap=msg[:].rearrange("p (t e) -> p t e", e=128),
                        in_ap=table[0: cfg.PAIRS, :],
                        idxs_ap=idx_t[:],
                        num_idxs=Tc * 128,
                        num_idxs_reg=regs[Tc],
                        elem_size=128,
                        single_packet=False,
                    )
                    aggP = aggpool.tile([128, CH * 64], F32)
                    for b in range(CH):
                        k2 = cap2[c][b]
                        kE = capsE[c][b]
                        io = iota_off[k2]
                        S = spool.tile([128, 128 * k2], BF16)
                        Sv = S[:].rearrange("p (v t) -> p v t", t=k2)
                        nc.vector.tensor_tensor(
                            out=Sv,
                            in0=iota_sb[:, io: io + 128 * k2]
                                .rearrange("p (v t) -> p v t", t=k2),
                            in1=dstl_sb[:, colbase[c][b]: colbase[c][b] + k2]
                                .to_broadcast([128, k2, 128])
                                .rearrange("p t v -> p v t"),
                            op=ALU.is_equal,
                        )
                        for t in range(k2):
                            j = tbase[c][b] + t
                            off = 0 if t < kE else 64
                            nc.tensor.matmul(
                                out=aggP[:, b * 64: (b + 1) * 64],
                                lhsT=Sv[:, :, t],
                                rhs=msg[:, j * 128 + off: j * 128 + off + 64],
                                start=(t == 0),
                                stop=(t == k2 - 1),
                            )
                    return aggP

                def finish_l1(c, aggP):
                    # u1 = agg + t1 (f32), scale by dinv in f32 via ACT,
                    # single bf16 rounding at aggV before the transposes
                    u1 = finp.tile([128, CH * 64], F32, tag="u1")
                    nc.vector.tensor_tensor(
                        out=u1[:], in0=aggP[:],
                        in1=t1own_sb[:, c * CH * 64: (c + 1) * CH * 64],
                        op=ALU.add)
                    aggV = finp.tile([128, CH * 64], BF16, tag="aggV")
                    tP = tppool.tile([64, CH * 128], BF16, tag="tp")
                    for b in range(CH):
                        gb = c * CH + b
                        nc.scalar.activation(
                            aggV[:, b * 64: (b + 1) * 64],
                            u1[:, b * 64: (b + 1) * 64],
                            AF.Copy, scale=dinv_sb[:, gb: gb + 1])
                        nc.tensor.transpose(
                            out=tP[:, b * 128: (b + 1) * 128],
                            in_=aggV[:, b * 64: (b + 1) * 64],
                            identity=identb[:])
                    aggT = finp.tile([64, CH * 128], BF16, tag="aggT")
                    nc.scalar.activation(aggT[:], tP[:], AF.Copy)
                    h1P = h1pool.tile([128, CH * 128], F32, tag="h1p")
                    nc.tensor.matmul(out=h1P[:], lhsT=w1bf[:], rhs=aggT[:],
                                     start=True, stop=True)
                    m = finp2.tile([128, CH * 128], F32, tag="m")
                    nc.vector.tensor_scalar(
                        out=m[:], in0=h1P[:], scalar1=b1sb[:, 0:1],
                        scalar2=0.0, op0=ALU.add, op1=ALU.min)
                    x1 = finp2.tile([128, CH * 128], F32, tag="x1")
                    nc.vector.tensor_scalar(
                        out=x1[:], in0=h1P[:], scalar1=b1p1sb[:, 0:1],
                        scalar2=None, op0=ALU.add)
                    ex = finp2.tile([128, CH * 128], F32, tag="ex")
                    nc.scalar.activation(ex[:], m[:], AF.Exp)
                    fmx = finp2.tile([128, CH * 128], F32, tag="fmx")
                    nc.vector.tensor_tensor(out=fmx[:], in0=ex[:], in1=x1[:],
                                            op=ALU.max)
                    h1f = finp2.tile([128, CH * 128], BF16, tag="h1f")
                    nc.vector.tensor_scalar(
                        out=h1f[:], in0=fmx[:], scalar1=-1.0, scalar2=None,
                        op0=ALU.add)
                    zP = zpool.tile([64, CH * 128], F32, tag="zp")
                    nc.tensor.matmul(out=zP[:], lhsT=w2bf[:], rhs=h1f[:],
                                     start=True, stop=True)
                    zsb = finp.tile([64, CH * 128], BF16, tag="zsb")
                    nc.scalar.activation(zsb[:], zP[:], AF.Copy)
                    stage = stgp.tile([128, CH * 64], BF16, tag="stage")
                    t2P = t2ppool.tile([128, CH * 64], BF16, tag="t2P")
                    last_act = None
                    for b in range(CH):
                        gb = c * CH + b
                        nc.tensor.transpose(
                            out=t2P[:, b * 64: (b + 1) * 64],
                            in_=zsb[:, b * 128: (b + 1) * 128],
                            identity=identb[:64, :64])
                        last_act = nc.scalar.activation(
                            stage[:, b * 64: (b + 1) * 64],
                            t2P[:, b * 64: (b + 1) * 64],
                            AF.Copy, scale=dinv_sb[:, gb: gb + 1])
                    cp = nc.vector.tensor_copy(
                        out=t2own_sb[:, c * CH * 64: (c + 1) * CH * 64],
                        in_=stage[:])
                    t2own_copies.append(cp)
                    nc.sync.dma_start(
                        out=t2pin[c * CH * 64: (c + 1) * CH * 64, :]
                        .rearrange("(b q) (r e) -> (q r) b e",
                                   q=64, r=2, e=64),
                        in_=stage[:].rearrange("p (b e) -> p b e", e=64),
                    )

                def emit_piece(p):
                    # no manual sems: the shadow tracker orders the
                    # collective after the t2pin region writes and the L2
                    # gathers after the collective outputs
                    pc = pieces[p]
                    nc.gpsimd.collective_compute(
                        "AllGather", ALU.bypass,
                        replica_groups=[[0, 1], [2, 3], [4, 5], [6, 7]],
                        ins=[t2pin[pc["inb"]: pc["inb"] + pc["rows"], :]
                             .opt()],
                        outs=[t2pout[pc["outb"]:
                                     pc["outb"] + 2 * pc["rows"], :]
                              .opt()],
                    )

                # ---------------- Layer 1 (pieces interleaved) ------------
                t2own_copies = []
                next_piece = 0
                for c in range(NCH):
                    aggP = chunk_agg(c, t1p_in, idx1_in)
                    finish_l1(c, aggP)
                    # emit piece p two chunks after its last input chunk so
                    # the Pool engine has gathers in flight while it waits
                    if (next_piece < NPIECE - 1
                            and c == bounds[next_piece + 1] + 1):
                        emit_piece(next_piece)
                        next_piece += 1
                for p in range(next_piece, NPIECE):
                    emit_piece(p)

                # ---------------- Layer 2 ----------------
                for c in range(NCH):
                    aggP = chunk_agg(c, t2pout, idx2_in)
                    u = finp.tile([128, CH * 64], F32, tag="u")
                    uadd = nc.vector.tensor_tensor(
                        out=u[:], in0=aggP[:],
                        in1=t2own_sb[:, c * CH * 64: (c + 1) * CH * 64],
                        op=ALU.add)
                    # raw-sbuf RAW hazard: order the L2 self-loop read after
                    # the L1 writer of the same t2own region explicitly
                    add_dep_helper(uadd.ins, t2own_copies[c].ins,
                                   reason="L2 self-loop reads t2own chunk")
                    y = finp2.tile([128, CH * 64], F32, tag="y")
                    for b in range(CH):
                        gb = c * CH + b
                        nc.vector.tensor_scalar_mul(
                            y[:, b * 64: (b + 1) * 64],
                            u[:, b * 64: (b + 1) * 64],
                            dinv_sb[:, gb: gb + 1])
                    yb = finp2.tile([128, CH * 64], F32, tag="yb")
                    nc.vector.tensor_tensor(out=yb[:], in0=y[:], in1=b2sb[:],
                                            op=ALU.add)
                    m2 = finp2.tile([128, CH * 64], F32, tag="m2")
                    nc.vector.tensor_scalar(
                        out=m2[:], in0=yb[:], scalar1=0.0, scalar2=None,
                        op0=ALU.min)
                    x12 = finp2.tile([128, CH * 64], F32, tag="x12")
                    nc.vector.tensor_scalar(
                        out=x12[:], in0=yb[:], scalar1=1.0, scalar2=None,
                        op0=ALU.add)
                    e2 = finp2.tile([128, CH * 64], F32, tag="e2")
                    nc.scalar.activation(e2[:], m2[:], AF.Exp)
                    f2 = finp2.tile([128, CH * 64], F32, tag="f2")
                    nc.vector.tensor_tensor(out=f2[:], in0=e2[:], in1=x12[:],
                                            op=ALU.max)
                    stage2 = stgp.tile([128, CH * 64], F32, tag="stage2")
                    nc.vector.tensor_scalar(
                        out=stage2[:], in0=f2[:], scalar1=-1.0, scalar2=None,
                        op0=ALU.add)
                    nc.sync.dma_start(
                        out=oh_out[c * CH * 128: (c + 1) * CH * 128, :]
                        .rearrange("(b p) e -> p b e", p=128),
                        in_=stage2[:].rearrange("p (b e) -> p b e", e=64),
                    )
    nc.finalize()
    return nc


# ---------------------------------------------------------------------------
# Driver
# ---------------------------------------------------------------------------
_NC_CACHE = {}
_PREP_CACHE = {}
LAST_TIMES = {}
_LAST_CAPS = None


def _get_nc(cfg, capsE, capsO):
    key = (cfg.N, cfg.E, capsE, capsO)
    if key not in _NC_CACHE:
        _NC_CACHE[key] = build_kernel(cfg, capsE, capsO)
    return _NC_CACHE[key]


def run(cfg, x, edge_index, W1, b1, W2, b2, spmd_kwargs=None):
    global _LAST_CAPS
    spmd_kwargs = spmd_kwargs or {}
    x = np.asarray(x, np.float32)
    W1 = np.asarray(W1, np.float32)
    b1 = np.asarray(b1, np.float32)
    W2 = np.asarray(W2, np.float32)
    b2 = np.asarray(b2, np.float32)

    import hashlib
    ekey = hashlib.sha1(np.ascontiguousarray(edge_index)).hexdigest()
    if ekey in _PREP_CACHE:
        cores, borders, capsE, capsO, lay, core_arr = _PREP_CACHE[ekey]
    else:
        cores, borders, capsE, capsO = preprocess(cfg, edge_index)
        lay = derive_layout(cfg, capsE, capsO)
        core_arr = [build_core_arrays(cfg, cores, borders, i, capsE, capsO,
                                      lay) for i in range(len(cores))]
        _PREP_CACHE[ekey] = (cores, borders, capsE, capsO, lay, core_arr)
    _LAST_CAPS = (capsE, capsO)
    nc = _get_nc(cfg, capsE, capsO)
    iota = _iota_arr(lay)

    in_maps = []
    for i, core in enumerate(cores):
        g, h = core["g"], core["h"]
        border = borders[i]
        idx1w, idx2w, dstl = core_arr[i]
        dinv = core["dinv"]
        t1 = np.zeros((cfg.NPAD, 64), np.float32)
        t1[: cfg.N] = x[g]
        t1 *= dinv[:, None]
        t1p = np.ascontiguousarray(t1.reshape(cfg.PAIRS, 128)).astype(BF_NP)
        lo = h * cfg.HALF
        t1h = t1[lo: lo + cfg.HALF].reshape(cfg.NBH, 128, 64)
        t1own = np.ascontiguousarray(
            t1h[border].transpose(1, 0, 2).reshape(128, cfg.NBH * 64)
        ).astype(BF_NP)
        dinv_own = np.ascontiguousarray(
            dinv[lo: lo + cfg.HALF].reshape(cfg.NBH, 128)[border].T)
        in_maps.append({
            "t1p": t1p,
            "t1own": t1own,
            "idx1": idx1w,
            "idx2": idx2w,
            "dstl": np.ascontiguousarray(dstl),
            "iota": iota,
            "dinv": dinv_own,
            "w1": np.ascontiguousarray(W1[g]),
            "b1": np.ascontiguousarray(b1[g].reshape(128, 1)),
            "b1p1": np.ascontiguousarray(b1[g].reshape(128, 1) + 1.0),
            "w2": np.ascontiguousarray(W2[g]),
            "b2b": np.ascontiguousarray(
                np.tile(b2[g], (128, cfg.CHUNK)).astype(np.float32)),
        })
    import time as _time
    _t = _time.monotonic()
    res = run_bass_kernel_spmd(nc, in_maps, core_ids=list(range(8)),
                               **spmd_kwargs)
    LAST_TIMES["launch_wall_s"] = _time.monotonic() - _t

    out = np.empty((cfg.G * cfg.N, 64), np.float32)
    for g in range(cfg.G):
        for h in range(2):
            i = 2 * g + h
            oh = res.results[i]["oh"].reshape(cfg.NBH, 128, 64)
            inv = np.empty(cfg.NBH, np.int64)
            inv[borders[i]] = np.arange(cfg.NBH)
            nat = oh[inv].reshape(cfg.HALF, 64)
            lo = g * cfg.N + h * cfg.HALF
            n_rows = min(cfg.HALF, cfg.N - h * cfg.HALF)
            out[lo: lo + n_rows] = nat[:n_rows]
    return out, res


def kernel(x, edge_index, W1, b1, W2, b2):
    out, _ = run(CFG, x, edge_index, W1, b1, W2, b2)
    return out


# revision 3
# speedup vs baseline: 1.0494x; 1.0494x over previous
"""Multi-graph 2-layer GCN on 8 Trainium2 NeuronCores — fused single launch, v3.

v3 over v2:
- Per-core dst blocks are sorted by edge count and packed into chunk
  positions with per-position tile caps (max over cores), cutting gather
  padding from ~19% to ~3%.
- The t2-half AllGather is split into 4 pieces interleaved into the L1
  chunk stream so most of the exchange hides behind L1 gathers.
- Layer-2 gathers use a second index table (idx2) addressing the
  piecewise/sorted t2 table layout; layer-1 indices stay natural.

See kernel_v2 docstring for the base design (bf16 pair-row gather tables,
parity tiles, one-hot S' matmuls with packed-bf16 DVE builds, dinv folded
into PE transposes, ELU = max(x+1, exp(min(x,0))) - 1).
"""

import sys

try:
    import concourse.bass as bass  # noqa: F401
except ImportError:
    sys.path.insert(0, "/opt/trn_rl_repo")
    import concourse.bass as bass

import numpy as np
import ml_dtypes

import concourse.tile as tile_mod  # noqa: F401
from concourse import bacc
import concourse.mybir as mybir
from concourse.bass_utils import run_bass_kernel_spmd
from concourse.tile import TileContext, add_dep_helper
from concourse.masks import make_identity

AF = mybir.ActivationFunctionType
ALU = mybir.AluOpType
F32 = mybir.dt.float32
BF16 = mybir.dt.bfloat16
I16 = mybir.dt.int16

BF_NP = ml_dtypes.bfloat16


def _patched_drain_and_barrier(self, tick_clock, wait_clock):
    from bass_rust import ScopedClock

    probe = self.nc.sync.nop()
    wait_clock.add_sem_waits(probe.ins, ScopedClock({None: tick_clock.global_clock}))
    si = probe.ins.sync_info
    waits = list(si.on_wait) if si and si.on_wait else []
    if si is not None:
        si.on_wait = waits[:1]
    for w in waits[1:]:
        n = self.nc.sync.nop()
        nsi = n.ins.sync_info
        if nsi is None:
            n.ins.sync_info = mybir.SyncInfo(on_wait=[w], on_update=[])
        else:
            nsi.on_wait = [w]
    self.nc.sync.drain()
    self.nc.all_engine_barrier()
    popped = self.nc._tile_sem_poison_stack.pop()
    assert popped is self._sem_poison
    self.nc.clear_and_free_semaphores(list(self.sems.allocated().values()))
    self.nc.all_engine_barrier()


TileContext._drain_and_barrier = _patched_drain_and_barrier

_orig_add_instruction = TileContext._add_instruction
_waitsplit_counter = [0]


def _patched_add_instruction(self, inst):
    """walrus rejects instructions carrying >1 sem wait; hoist excess waits
    onto same-engine nops inserted immediately before the instruction."""
    si = inst.sync_info
    if (si is not None and si.on_wait and len(si.on_wait) > 1
            and inst.engine != mybir.EngineType.Unassigned):
        waits = list(si.on_wait)
        si.on_wait = waits[-1:]
        for w in waits[:-1]:
            _waitsplit_counter[0] += 1
            nop = mybir.InstNoOp(
                name=f"I-wsplit-{_waitsplit_counter[0]}", ins=[], outs=[])
            nop.engine = inst.engine
            nop.sync_info = mybir.SyncInfo(on_wait=[w], on_update=[])
            _orig_add_instruction(self, nop)
    _orig_add_instruction(self, inst)


TileContext._add_instruction = _patched_add_instruction


# ---------------------------------------------------------------------------
# Config
# ---------------------------------------------------------------------------
class Cfg:
    def __init__(self, G, N, E, F_IN, HID, OUT, chunk=4):
        self.G, self.N, self.E = G, N, E
        self.F_IN, self.HID, self.OUT = F_IN, HID, OUT
        assert F_IN == OUT == 64 and HID == 128
        self.NB = (N + 255) // 256 * 2
        self.NPAD = self.NB * 128
        self.NBH = self.NB // 2
        self.HALF = self.NBH * 128
        self.PAIRS = self.NPAD // 2
        self.PHALF = self.HALF // 2
        self.CHUNK = chunk
        assert self.NBH % chunk == 0
        self.NCHUNK = self.NBH // chunk


CFG = Cfg(G=4, N=50000, E=800000, F_IN=64, HID=128, OUT=64, chunk=4)
NPIECE = 4


def _piece_bounds(cfg):
    n = cfg.NCHUNK
    step = n // NPIECE
    return [p * step for p in range(NPIECE)] + [n]


# ---------------------------------------------------------------------------
# Layout derivation shared by host packing and device program
# ---------------------------------------------------------------------------
def derive_layout(cfg, capsE, capsO, capsM):
    """caps*: [NCHUNK][CHUNK] ints (full-E, full-O, mixed-tail tiles).
    Gather tiles per block: E+O+M; S/dstl logical columns: E+O+2M (each
    mixed tile is swept twice, with left and right rhs halves)."""
    NCH, CH = cfg.NCHUNK, cfg.CHUNK
    cap2 = [[capsE[c][b] + capsO[c][b] + capsM[c][b] for b in range(CH)]
            for c in range(NCH)]
    scols = [[capsE[c][b] + capsO[c][b] + 2 * capsM[c][b] for b in range(CH)]
             for c in range(NCH)]
    T = [sum(cap2[c]) for c in range(NCH)]
    tbase = [[0] * CH for _ in range(NCH)]
    for c in range(NCH):
        for b in range(1, CH):
            tbase[c][b] = tbase[c][b - 1] + cap2[c][b - 1]
    colbase = [[0] * CH for _ in range(NCH)]
    acc = 0
    for c in range(NCH):
        for b in range(CH):
            colbase[c][b] = acc
            acc += scols[c][b]
    totcols = acc
    slotoff = [0] * NCH
    for c in range(1, NCH):
        slotoff[c] = slotoff[c - 1] + T[c - 1] * 128
    total_slots = slotoff[-1] + T[-1] * 128
    variants = sorted({scols[c][b] for c in range(NCH) for b in range(CH)})
    iota_off = {}
    acc = 0
    for v in variants:
        iota_off[v] = acc
        acc += 128 * v
    iota_cols = acc
    bounds = _piece_bounds(cfg)
    pieces = []
    outb = 0
    for p in range(NPIECE):
        c0, c1 = bounds[p], bounds[p + 1]
        rows = (c1 - c0) * CH * 64
        pieces.append(dict(c0=c0, c1=c1, inb=c0 * CH * 64, rows=rows,
                           outb=outb))
        outb += 2 * rows
    return dict(cap2=cap2, scols=scols, T=T, tbase=tbase, colbase=colbase,
                totcols=totcols, slotoff=slotoff, total_slots=total_slots,
                variants=variants, iota_off=iota_off, iota_cols=iota_cols,
                pieces=pieces)


# ---------------------------------------------------------------------------
# Host-side preprocessing
# ---------------------------------------------------------------------------
def _wrap16(flat_i16):
    s = flat_i16.shape[0]
    assert s % 16 == 0
    w = flat_i16.reshape(s // 16, 16).T
    return np.tile(w, (8, 1))


def preprocess(cfg, edge_index):
    cores = []
    for g in range(cfg.G):
        src_g = np.asarray(edge_index[g, 0], np.int64)
        dst_g = np.asarray(edge_index[g, 1], np.int64)
        deg = np.bincount(dst_g, minlength=cfg.NPAD).astype(np.float64) + 1.0
        dinv = (1.0 / np.sqrt(deg)).astype(np.float32)
        for h in range(2):
            lo, hi = h * cfg.HALF, (h + 1) * cfg.HALF
            sel = (dst_g >= lo) & (dst_g < hi)
            s = src_g[sel]
            d = dst_g[sel] - lo
            blk = d >> 7
            dloc = d & 127
            par = s & 1
            prow = s >> 1
            order = np.lexsort((par, blk))
            s, blk, dloc, par, prow = (a[order] for a in
                                       (s, blk, dloc, par, prow))
            key = blk * 2 + par
            counts = np.bincount(key, minlength=cfg.NBH * 2)
            starts = np.concatenate([[0], np.cumsum(counts)[:-1]])
            rank = np.arange(len(s)) - starts[key]
            cores.append({
                "g": g, "h": h, "dinv": dinv, "src": s,
                "blk": blk, "dloc": dloc, "par": par, "prow": prow,
                "rank": rank, "countsE": counts[0::2], "countsO": counts[1::2],
            })
    borders = []
    NCH, CH = cfg.NCHUNK, cfg.CHUNK
    capsE = np.zeros((NCH, CH), np.int64)   # full E tiles (floor-based)
    capsO = np.zeros((NCH, CH), np.int64)
    nEs, nOs = [], []
    for core in cores:
        tot = core["countsE"] + core["countsO"]
        border = np.argsort(-tot, kind="stable")
        borders.append(border)
        nE = core["countsE"][border].reshape(NCH, CH)
        nO = core["countsO"][border].reshape(NCH, CH)
        nEs.append(nE)
        nOs.append(nO)
        capsE = np.maximum(capsE, nE // 128)
        capsO = np.maximum(capsO, nO // 128)
    # mixed-tail tiles: each core's E/O overflow beyond the full tiles
    # shares per-position mixed tiles (one gather tile, two matmul passes)
    capsM = np.zeros((NCH, CH), np.int64)
    for nE, nO in zip(nEs, nOs):
        tails = (np.maximum(0, nE - 128 * capsE)
                 + np.maximum(0, nO - 128 * capsO))
        capsM = np.maximum(capsM, (tails + 127) // 128)
    capsE_t = tuple(tuple(int(x) for x in r) for r in capsE)
    capsO_t = tuple(tuple(int(x) for x in r) for r in capsO)
    capsM_t = tuple(tuple(int(x) for x in r) for r in capsM)
    return cores, borders, capsE_t, capsO_t, capsM_t


def build_core_arrays(cfg, cores, borders, i, capsE, capsO, capsM, lay):
    """idx1/idx2 (wrapped int16) + dstl (bf16) for core i."""
    core = cores[i]
    g = core["g"]
    border = borders[i]
    inv = np.empty(cfg.NBH, np.int64)
    inv[border] = np.arange(cfg.NBH)
    capsE_a = np.asarray(capsE)
    capsO_a = np.asarray(capsO)
    capsM_a = np.asarray(capsM)
    blk, dloc, par, prow, rank = (core[k] for k in
                                  ("blk", "dloc", "par", "prow", "rank"))
    pos = inv[blk]
    c = pos >> 2
    b = pos & 3
    capE_cb = capsE_a[c, b]
    capO_cb = capsO_a[c, b]
    capM_cb = capsM_a[c, b]
    # full-tile edges vs mixed-tail edges
    full = np.where(par == 0, rank < 128 * capE_cb, rank < 128 * capO_cb)
    # per-(block) count of E-tail edges (to place O-tails after E-tails)
    nE = core["countsE"][border].reshape(-1)[pos]          # nE of own block
    tailE_cnt = np.maximum(0, nE - 128 * capE_cb)
    tail_idx = np.where(par == 0, rank - 128 * capE_cb,
                        tailE_cnt + rank - 128 * capO_cb)
    # gather tile within block and slot row
    gt = np.where(full,
                  np.where(par == 0, rank >> 7, capE_cb + (rank >> 7)),
                  capE_cb + capO_cb + (tail_idx >> 7))
    srow = np.where(full, rank & 127, tail_idx & 127)
    # S/dstl logical column within block
    scol = np.where(full,
                    np.where(par == 0, rank >> 7, capE_cb + (rank >> 7)),
                    capE_cb + capO_cb + (tail_idx >> 7)
                    + np.where(par == 0, 0, capM_cb))
    tbase = np.asarray(lay["tbase"])
    colbase = np.asarray(lay["colbase"])
    slotoff = np.asarray(lay["slotoff"])
    slot = slotoff[c] + (tbase[c, b] + gt) * 128 + srow
    idx1 = np.zeros(lay["total_slots"], np.int16)
    idx1[slot] = prow.astype(np.int16)
    # idx2: position of src's pair row in the piecewise/sorted t2 layout
    inv_of = []
    for hs in range(2):
        bo = borders[2 * g + hs]
        io = np.empty(cfg.NBH, np.int64)
        io[bo] = np.arange(cfg.NBH)
        inv_of.append(io)
    s = core["src"]
    hs = s // cfg.HALF
    srel = s - hs * cfg.HALF
    nb_nat = srel >> 7
    pos_o = np.where(hs == 0, inv_of[0][nb_nat], inv_of[1][nb_nat])
    c2 = pos_o >> 2
    rowo = pos_o * 64 + ((srel & 127) >> 1)
    bounds = _piece_bounds(cfg)
    piece_of_chunk = np.zeros(cfg.NCHUNK, np.int64)
    for p in range(NPIECE):
        piece_of_chunk[bounds[p]: bounds[p + 1]] = p
    pieces = lay["pieces"]
    inb = np.array([pc["inb"] for pc in pieces])
    rows = np.array([pc["rows"] for pc in pieces])
    outb = np.array([pc["outb"] for pc in pieces])
    pc = piece_of_chunk[c2]
    row2 = outb[pc] + hs * rows[pc] + (rowo - inb[pc])
    idx2 = np.zeros(lay["total_slots"], np.int16)
    idx2[slot] = row2.astype(np.int16)
    dstl = np.full((128, lay["totcols"]), -1.0, np.float32)
    dstl[srow, colbase[c, b] + scol] = dloc
    return _wrap16(idx1), _wrap16(idx2), dstl.astype(BF_NP)


def _iota_arr(lay):
    cols = np.empty(lay["iota_cols"], np.float32)
    for v in lay["variants"]:
        o = lay["iota_off"][v]
        cols[o: o + 128 * v] = np.repeat(np.arange(128, dtype=np.float32), v)
    return np.tile(cols, (128, 1)).astype(BF_NP)


# ---------------------------------------------------------------------------
# Device kernel
# ---------------------------------------------------------------------------
def build_kernel(cfg, capsE, capsO, capsM):
    lay = derive_layout(cfg, capsE, capsO, capsM)
    NCH, CH = cfg.NCHUNK, cfg.CHUNK
    cap2, scols, T, tbase, colbase = (lay[k] for k in
                                      ("cap2", "scols", "T", "tbase",
                                       "colbase"))
    slotoff, totcols = lay["slotoff"], lay["totcols"]
    TOT = lay["total_slots"]
    J16 = TOT // 16
    iota_off, iota_cols = lay["iota_off"], lay["iota_cols"]
    pieces = lay["pieces"]
    bounds = _piece_bounds(cfg)
    nc = bacc.Bacc(target_bir_lowering=False)

    t1p_in = nc.dram_tensor("t1p", [cfg.PAIRS, 128], BF16, kind="ExternalInput")
    t1own_in = nc.dram_tensor("t1own", [128, cfg.NBH * 64], BF16,
                              kind="ExternalInput")
    idx1_in = nc.dram_tensor("idx1", [128, J16], I16, kind="ExternalInput")
    idx2_in = nc.dram_tensor("idx2", [128, J16], I16, kind="ExternalInput")
    dstl_in = nc.dram_tensor("dstl", [128, totcols], BF16,
                             kind="ExternalInput")
    iota_in = nc.dram_tensor("iota", [128, iota_cols], BF16,
                             kind="ExternalInput")
    dinv_in = nc.dram_tensor("dinv", [128, cfg.NBH], F32, kind="ExternalInput")
    w1_in = nc.dram_tensor("w1", [64, 128], F32, kind="ExternalInput")
    b1_in = nc.dram_tensor("b1", [128, 1], F32, kind="ExternalInput")
    b1p1_in = nc.dram_tensor("b1p1", [128, 1], F32, kind="ExternalInput")
    w2_in = nc.dram_tensor("w2", [128, 64], F32, kind="ExternalInput")
    b2b_in = nc.dram_tensor("b2b", [128, CH * 64], F32, kind="ExternalInput")
    oh_out = nc.dram_tensor("oh", [cfg.HALF, 64], F32, kind="ExternalOutput")
    t2pin = nc.dram_tensor("t2pin", [cfg.PHALF, 128], BF16)
    t2pout = nc.dram_tensor("t2pout", [cfg.PAIRS, 128], BF16)

    with (
        nc.sbuf_tensor("dstl_sb", [128, totcols], BF16) as dstl_sb,
        nc.sbuf_tensor("iota_sb", [128, iota_cols], BF16) as iota_sb,
        nc.sbuf_tensor("dinv_sb", [128, cfg.NBH], F32) as dinv_sb,
        nc.sbuf_tensor("t1own_sb", [128, cfg.NBH * 64], BF16) as t1own_sb,
        nc.sbuf_tensor("t2own_sb", [128, cfg.NBH * 64], BF16) as t2own_sb,
        nc.sbuf_tensor("identb", [128, 128], BF16) as identb,
        nc.sbuf_tensor("w1bf", [64, 128], BF16) as w1bf,
        nc.sbuf_tensor("w2bf", [128, 64], BF16) as w2bf,
        nc.sbuf_tensor("b1sb", [128, 1], F32) as b1sb,
        nc.sbuf_tensor("b1p1sb", [128, 1], F32) as b1p1sb,
        nc.sbuf_tensor("b2sb", [128, CH * 64], F32) as b2sb,
        nc.semaphore("wsem") as wsem,
        nc.semaphore("ccsem") as ccsem,
    ):
        with TileContext(nc) as tc:
            with tc.tile_pool(name="pre", bufs=2) as pre:
                make_identity(nc, identb[:])
                nc.sync.dma_start(out=dstl_sb[:], in_=dstl_in[:])
                nc.sync.dma_start(out=iota_sb[:], in_=iota_in[:])
                nc.sync.dma_start(out=dinv_sb[:], in_=dinv_in[:])
                nc.sync.dma_start(out=t1own_sb[:], in_=t1own_in[:])
                nc.sync.dma_start(out=b1sb[:], in_=b1_in[:])
                nc.sync.dma_start(out=b1p1sb[:], in_=b1p1_in[:])
                nc.sync.dma_start(out=b2sb[:], in_=b2b_in[:])
                wt = pre.tile([64, 128], F32, tag="w1")
                nc.sync.dma_start(out=wt[:], in_=w1_in[:])
                nc.vector.tensor_copy(out=w1bf[:], in_=wt[:])
                wt2 = pre.tile([128, 64], F32, tag="w2")
                nc.sync.dma_start(out=wt2[:], in_=w2_in[:])
                nc.vector.tensor_copy(out=w2bf[:], in_=wt2[:])

        from contextlib import ExitStack
        with TileContext(nc) as tc:
            with ExitStack() as stack:
                idxp = stack.enter_context(tc.tile_pool(name="idxp", bufs=3))
                msgp = stack.enter_context(tc.tile_pool(name="msgp", bufs=2))
                spool = stack.enter_context(tc.tile_pool(name="sp", bufs=3))
                aggpool = stack.enter_context(
                    tc.tile_pool(name="aggp", bufs=2, space="PSUM"))
                tppool = stack.enter_context(
                    tc.tile_pool(name="tpp", bufs=1, space="PSUM"))
                h1pool = stack.enter_context(
                    tc.tile_pool(name="h1p", bufs=2, space="PSUM"))
                zpool = stack.enter_context(
                    tc.tile_pool(name="zpp", bufs=1, space="PSUM"))
                t2ppool = stack.enter_context(
                    tc.tile_pool(name="t2pp", bufs=2, space="PSUM"))
                finp = stack.enter_context(tc.tile_pool(name="fin", bufs=2))
                finp2 = stack.enter_context(tc.tile_pool(name="fin2", bufs=2))
                stgp = stack.enter_context(tc.tile_pool(name="stg", bufs=2))

                regs = {}
                for v in sorted(set(T)):
                    regs[v] = nc.gpsimd.to_reg(v * 128)

                def chunk_agg(c, table, idx_dram):
                    Tc = T[c]
                    idx_t = idxp.tile([128, Tc * 128 // 16], I16)
                    nc.sync.dma_start(
                        out=idx_t[:],
                        in_=idx_dram[:, slotoff[c] // 16:
                                     slotoff[c] // 16 + Tc * 128 // 16])
                    msg = msgp.tile([128, Tc * 128], BF16)
                    nc.gpsimd.dma_gather(
                        out_ap=msg[:].rearrange("p (t e) -> p t e", e=128),
                        in_ap=table[0: cfg.PAIRS, :],
                        idxs_ap=idx_t[:],
                        num_idxs=Tc * 128,
                        num_idxs_reg=regs[Tc],
                        elem_size=128,
                        single_packet=False,
                    )
                    aggP = aggpool.tile([128, CH * 64], F32)
                    for b in range(CH):
                        ks = scols[c][b]
                        kE = capsE[c][b]
                        kO = capsO[c][b]
                        kM = capsM[c][b]
                        io = iota_off[ks]
                        S = spool.tile([128, 128 * ks], BF16)
                        Sv = S[:].rearrange("p (v t) -> p v t", t=ks)
                        nc.vector.tensor_tensor(
                            out=Sv,
                            in0=iota_sb[:, io: io + 128 * ks]
                                .rearrange("p (v t) -> p v t", t=ks),
                            in1=dstl_sb[:, colbase[c][b]: colbase[c][b] + ks]
                                .to_broadcast([128, ks, 128])
                                .rearrange("p t v -> p v t"),
                            op=ALU.is_equal,
                        )
                        for ln in range(ks):
                            # logical col -> (gather tile, rhs half): full E,
                            # full O, mixed L-pass, mixed R-pass
                            if ln < kE + kO + kM:
                                gt = ln
                                off = 0 if (ln < kE or ln >= kE + kO) else 64
                            else:
                                gt = ln - kM
                                off = 64
                            j = tbase[c][b] + gt
                            nc.tensor.matmul(
                                out=aggP[:, b * 64: (b + 1) * 64],
                                lhsT=Sv[:, :, ln],
                                rhs=msg[:, j * 128 + off: j * 128 + off + 64],
                                start=(ln == 0),
                                stop=(ln == ks - 1),
                            )
                    return aggP

                def finish_l1(c, aggP):
                    # u1 = agg + t1 (f32), scale by dinv in f32 via ACT,
                    # single bf16 rounding at aggV before the transposes
                    u1 = finp.tile([128, CH * 64], F32, tag="u1")
                    nc.vector.tensor_tensor(
                        out=u1[:], in0=aggP[:],
                        in1=t1own_sb[:, c * CH * 64: (c + 1) * CH * 64],
                        op=ALU.add)
                    aggV = finp.tile([128, CH * 64], BF16, tag="aggV")
                    tP = tppool.tile([64, CH * 128], BF16, tag="tp")
                    for b in range(CH):
                        gb = c * CH + b
                        nc.scalar.activation(
                            aggV[:, b * 64: (b + 1) * 64],
                            u1[:, b * 64: (b + 1) * 64],
                            AF.Copy, scale=dinv_sb[:, gb: gb + 1])
                        nc.tensor.transpose(
                            out=tP[:, b * 128: (b + 1) * 128],
                            in_=aggV[:, b * 64: (b + 1) * 64],
                            identity=identb[:])
                    aggT = finp.tile([64, CH * 128], BF16, tag="aggT")
                    nc.scalar.activation(aggT[:], tP[:], AF.Copy)
                    h1P = h1pool.tile([128, CH * 128], F32, tag="h1p")
                    nc.tensor.matmul(out=h1P[:], lhsT=w1bf[:], rhs=aggT[:],
                                     start=True, stop=True)
                    m = finp2.tile([128, CH * 128], F32, tag="m")
                    nc.vector.tensor_scalar(
                        out=m[:], in0=h1P[:], scalar1=b1sb[:, 0:1],
                        scalar2=0.0, op0=ALU.add, op1=ALU.min)
                    x1 = finp2.tile([128, CH * 128], F32, tag="x1")
                    nc.vector.tensor_scalar(
                        out=x1[:], in0=h1P[:], scalar1=b1p1sb[:, 0:1],
                        scalar2=None, op0=ALU.add)
                    ex = finp2.tile([128, CH * 128], F32, tag="ex")
                    nc.scalar.activation(ex[:], m[:], AF.Exp)
                    fmx = finp2.tile([128, CH * 128], F32, tag="fmx")
                    nc.vector.tensor_tensor(out=fmx[:], in0=ex[:], in1=x1[:],
                                            op=ALU.max)
                    h1f = finp2.tile([128, CH * 128], BF16, tag="h1f")
                    nc.vector.tensor_scalar(
                        out=h1f[:], in0=fmx[:], scalar1=-1.0, scalar2=None,
                        op0=ALU.add)
                    zP = zpool.tile([64, CH * 128], F32, tag="zp")
                    nc.tensor.matmul(out=zP[:], lhsT=w2bf[:], rhs=h1f[:],
                                     start=True, stop=True)
                    zsb = finp.tile([64, CH * 128], BF16, tag="zsb")
                    nc.scalar.activation(zsb[:], zP[:], AF.Copy)
                    stage = stgp.tile([128, CH * 64], BF16, tag="stage")
                    t2P = t2ppool.tile([128, CH * 64], BF16, tag="t2P")
                    last_act = None
                    for b in range(CH):
                        gb = c * CH + b
                        nc.tensor.transpose(
                            out=t2P[:, b * 64: (b + 1) * 64],
                            in_=zsb[:, b * 128: (b + 1) * 128],
                            identity=identb[:64, :64])
                        last_act = nc.scalar.activation(
                            stage[:, b * 64: (b + 1) * 64],
                            t2P[:, b * 64: (b + 1) * 64],
                            AF.Copy, scale=dinv_sb[:, gb: gb + 1])
                    cp = nc.vector.tensor_copy(
                        out=t2own_sb[:, c * CH * 64: (c + 1) * CH * 64],
                        in_=stage[:])
                    t2own_copies.append(cp)
                    nc.sync.dma_start(
                        out=t2pin[c * CH * 64: (c + 1) * CH * 64, :]
                        .rearrange("(b q) (r e) -> (q r) b e",
                                   q=64, r=2, e=64),
                        in_=stage[:].rearrange("p (b e) -> p b e", e=64),
                    )

                def emit_piece(p):
                    # no manual sems: the shadow tracker orders the
                    # collective after the t2pin region writes and the L2
                    # gathers after the collective outputs
                    pc = pieces[p]
                    nc.gpsimd.collective_compute(
                        "AllGather", ALU.bypass,
                        replica_groups=[[0, 1], [2, 3], [4, 5], [6, 7]],
                        ins=[t2pin[pc["inb"]: pc["inb"] + pc["rows"], :]
                             .opt()],
                        outs=[t2pout[pc["outb"]:
                                     pc["outb"] + 2 * pc["rows"], :]
                              .opt()],
                    )

                # ---------------- Layer 1 (pieces interleaved) ------------
                t2own_copies = []
                next_piece = 0
                for c in range(NCH):
                    aggP = chunk_agg(c, t1p_in, idx1_in)
                    finish_l1(c, aggP)
                    # emit piece p two chunks after its last input chunk so
                    # the Pool engine has gathers in flight while it waits
                    if (next_piece < NPIECE - 1
                            and c == bounds[next_piece + 1] + 1):
                        emit_piece(next_piece)
                        next_piece += 1
                for p in range(next_piece, NPIECE):
                    emit_piece(p)

                # ---------------- Layer 2 ----------------
                for c in range(NCH):
                    aggP = chunk_agg(c, t2pout, idx2_in)
                    u = finp.tile([128, CH * 64], F32, tag="u")
                    uadd = nc.vector.tensor_tensor(
                        out=u[:], in0=aggP[:],
                        in1=t2own_sb[:, c * CH * 64: (c + 1) * CH * 64],
                        op=ALU.add)
                    # raw-sbuf RAW hazard: order the L2 self-loop read after
                    # the L1 writer of the same t2own region explicitly
                    add_dep_helper(uadd.ins, t2own_copies[c].ins,
                                   reason="L2 self-loop reads t2own chunk")
                    y = finp2.tile([128, CH * 64], F32, tag="y")
                    for b in range(CH):
                        gb = c * CH + b
                        nc.vector.tensor_scalar_mul(
                            y[:, b * 64: (b + 1) * 64],
                            u[:, b * 64: (b + 1) * 64],
                            dinv_sb[:, gb: gb + 1])
                    yb = finp2.tile([128, CH * 64], F32, tag="yb")
                    nc.vector.tensor_tensor(out=yb[:], in0=y[:], in1=b2sb[:],
                                            op=ALU.add)
                    m2 = finp2.tile([128, CH * 64], F32, tag="m2")
                    nc.vector.tensor_scalar(
                        out=m2[:], in0=yb[:], scalar1=0.0, scalar2=None,
                        op0=ALU.min)
                    x12 = finp2.tile([128, CH * 64], F32, tag="x12")
                    nc.vector.tensor_scalar(
                        out=x12[:], in0=yb[:], scalar1=1.0, scalar2=None,
                        op0=ALU.add)
                    e2 = finp2.tile([128, CH * 64], F32, tag="e2")
                    nc.scalar.activation(e2[:], m2[:], AF.Exp)
                    f2 = finp2.tile([128, CH * 64], F32, tag="f2")
                    nc.vector.tensor_tensor(out=f2[:], in0=e2[:], in1=x12[:],
                                            op=ALU.max)
                    stage2 = stgp.tile([128, CH * 64], F32, tag="stage2")
                    nc.vector.tensor_scalar(
                        out=stage2[:], in0=f2[:], scalar1=-1.0, scalar2=None,
                        op0=ALU.add)
                    nc.sync.dma_start(
                        out=oh_out[c * CH * 128: (c + 1) * CH * 128, :]
                        .rearrange("(b p) e -> p b e", p=128),
                        in_=stage2[:].rearrange("p (b e) -> p b e", e=64),
                    )
    nc.finalize()
    return nc


# ---------------------------------------------------------------------------
# Driver
# ---------------------------------------------------------------------------
_NC_CACHE = {}
_PREP_CACHE = {}
LAST_TIMES = {}
_LAST_CAPS = None


def _get_nc(cfg, capsE, capsO, capsM):
    key = (cfg.N, cfg.E, capsE, capsO, capsM)
    if key not in _NC_CACHE:
        _NC_CACHE[key] = build_kernel(cfg, capsE, capsO, capsM)
    return _NC_CACHE[key]


def run(cfg, x, edge_index, W1, b1, W2, b2, spmd_kwargs=None):
    global _LAST_CAPS
    spmd_kwargs = spmd_kwargs or {}
    x = np.asarray(x, np.float32)
    W1 = np.asarray(W1, np.float32)
    b1 = np.asarray(b1, np.float32)
    W2 = np.asarray(W2, np.float32)
    b2 = np.asarray(b2, np.float32)

    import hashlib
    ekey = hashlib.sha1(np.ascontiguousarray(edge_index)).hexdigest()
    if ekey in _PREP_CACHE:
        cores, borders, capsE, capsO, capsM, lay, core_arr = _PREP_CACHE[ekey]
    else:
        cores, borders, capsE, capsO, capsM = preprocess(cfg, edge_index)
        lay = derive_layout(cfg, capsE, capsO, capsM)
        core_arr = [build_core_arrays(cfg, cores, borders, i, capsE, capsO,
                                      capsM, lay) for i in range(len(cores))]
        _PREP_CACHE[ekey] = (cores, borders, capsE, capsO, capsM, lay,
                             core_arr)
    _LAST_CAPS = (capsE, capsO, capsM)
    nc = _get_nc(cfg, capsE, capsO, capsM)
    iota = _iota_arr(lay)

    in_maps = []
    for i, core in enumerate(cores):
        g, h = core["g"], core["h"]
        border = borders[i]
        idx1w, idx2w, dstl = core_arr[i]
        dinv = core["dinv"]
        t1 = np.zeros((cfg.NPAD, 64), np.float32)
        t1[: cfg.N] = x[g]
        t1 *= dinv[:, None]
        t1p = np.ascontiguousarray(t1.reshape(cfg.PAIRS, 128)).astype(BF_NP)
        lo = h * cfg.HALF
        t1h = t1[lo: lo + cfg.HALF].reshape(cfg.NBH, 128, 64)
        t1own = np.ascontiguousarray(
            t1h[border].transpose(1, 0, 2).reshape(128, cfg.NBH * 64)
        ).astype(BF_NP)
        dinv_own = np.ascontiguousarray(
            dinv[lo: lo + cfg.HALF].reshape(cfg.NBH, 128)[border].T)
        in_maps.append({
            "t1p": t1p,
            "t1own": t1own,
            "idx1": idx1w,
            "idx2": idx2w,
            "dstl": np.ascontiguousarray(dstl),
            "iota": iota,
            "dinv": dinv_own,
            "w1": np.ascontiguousarray(W1[g]),
            "b1": np.ascontiguousarray(b1[g].reshape(128, 1)),
            "b1p1": np.ascontiguousarray(b1[g].reshape(128, 1) + 1.0),
            "w2": np.ascontiguousarray(W2[g]),
            "b2b": np.ascontiguousarray(
                np.tile(b2[g], (128, cfg.CHUNK)).astype(np.float32)),
        })
    import time as _time
    _t = _time.monotonic()
    res = run_bass_kernel_spmd(nc, in_maps, core_ids=list(range(8)),
                               **spmd_kwargs)
    LAST_TIMES["launch_wall_s"] = _time.monotonic() - _t

    out = np.empty((cfg.G * cfg.N, 64), np.float32)
    for g in range(cfg.G):
        for h in range(2):
            i = 2 * g + h
            oh = res.results[i]["oh"].reshape(cfg.NBH, 128, 64)
            inv = np.empty(cfg.NBH, np.int64)
            inv[borders[i]] = np.arange(cfg.NBH)
            nat = oh[inv].reshape(cfg.HALF, 64)
            lo = g * cfg.N + h * cfg.HALF
            n_rows = min(cfg.HALF, cfg.N - h * cfg.HALF)
            out[lo: lo + n_rows] = nat[:n_rows]
    return out, res


def kernel(x, edge_index, W1, b1, W2, b2):
    out, _ = run(CFG, x, edge_index, W1, b1, W2, b2)
    return out


# revision 4
# speedup vs baseline: 1.0713x; 1.0209x over previous
"""Multi-graph 2-layer GCN on 8 Trainium2 NeuronCores — fused single launch, v3.

v3 over v2:
- Per-core dst blocks are sorted by edge count and packed into chunk
  positions with per-position tile caps (max over cores), cutting gather
  padding from ~19% to ~3%.
- The t2-half AllGather is split into 4 pieces interleaved into the L1
  chunk stream so most of the exchange hides behind L1 gathers.
- Layer-2 gathers use a second index table (idx2) addressing the
  piecewise/sorted t2 table layout; layer-1 indices stay natural.

See kernel_v2 docstring for the base design (bf16 pair-row gather tables,
parity tiles, one-hot S' matmuls with packed-bf16 DVE builds, dinv folded
into PE transposes, ELU = max(x+1, exp(min(x,0))) - 1).
"""

import sys

try:
    import concourse.bass as bass  # noqa: F401
except ImportError:
    sys.path.insert(0, "/opt/trn_rl_repo")
    import concourse.bass as bass

import numpy as np
import ml_dtypes

import concourse.tile as tile_mod  # noqa: F401
from concourse import bacc
import concourse.mybir as mybir
from concourse.bass_utils import run_bass_kernel_spmd
from concourse.tile import TileContext, add_dep_helper
from concourse.masks import make_identity

AF = mybir.ActivationFunctionType
ALU = mybir.AluOpType
F32 = mybir.dt.float32
BF16 = mybir.dt.bfloat16
I16 = mybir.dt.int16

BF_NP = ml_dtypes.bfloat16


def _patched_drain_and_barrier(self, tick_clock, wait_clock):
    from bass_rust import ScopedClock

    probe = self.nc.sync.nop()
    wait_clock.add_sem_waits(probe.ins, ScopedClock({None: tick_clock.global_clock}))
    si = probe.ins.sync_info
    waits = list(si.on_wait) if si and si.on_wait else []
    if si is not None:
        si.on_wait = waits[:1]
    for w in waits[1:]:
        n = self.nc.sync.nop()
        nsi = n.ins.sync_info
        if nsi is None:
            n.ins.sync_info = mybir.SyncInfo(on_wait=[w], on_update=[])
        else:
            nsi.on_wait = [w]
    self.nc.sync.drain()
    self.nc.all_engine_barrier()
    popped = self.nc._tile_sem_poison_stack.pop()
    assert popped is self._sem_poison
    self.nc.clear_and_free_semaphores(list(self.sems.allocated().values()))
    self.nc.all_engine_barrier()


TileContext._drain_and_barrier = _patched_drain_and_barrier

_orig_add_instruction = TileContext._add_instruction
_waitsplit_counter = [0]


def _patched_add_instruction(self, inst):
    """walrus rejects instructions carrying >1 sem wait; hoist excess waits
    onto same-engine nops inserted immediately before the instruction."""
    si = inst.sync_info
    if (si is not None and si.on_wait and len(si.on_wait) > 1
            and inst.engine != mybir.EngineType.Unassigned):
        waits = list(si.on_wait)
        si.on_wait = waits[-1:]
        for w in waits[:-1]:
            _waitsplit_counter[0] += 1
            nop = mybir.InstNoOp(
                name=f"I-wsplit-{_waitsplit_counter[0]}", ins=[], outs=[])
            nop.engine = inst.engine
            nop.sync_info = mybir.SyncInfo(on_wait=[w], on_update=[])
            _orig_add_instruction(self, nop)
    _orig_add_instruction(self, inst)


TileContext._add_instruction = _patched_add_instruction


# ---------------------------------------------------------------------------
# Config
# ---------------------------------------------------------------------------
class Cfg:
    def __init__(self, G, N, E, F_IN, HID, OUT, chunk=4):
        self.G, self.N, self.E = G, N, E
        self.F_IN, self.HID, self.OUT = F_IN, HID, OUT
        assert F_IN == OUT == 64 and HID == 128
        self.NB = (N + 255) // 256 * 2
        self.NPAD = self.NB * 128
        self.NBH = self.NB // 2
        self.HALF = self.NBH * 128
        self.PAIRS = self.NPAD // 2
        self.PHALF = self.HALF // 2
        self.CHUNK = chunk
        assert self.NBH % chunk == 0
        self.NCHUNK = self.NBH // chunk


CFG = Cfg(G=4, N=50000, E=800000, F_IN=64, HID=128, OUT=64, chunk=4)
NPIECE = 4


def _piece_bounds(cfg):
    n = cfg.NCHUNK
    step = n // NPIECE
    return [p * step for p in range(NPIECE)] + [n]


# ---------------------------------------------------------------------------
# Layout derivation shared by host packing and device program
# ---------------------------------------------------------------------------
def derive_layout(cfg, capsE, capsO, capsM):
    """caps*: [NCHUNK][CHUNK] ints (full-E, full-O, mixed-tail tiles).
    Gather tiles per block: E+O+M; S/dstl logical columns: E+O+2M (each
    mixed tile is swept twice, with left and right rhs halves)."""
    NCH, CH = cfg.NCHUNK, cfg.CHUNK
    cap2 = [[capsE[c][b] + capsO[c][b] + capsM[c][b] for b in range(CH)]
            for c in range(NCH)]
    scols = [[capsE[c][b] + capsO[c][b] + 2 * capsM[c][b] for b in range(CH)]
             for c in range(NCH)]
    T = [sum(cap2[c]) for c in range(NCH)]
    tbase = [[0] * CH for _ in range(NCH)]
    for c in range(NCH):
        for b in range(1, CH):
            tbase[c][b] = tbase[c][b - 1] + cap2[c][b - 1]
    colbase = [[0] * CH for _ in range(NCH)]
    acc = 0
    for c in range(NCH):
        for b in range(CH):
            colbase[c][b] = acc
            acc += scols[c][b]
    totcols = acc
    slotoff = [0] * NCH
    for c in range(1, NCH):
        slotoff[c] = slotoff[c - 1] + T[c - 1] * 128
    total_slots = slotoff[-1] + T[-1] * 128
    variants = sorted({scols[c][b] for c in range(NCH) for b in range(CH)})
    iota_off = {}
    acc = 0
    for v in variants:
        iota_off[v] = acc
        acc += 128 * v
    iota_cols = acc
    bounds = _piece_bounds(cfg)
    pieces = []
    outb = 0
    for p in range(NPIECE):
        c0, c1 = bounds[p], bounds[p + 1]
        rows = (c1 - c0) * CH * 64
        pieces.append(dict(c0=c0, c1=c1, inb=c0 * CH * 64, rows=rows,
                           outb=outb))
        outb += 2 * rows
    return dict(cap2=cap2, scols=scols, T=T, tbase=tbase, colbase=colbase,
                totcols=totcols, slotoff=slotoff, total_slots=total_slots,
                variants=variants, iota_off=iota_off, iota_cols=iota_cols,
                pieces=pieces)


# ---------------------------------------------------------------------------
# Host-side preprocessing
# ---------------------------------------------------------------------------
def _wrap16(flat_i16):
    s = flat_i16.shape[0]
    assert s % 16 == 0
    w = flat_i16.reshape(s // 16, 16).T
    return np.tile(w, (8, 1))


def preprocess(cfg, edge_index):
    cores = []
    for g in range(cfg.G):
        src_g = np.asarray(edge_index[g, 0], np.int64)
        dst_g = np.asarray(edge_index[g, 1], np.int64)
        deg = np.bincount(dst_g, minlength=cfg.NPAD).astype(np.float64) + 1.0
        dinv = (1.0 / np.sqrt(deg)).astype(np.float32)
        for h in range(2):
            lo, hi = h * cfg.HALF, (h + 1) * cfg.HALF
            sel = (dst_g >= lo) & (dst_g < hi)
            s = src_g[sel]
            d = dst_g[sel] - lo
            blk = d >> 7
            dloc = d & 127
            par = s & 1
            prow = s >> 1
            # fuse L/R collisions: an even- and an odd-parity edge of the
            # same block hitting the same pair row share one gather slot
            # (the mixed tiles' dual L/R columns handle the two dsts)
            n_e = len(s)
            order = np.lexsort((par, prow, blk))
            s, blk, dloc, par, prow = (a[order] for a in
                                       (s, blk, dloc, par, prow))
            gkey = blk * cfg.PAIRS + prow
            gid = np.concatenate([[0], np.cumsum(gkey[1:] != gkey[:-1])])
            gcounts = np.bincount(gid)
            gstart = np.concatenate([[0], np.cumsum(gcounts)[:-1]])
            # per-(blk,prow) group: evens come first; j = rank within parity
            idx_in_g = np.arange(n_e) - gstart[gid]
            gp = np.bincount(gid * 2 + par, minlength=2 * (len(gcounts)))
            ne_in_g = gp[0::2][gid]
            no_in_g = gp[1::2][gid]
            j = np.where(par == 0, idx_in_g, idx_in_g - ne_in_g)
            m = np.minimum(ne_in_g, no_in_g)
            fused = j < m
            # fused-pair index within block: k-th fused-even pairs with the
            # k-th fused-odd (identical (blk,prow,j) enumeration order)
            frank = np.zeros(n_e, np.int64)
            for pv in (0, 1):
                sel = fused & (par == pv)
                bsel = blk[sel]
                cnts = np.bincount(bsel, minlength=cfg.NBH)
                st = np.concatenate([[0], np.cumsum(cnts)[:-1]])
                frank[sel] = np.arange(sel.sum()) - st[bsel]
            countsP = np.bincount(blk[fused & (par == 0)], minlength=cfg.NBH)
            # singles re-ranked within (blk, parity)
            rank = np.zeros(n_e, np.int64)
            for pv in (0, 1):
                sel = (~fused) & (par == pv)
                bsel = blk[sel]
                cnts = np.bincount(bsel, minlength=cfg.NBH)
                st = np.concatenate([[0], np.cumsum(cnts)[:-1]])
                rank[sel] = np.arange(sel.sum()) - st[bsel]
                if pv == 0:
                    countsE = cnts
                else:
                    countsO = cnts
            cores.append({
                "g": g, "h": h, "dinv": dinv, "src": s,
                "blk": blk, "dloc": dloc, "par": par, "prow": prow,
                "rank": rank, "fused": fused, "frank": frank,
                "countsE": countsE, "countsO": countsO, "countsP": countsP,
            })
    borders = []
    NCH, CH = cfg.NCHUNK, cfg.CHUNK
    capsE = np.zeros((NCH, CH), np.int64)   # full single-E tiles (floor)
    capsO = np.zeros((NCH, CH), np.int64)
    nEs, nOs, nPs = [], [], []
    for core in cores:
        tot = core["countsE"] + core["countsO"] + core["countsP"]
        border = np.argsort(-tot, kind="stable")
        borders.append(border)
        nE = core["countsE"][border].reshape(NCH, CH)
        nO = core["countsO"][border].reshape(NCH, CH)
        nP = core["countsP"][border].reshape(NCH, CH)
        nEs.append(nE)
        nOs.append(nO)
        nPs.append(nP)
        capsE = np.maximum(capsE, nE // 128)
        capsO = np.maximum(capsO, nO // 128)
    # mixed tiles hold fused pairs plus each parity's overflow beyond the
    # full tiles (one gather tile, two matmul passes with L/R columns)
    capsM = np.zeros((NCH, CH), np.int64)
    for nE, nO, nP in zip(nEs, nOs, nPs):
        tails = (nP + np.maximum(0, nE - 128 * capsE)
                 + np.maximum(0, nO - 128 * capsO))
        capsM = np.maximum(capsM, (tails + 127) // 128)
    capsE_t = tuple(tuple(int(x) for x in r) for r in capsE)
    capsO_t = tuple(tuple(int(x) for x in r) for r in capsO)
    capsM_t = tuple(tuple(int(x) for x in r) for r in capsM)
    return cores, borders, capsE_t, capsO_t, capsM_t


def build_core_arrays(cfg, cores, borders, i, capsE, capsO, capsM, lay):
    """idx1/idx2 (wrapped int16) + dstl (bf16) for core i."""
    core = cores[i]
    g = core["g"]
    border = borders[i]
    inv = np.empty(cfg.NBH, np.int64)
    inv[border] = np.arange(cfg.NBH)
    capsE_a = np.asarray(capsE)
    capsO_a = np.asarray(capsO)
    capsM_a = np.asarray(capsM)
    blk, dloc, par, prow, rank = (core[k] for k in
                                  ("blk", "dloc", "par", "prow", "rank"))
    fused, frank = core["fused"], core["frank"]
    pos = inv[blk]
    c = pos >> 2
    b = pos & 3
    capE_cb = capsE_a[c, b]
    capO_cb = capsO_a[c, b]
    capM_cb = capsM_a[c, b]
    # full-tile singles vs mixed-region slots (fused pairs first, then
    # E-tails, then O-tails)
    full = (~fused) & np.where(par == 0, rank < 128 * capE_cb,
                               rank < 128 * capO_cb)
    nEb = core["countsE"][blk]
    P_b = core["countsP"][blk]
    tailE_cnt = np.maximum(0, nEb - 128 * capE_cb)
    tail_idx = np.where(par == 0, P_b + rank - 128 * capE_cb,
                        P_b + tailE_cnt + rank - 128 * capO_cb)
    mix_idx = np.where(fused, frank, tail_idx)
    # gather tile within block and slot row
    gt = np.where(full,
                  np.where(par == 0, rank >> 7, capE_cb + (rank >> 7)),
                  capE_cb + capO_cb + (mix_idx >> 7))
    srow = np.where(full, rank & 127, mix_idx & 127)
    # S/dstl logical column within block (odd parity uses the R column set)
    scol = np.where(full,
                    np.where(par == 0, rank >> 7, capE_cb + (rank >> 7)),
                    capE_cb + capO_cb + (mix_idx >> 7)
                    + np.where(par == 0, 0, capM_cb))
    tbase = np.asarray(lay["tbase"])
    colbase = np.asarray(lay["colbase"])
    slotoff = np.asarray(lay["slotoff"])
    slot = slotoff[c] + (tbase[c, b] + gt) * 128 + srow
    idx1 = np.zeros(lay["total_slots"], np.int16)
    idx1[slot] = prow.astype(np.int16)
    # idx2: position of src's pair row in the piecewise/sorted t2 layout
    inv_of = []
    for hs in range(2):
        bo = borders[2 * g + hs]
        io = np.empty(cfg.NBH, np.int64)
        io[bo] = np.arange(cfg.NBH)
        inv_of.append(io)
    s = core["src"]
    hs = s // cfg.HALF
    srel = s - hs * cfg.HALF
    nb_nat = srel >> 7
    pos_o = np.where(hs == 0, inv_of[0][nb_nat], inv_of[1][nb_nat])
    c2 = pos_o >> 2
    rowo = pos_o * 64 + ((srel & 127) >> 1)
    bounds = _piece_bounds(cfg)
    piece_of_chunk = np.zeros(cfg.NCHUNK, np.int64)
    for p in range(NPIECE):
        piece_of_chunk[bounds[p]: bounds[p + 1]] = p
    pieces = lay["pieces"]
    inb = np.array([pc["inb"] for pc in pieces])
    rows = np.array([pc["rows"] for pc in pieces])
    outb = np.array([pc["outb"] for pc in pieces])
    pc = piece_of_chunk[c2]
    row2 = outb[pc] + hs * rows[pc] + (rowo - inb[pc])
    idx2 = np.zeros(lay["total_slots"], np.int16)
    idx2[slot] = row2.astype(np.int16)
    dstl = np.full((128, lay["totcols"]), -1.0, np.float32)
    dstl[srow, colbase[c, b] + scol] = dloc
    return _wrap16(idx1), _wrap16(idx2), dstl.astype(BF_NP)


def _iota_arr(lay):
    cols = np.empty(lay["iota_cols"], np.float32)
    for v in lay["variants"]:
        o = lay["iota_off"][v]
        cols[o: o + 128 * v] = np.repeat(np.arange(128, dtype=np.float32), v)
    return np.tile(cols, (128, 1)).astype(BF_NP)


# ---------------------------------------------------------------------------
# Device kernel
# ---------------------------------------------------------------------------
def build_kernel(cfg, capsE, capsO, capsM):
    lay = derive_layout(cfg, capsE, capsO, capsM)
    NCH, CH = cfg.NCHUNK, cfg.CHUNK
    cap2, scols, T, tbase, colbase = (lay[k] for k in
                                      ("cap2", "scols", "T", "tbase",
                                       "colbase"))
    slotoff, totcols = lay["slotoff"], lay["totcols"]
    TOT = lay["total_slots"]
    J16 = TOT // 16
    iota_off, iota_cols = lay["iota_off"], lay["iota_cols"]
    pieces = lay["pieces"]
    bounds = _piece_bounds(cfg)
    nc = bacc.Bacc(target_bir_lowering=False)

    t1p_in = nc.dram_tensor("t1p", [cfg.PAIRS, 128], BF16, kind="ExternalInput")
    t1own_in = nc.dram_tensor("t1own", [128, cfg.NBH * 64], BF16,
                              kind="ExternalInput")
    idx1_in = nc.dram_tensor("idx1", [128, J16], I16, kind="ExternalInput")
    idx2_in = nc.dram_tensor("idx2", [128, J16], I16, kind="ExternalInput")
    dstl_in = nc.dram_tensor("dstl", [128, totcols], BF16,
                             kind="ExternalInput")
    iota_in = nc.dram_tensor("iota", [128, iota_cols], BF16,
                             kind="ExternalInput")
    dinv_in = nc.dram_tensor("dinv", [128, cfg.NBH], F32, kind="ExternalInput")
    w1_in = nc.dram_tensor("w1", [64, 128], F32, kind="ExternalInput")
    b1_in = nc.dram_tensor("b1", [128, 1], F32, kind="ExternalInput")
    b1p1_in = nc.dram_tensor("b1p1", [128, 1], F32, kind="ExternalInput")
    w2_in = nc.dram_tensor("w2", [128, 64], F32, kind="ExternalInput")
    b2b_in = nc.dram_tensor("b2b", [128, CH * 64], F32, kind="ExternalInput")
    oh_out = nc.dram_tensor("oh", [cfg.HALF, 64], F32, kind="ExternalOutput")
    t2pin = nc.dram_tensor("t2pin", [cfg.PHALF, 128], BF16)
    t2pout = nc.dram_tensor("t2pout", [cfg.PAIRS, 128], BF16)

    with (
        nc.sbuf_tensor("dstl_sb", [128, totcols], BF16) as dstl_sb,
        nc.sbuf_tensor("iota_sb", [128, iota_cols], BF16) as iota_sb,
        nc.sbuf_tensor("dinv_sb", [128, cfg.NBH], F32) as dinv_sb,
        nc.sbuf_tensor("t1own_sb", [128, cfg.NBH * 64], BF16) as t1own_sb,
        nc.sbuf_tensor("t2own_sb", [128, cfg.NBH * 64], BF16) as t2own_sb,
        nc.sbuf_tensor("identb", [128, 128], BF16) as identb,
        nc.sbuf_tensor("w1bf", [64, 128], BF16) as w1bf,
        nc.sbuf_tensor("w2bf", [128, 64], BF16) as w2bf,
        nc.sbuf_tensor("b1sb", [128, 1], F32) as b1sb,
        nc.sbuf_tensor("b1p1sb", [128, 1], F32) as b1p1sb,
        nc.sbuf_tensor("b2sb", [128, CH * 64], F32) as b2sb,
        nc.semaphore("wsem") as wsem,
        nc.semaphore("ccsem") as ccsem,
    ):
        with TileContext(nc) as tc:
            with tc.tile_pool(name="pre", bufs=2) as pre:
                make_identity(nc, identb[:])
                nc.sync.dma_start(out=dstl_sb[:], in_=dstl_in[:])
                nc.sync.dma_start(out=iota_sb[:], in_=iota_in[:])
                nc.sync.dma_start(out=dinv_sb[:], in_=dinv_in[:])
                nc.sync.dma_start(out=t1own_sb[:], in_=t1own_in[:])
                nc.sync.dma_start(out=b1sb[:], in_=b1_in[:])
                nc.sync.dma_start(out=b1p1sb[:], in_=b1p1_in[:])
                nc.sync.dma_start(out=b2sb[:], in_=b2b_in[:])
                wt = pre.tile([64, 128], F32, tag="w1")
                nc.sync.dma_start(out=wt[:], in_=w1_in[:])
                nc.vector.tensor_copy(out=w1bf[:], in_=wt[:])
                wt2 = pre.tile([128, 64], F32, tag="w2")
                nc.sync.dma_start(out=wt2[:], in_=w2_in[:])
                nc.vector.tensor_copy(out=w2bf[:], in_=wt2[:])

        from contextlib import ExitStack
        with TileContext(nc) as tc:
            with ExitStack() as stack:
                idxp = stack.enter_context(tc.tile_pool(name="idxp", bufs=3))
                msgp = stack.enter_context(tc.tile_pool(name="msgp", bufs=2))
                spool = stack.enter_context(tc.tile_pool(name="sp", bufs=3))
                aggpool = stack.enter_context(
                    tc.tile_pool(name="aggp", bufs=2, space="PSUM"))
                tppool = stack.enter_context(
                    tc.tile_pool(name="tpp", bufs=1, space="PSUM"))
                h1pool = stack.enter_context(
                    tc.tile_pool(name="h1p", bufs=2, space="PSUM"))
                zpool = stack.enter_context(
                    tc.tile_pool(name="zpp", bufs=1, space="PSUM"))
                t2ppool = stack.enter_context(
                    tc.tile_pool(name="t2pp", bufs=2, space="PSUM"))
                finp = stack.enter_context(tc.tile_pool(name="fin", bufs=2))
                finp2 = stack.enter_context(tc.tile_pool(name="fin2", bufs=2))
                stgp = stack.enter_context(tc.tile_pool(name="stg", bufs=2))

                regs = {}
                for v in sorted(set(T)):
                    regs[v] = nc.gpsimd.to_reg(v * 128)

                def chunk_agg(c, table, idx_dram):
                    Tc = T[c]
                    idx_t = idxp.tile([128, Tc * 128 // 16], I16)
                    nc.sync.dma_start(
                        out=idx_t[:],
                        in_=idx_dram[:, slotoff[c] // 16:
                                     slotoff[c] // 16 + Tc * 128 // 16])
                    msg = msgp.tile([128, Tc * 128], BF16)
                    nc.gpsimd.dma_gather(
                        out_ap=msg[:].rearrange("p (t e) -> p t e", e=128),
                        in_ap=table[0: cfg.PAIRS, :],
                        idxs_ap=idx_t[:],
                        num_idxs=Tc * 128,
                        num_idxs_reg=regs[Tc],
                        elem_size=128,
                        single_packet=False,
                    )
                    aggP = aggpool.tile([128, CH * 64], F32)
                    for b in range(CH):
                        ks = scols[c][b]
                        kE = capsE[c][b]
                        kO = capsO[c][b]
                        kM = capsM[c][b]
                        io = iota_off[ks]
                        S = spool.tile([128, 128 * ks], BF16)
                        Sv = S[:].rearrange("p (v t) -> p v t", t=ks)
                        nc.vector.tensor_tensor(
                            out=Sv,
                            in0=iota_sb[:, io: io + 128 * ks]
                                .rearrange("p (v t) -> p v t", t=ks),
                            in1=dstl_sb[:, colbase[c][b]: colbase[c][b] + ks]
                                .to_broadcast([128, ks, 128])
                                .rearrange("p t v -> p v t"),
                            op=ALU.is_equal,
                        )
                        for ln in range(ks):
                            # logical col -> (gather tile, rhs half): full E,
                            # full O, mixed L-pass, mixed R-pass
                            if ln < kE + kO + kM:
                                gt = ln
                                off = 0 if (ln < kE or ln >= kE + kO) else 64
                            else:
                                gt = ln - kM
                                off = 64
                            j = tbase[c][b] + gt
                            nc.tensor.matmul(
                                out=aggP[:, b * 64: (b + 1) * 64],
                                lhsT=Sv[:, :, ln],
                                rhs=msg[:, j * 128 + off: j * 128 + off + 64],
                                start=(ln == 0),
                                stop=(ln == ks - 1),
                            )
                    return aggP

                def finish_l1(c, aggP):
                    # u1 = agg + t1 (f32), scale by dinv in f32 via ACT,
                    # single bf16 rounding at aggV before the transposes
                    u1 = finp.tile([128, CH * 64], F32, tag="u1")
                    nc.vector.tensor_tensor(
                        out=u1[:], in0=aggP[:],
                        in1=t1own_sb[:, c * CH * 64: (c + 1) * CH * 64],
                        op=ALU.add)
                    aggV = finp.tile([128, CH * 64], BF16, tag="aggV")
                    tP = tppool.tile([64, CH * 128], BF16, tag="tp")
                    for b in range(CH):
                        gb = c * CH + b
                        nc.scalar.activation(
                            aggV[:, b * 64: (b + 1) * 64],
                            u1[:, b * 64: (b + 1) * 64],
                            AF.Copy, scale=dinv_sb[:, gb: gb + 1])
                        nc.tensor.transpose(
                            out=tP[:, b * 128: (b + 1) * 128],
                            in_=aggV[:, b * 64: (b + 1) * 64],
                            identity=identb[:])
                    aggT = finp.tile([64, CH * 128], BF16, tag="aggT")
                    nc.scalar.activation(aggT[:], tP[:], AF.Copy)
                    h1P = h1pool.tile([128, CH * 128], F32, tag="h1p")
                    nc.tensor.matmul(out=h1P[:], lhsT=w1bf[:], rhs=aggT[:],
                                     start=True, stop=True)
                    m = finp2.tile([128, CH * 128], F32, tag="m")
                    nc.vector.tensor_scalar(
                        out=m[:], in0=h1P[:], scalar1=b1sb[:, 0:1],
                        scalar2=0.0, op0=ALU.add, op1=ALU.min)
                    x1 = finp2.tile([128, CH * 128], F32, tag="x1")
                    nc.vector.tensor_scalar(
                        out=x1[:], in0=h1P[:], scalar1=b1p1sb[:, 0:1],
                        scalar2=None, op0=ALU.add)
                    ex = finp2.tile([128, CH * 128], F32, tag="ex")
                    nc.scalar.activation(ex[:], m[:], AF.Exp)
                    fmx = finp2.tile([128, CH * 128], F32, tag="fmx")
                    nc.vector.tensor_tensor(out=fmx[:], in0=ex[:], in1=x1[:],
                                            op=ALU.max)
                    h1f = finp2.tile([128, CH * 128], BF16, tag="h1f")
                    nc.vector.tensor_scalar(
                        out=h1f[:], in0=fmx[:], scalar1=-1.0, scalar2=None,
                        op0=ALU.add)
                    zP = zpool.tile([64, CH * 128], F32, tag="zp")
                    nc.tensor.matmul(out=zP[:], lhsT=w2bf[:], rhs=h1f[:],
                                     start=True, stop=True)
                    zsb = finp.tile([64, CH * 128], BF16, tag="zsb")
                    nc.scalar.activation(zsb[:], zP[:], AF.Copy)
                    stage = stgp.tile([128, CH * 64], BF16, tag="stage")
                    t2P = t2ppool.tile([128, CH * 64], BF16, tag="t2P")
                    last_act = None
                    for b in range(CH):
                        gb = c * CH + b
                        nc.tensor.transpose(
                            out=t2P[:, b * 64: (b + 1) * 64],
                            in_=zsb[:, b * 128: (b + 1) * 128],
                            identity=identb[:64, :64])
                        last_act = nc.scalar.activation(
                            stage[:, b * 64: (b + 1) * 64],
                            t2P[:, b * 64: (b + 1) * 64],
                            AF.Copy, scale=dinv_sb[:, gb: gb + 1])
                    cp = nc.vector.tensor_copy(
                        out=t2own_sb[:, c * CH * 64: (c + 1) * CH * 64],
                        in_=stage[:])
                    t2own_copies.append(cp)
                    nc.sync.dma_start(
                        out=t2pin[c * CH * 64: (c + 1) * CH * 64, :]
                        .rearrange("(b q) (r e) -> (q r) b e",
                                   q=64, r=2, e=64),
                        in_=stage[:].rearrange("p (b e) -> p b e", e=64),
                    )

                def emit_piece(p):
                    # no manual sems: the shadow tracker orders the
                    # collective after the t2pin region writes and the L2
                    # gathers after the collective outputs
                    pc = pieces[p]
                    nc.gpsimd.collective_compute(
                        "AllGather", ALU.bypass,
                        replica_groups=[[0, 1], [2, 3], [4, 5], [6, 7]],
                        ins=[t2pin[pc["inb"]: pc["inb"] + pc["rows"], :]
                             .opt()],
                        outs=[t2pout[pc["outb"]:
                                     pc["outb"] + 2 * pc["rows"], :]
                              .opt()],
                    )

                # ---------------- Layer 1 (pieces interleaved) ------------
                t2own_copies = []
                next_piece = 0
                for c in range(NCH):
                    aggP = chunk_agg(c, t1p_in, idx1_in)
                    finish_l1(c, aggP)
                    # emit piece p two chunks after its last input chunk so
                    # the Pool engine has gathers in flight while it waits
                    if (next_piece < NPIECE - 1
                            and c == bounds[next_piece + 1] + 1):
                        emit_piece(next_piece)
                        next_piece += 1
                for p in range(next_piece, NPIECE):
                    emit_piece(p)

                # ---------------- Layer 2 ----------------
                for c in range(NCH):
                    aggP = chunk_agg(c, t2pout, idx2_in)
                    u = finp.tile([128, CH * 64], F32, tag="u")
                    uadd = nc.vector.tensor_tensor(
                        out=u[:], in0=aggP[:],
                        in1=t2own_sb[:, c * CH * 64: (c + 1) * CH * 64],
                        op=ALU.add)
                    # raw-sbuf RAW hazard: order the L2 self-loop read after
                    # the L1 writer of the same t2own region explicitly
                    add_dep_helper(uadd.ins, t2own_copies[c].ins,
                                   reason="L2 self-loop reads t2own chunk")
                    y = finp2.tile([128, CH * 64], F32, tag="y")
                    for b in range(CH):
                        gb = c * CH + b
                        nc.vector.tensor_scalar_mul(
                            y[:, b * 64: (b + 1) * 64],
                            u[:, b * 64: (b + 1) * 64],
                            dinv_sb[:, gb: gb + 1])
                    yb = finp2.tile([128, CH * 64], F32, tag="yb")
                    nc.vector.tensor_tensor(out=yb[:], in0=y[:], in1=b2sb[:],
                                            op=ALU.add)
                    m2 = finp2.tile([128, CH * 64], F32, tag="m2")
                    nc.vector.tensor_scalar(
                        out=m2[:], in0=yb[:], scalar1=0.0, scalar2=None,
                        op0=ALU.min)
                    x12 = finp2.tile([128, CH * 64], F32, tag="x12")
                    nc.vector.tensor_scalar(
                        out=x12[:], in0=yb[:], scalar1=1.0, scalar2=None,
                        op0=ALU.add)
                    e2 = finp2.tile([128, CH * 64], F32, tag="e2")
                    nc.scalar.activation(e2[:], m2[:], AF.Exp)
                    f2 = finp2.tile([128, CH * 64], F32, tag="f2")
                    nc.vector.tensor_tensor(out=f2[:], in0=e2[:], in1=x12[:],
                                            op=ALU.max)
                    stage2 = stgp.tile([128, CH * 64], F32, tag="stage2")
                    nc.vector.tensor_scalar(
                        out=stage2[:], in0=f2[:], scalar1=-1.0, scalar2=None,
                        op0=ALU.add)
                    nc.sync.dma_start(
                        out=oh_out[c * CH * 128: (c + 1) * CH * 128, :]
                        .rearrange("(b p) e -> p b e", p=128),
                        in_=stage2[:].rearrange("p (b e) -> p b e", e=64),
                    )
    nc.finalize()
    return nc


# ---------------------------------------------------------------------------
# Driver
# ---------------------------------------------------------------------------
_NC_CACHE = {}
_PREP_CACHE = {}
LAST_TIMES = {}
_LAST_CAPS = None


def _get_nc(cfg, capsE, capsO, capsM):
    key = (cfg.N, cfg.E, capsE, capsO, capsM)
    if key not in _NC_CACHE:
        _NC_CACHE[key] = build_kernel(cfg, capsE, capsO, capsM)
    return _NC_CACHE[key]


def run(cfg, x, edge_index, W1, b1, W2, b2, spmd_kwargs=None):
    global _LAST_CAPS
    spmd_kwargs = spmd_kwargs or {}
    x = np.asarray(x, np.float32)
    W1 = np.asarray(W1, np.float32)
    b1 = np.asarray(b1, np.float32)
    W2 = np.asarray(W2, np.float32)
    b2 = np.asarray(b2, np.float32)

    import hashlib
    ekey = hashlib.sha1(np.ascontiguousarray(edge_index)).hexdigest()
    if ekey in _PREP_CACHE:
        cores, borders, capsE, capsO, capsM, lay, core_arr = _PREP_CACHE[ekey]
    else:
        cores, borders, capsE, capsO, capsM = preprocess(cfg, edge_index)
        lay = derive_layout(cfg, capsE, capsO, capsM)
        core_arr = [build_core_arrays(cfg, cores, borders, i, capsE, capsO,
                                      capsM, lay) for i in range(len(cores))]
        _PREP_CACHE[ekey] = (cores, borders, capsE, capsO, capsM, lay,
                             core_arr)
    _LAST_CAPS = (capsE, capsO, capsM)
    nc = _get_nc(cfg, capsE, capsO, capsM)
    iota = _iota_arr(lay)

    in_maps = []
    for i, core in enumerate(cores):
        g, h = core["g"], core["h"]
        border = borders[i]
        idx1w, idx2w, dstl = core_arr[i]
        dinv = core["dinv"]
        t1 = np.zeros((cfg.NPAD, 64), np.float32)
        t1[: cfg.N] = x[g]
        t1 *= dinv[:, None]
        t1p = np.ascontiguousarray(t1.reshape(cfg.PAIRS, 128)).astype(BF_NP)
        lo = h * cfg.HALF
        t1h = t1[lo: lo + cfg.HALF].reshape(cfg.NBH, 128, 64)
        t1own = np.ascontiguousarray(
            t1h[border].transpose(1, 0, 2).reshape(128, cfg.NBH * 64)
        ).astype(BF_NP)
        dinv_own = np.ascontiguousarray(
            dinv[lo: lo + cfg.HALF].reshape(cfg.NBH, 128)[border].T)
        in_maps.append({
            "t1p": t1p,
            "t1own": t1own,
            "idx1": idx1w,
            "idx2": idx2w,
            "dstl": np.ascontiguousarray(dstl),
            "iota": iota,
            "dinv": dinv_own,
            "w1": np.ascontiguousarray(W1[g]),
            "b1": np.ascontiguousarray(b1[g].reshape(128, 1)),
            "b1p1": np.ascontiguousarray(b1[g].reshape(128, 1) + 1.0),
            "w2": np.ascontiguousarray(W2[g]),
            "b2b": np.ascontiguousarray(
                np.tile(b2[g], (128, cfg.CHUNK)).astype(np.float32)),
        })
    import time as _time
    _t = _time.monotonic()
    res = run_bass_kernel_spmd(nc, in_maps, core_ids=list(range(8)),
                               **spmd_kwargs)
    LAST_TIMES["launch_wall_s"] = _time.monotonic() - _t

    out = np.empty((cfg.G * cfg.N, 64), np.float32)
    for g in range(cfg.G):
        for h in range(2):
            i = 2 * g + h
            oh = res.results[i]["oh"].reshape(cfg.NBH, 128, 64)
            inv = np.empty(cfg.NBH, np.int64)
            inv[borders[i]] = np.arange(cfg.NBH)
            nat = oh[inv].reshape(cfg.HALF, 64)
            lo = g * cfg.N + h * cfg.HALF
            n_rows = min(cfg.HALF, cfg.N - h * cfg.HALF)
            out[lo: lo + n_rows] = nat[:n_rows]
    return out, res


def kernel(x, edge_index, W1, b1, W2, b2):
    out, _ = run(CFG, x, edge_index, W1, b1, W2, b2)
    return out


# revision 5
# speedup vs baseline: 1.0837x; 1.0115x over previous
"""Multi-graph 2-layer GCN on 8 Trainium2 NeuronCores — fused single launch, v3.

v3 over v2:
- Per-core dst blocks are sorted by edge count and packed into chunk
  positions with per-position tile caps (max over cores), cutting gather
  padding from ~19% to ~3%.
- The t2-half AllGather is split into 4 pieces interleaved into the L1
  chunk stream so most of the exchange hides behind L1 gathers.
- Layer-2 gathers use a second index table (idx2) addressing the
  piecewise/sorted t2 table layout; layer-1 indices stay natural.

See kernel_v2 docstring for the base design (bf16 pair-row gather tables,
parity tiles, one-hot S' matmuls with packed-bf16 DVE builds, dinv folded
into PE transposes, ELU = max(x+1, exp(min(x,0))) - 1).
"""

import sys

try:
    import concourse.bass as bass  # noqa: F401
except ImportError:
    sys.path.insert(0, "/opt/trn_rl_repo")
    import concourse.bass as bass

import numpy as np
import ml_dtypes

import concourse.tile as tile_mod  # noqa: F401
from concourse import bacc
import concourse.mybir as mybir
from concourse.bass_utils import run_bass_kernel_spmd
from concourse.tile import TileContext, add_dep_helper
from concourse.masks import make_identity

AF = mybir.ActivationFunctionType
ALU = mybir.AluOpType
F32 = mybir.dt.float32
BF16 = mybir.dt.bfloat16
I16 = mybir.dt.int16

BF_NP = ml_dtypes.bfloat16


def _patched_drain_and_barrier(self, tick_clock, wait_clock):
    from bass_rust import ScopedClock

    probe = self.nc.sync.nop()
    wait_clock.add_sem_waits(probe.ins, ScopedClock({None: tick_clock.global_clock}))
    si = probe.ins.sync_info
    waits = list(si.on_wait) if si and si.on_wait else []
    if si is not None:
        si.on_wait = waits[:1]
    for w in waits[1:]:
        n = self.nc.sync.nop()
        nsi = n.ins.sync_info
        if nsi is None:
            n.ins.sync_info = mybir.SyncInfo(on_wait=[w], on_update=[])
        else:
            nsi.on_wait = [w]
    self.nc.sync.drain()
    self.nc.all_engine_barrier()
    popped = self.nc._tile_sem_poison_stack.pop()
    assert popped is self._sem_poison
    self.nc.clear_and_free_semaphores(list(self.sems.allocated().values()))
    self.nc.all_engine_barrier()


TileContext._drain_and_barrier = _patched_drain_and_barrier

_orig_add_instruction = TileContext._add_instruction
_waitsplit_counter = [0]


def _patched_add_instruction(self, inst):
    """walrus rejects instructions carrying >1 sem wait; hoist excess waits
    onto same-engine nops inserted immediately before the instruction."""
    si = inst.sync_info
    if (si is not None and si.on_wait and len(si.on_wait) > 1
            and inst.engine != mybir.EngineType.Unassigned):
        waits = list(si.on_wait)
        si.on_wait = waits[-1:]
        for w in waits[:-1]:
            _waitsplit_counter[0] += 1
            nop = mybir.InstNoOp(
                name=f"I-wsplit-{_waitsplit_counter[0]}", ins=[], outs=[])
            nop.engine = inst.engine
            nop.sync_info = mybir.SyncInfo(on_wait=[w], on_update=[])
            _orig_add_instruction(self, nop)
    _orig_add_instruction(self, inst)


TileContext._add_instruction = _patched_add_instruction


# ---------------------------------------------------------------------------
# Config
# ---------------------------------------------------------------------------
class Cfg:
    def __init__(self, G, N, E, F_IN, HID, OUT, chunk=4):
        self.G, self.N, self.E = G, N, E
        self.F_IN, self.HID, self.OUT = F_IN, HID, OUT
        assert F_IN == OUT == 64 and HID == 128
        self.NB = (N + 255) // 256 * 2
        self.NPAD = self.NB * 128
        self.NBH = self.NB // 2
        self.HALF = self.NBH * 128
        self.PAIRS = self.NPAD // 2
        self.PHALF = self.HALF // 2
        self.CHUNK = chunk
        assert self.NBH % chunk == 0
        self.NCHUNK = self.NBH // chunk


CFG = Cfg(G=4, N=50000, E=800000, F_IN=64, HID=128, OUT=64, chunk=4)
NPIECE = 6


def _piece_bounds(cfg):
    n = cfg.NCHUNK
    step = n // NPIECE
    return [p * step for p in range(NPIECE)] + [n]


# ---------------------------------------------------------------------------
# Layout derivation shared by host packing and device program
# ---------------------------------------------------------------------------
def derive_layout(cfg, capsE, capsO, capsM):
    """caps*: [NCHUNK][CHUNK] ints (full-E, full-O, mixed-tail tiles).
    Gather tiles per block: E+O+M; S/dstl logical columns: E+O+2M (each
    mixed tile is swept twice, with left and right rhs halves)."""
    NCH, CH = cfg.NCHUNK, cfg.CHUNK
    cap2 = [[capsE[c][b] + capsO[c][b] + capsM[c][b] for b in range(CH)]
            for c in range(NCH)]
    scols = [[capsE[c][b] + capsO[c][b] + 2 * capsM[c][b] for b in range(CH)]
             for c in range(NCH)]
    T = [sum(cap2[c]) for c in range(NCH)]
    tbase = [[0] * CH for _ in range(NCH)]
    for c in range(NCH):
        for b in range(1, CH):
            tbase[c][b] = tbase[c][b - 1] + cap2[c][b - 1]
    colbase = [[0] * CH for _ in range(NCH)]
    acc = 0
    for c in range(NCH):
        for b in range(CH):
            colbase[c][b] = acc
            acc += scols[c][b]
    totcols = acc
    slotoff = [0] * NCH
    for c in range(1, NCH):
        slotoff[c] = slotoff[c - 1] + T[c - 1] * 128
    total_slots = slotoff[-1] + T[-1] * 128
    variants = sorted({scols[c][b] for c in range(NCH) for b in range(CH)})
    iota_off = {}
    acc = 0
    for v in variants:
        iota_off[v] = acc
        acc += 128 * v
    iota_cols = acc
    bounds = _piece_bounds(cfg)
    pieces = []
    outb = 0
    for p in range(NPIECE):
        c0, c1 = bounds[p], bounds[p + 1]
        rows = (c1 - c0) * CH * 64
        pieces.append(dict(c0=c0, c1=c1, inb=c0 * CH * 64, rows=rows,
                           outb=outb))
        outb += 2 * rows
    return dict(cap2=cap2, scols=scols, T=T, tbase=tbase, colbase=colbase,
                totcols=totcols, slotoff=slotoff, total_slots=total_slots,
                variants=variants, iota_off=iota_off, iota_cols=iota_cols,
                pieces=pieces)


# ---------------------------------------------------------------------------
# Host-side preprocessing
# ---------------------------------------------------------------------------
def _wrap16(flat_i16):
    s = flat_i16.shape[0]
    assert s % 16 == 0
    w = flat_i16.reshape(s // 16, 16).T
    return np.tile(w, (8, 1))


def preprocess(cfg, edge_index):
    cores = []
    for g in range(cfg.G):
        src_g = np.asarray(edge_index[g, 0], np.int64)
        dst_g = np.asarray(edge_index[g, 1], np.int64)
        deg = np.bincount(dst_g, minlength=cfg.NPAD).astype(np.float64) + 1.0
        dinv = (1.0 / np.sqrt(deg)).astype(np.float32)
        for h in range(2):
            lo, hi = h * cfg.HALF, (h + 1) * cfg.HALF
            sel = (dst_g >= lo) & (dst_g < hi)
            s = src_g[sel]
            d = dst_g[sel] - lo
            blk = d >> 7
            dloc = d & 127
            par = s & 1
            prow = s >> 1
            # fuse L/R collisions: an even- and an odd-parity edge of the
            # same block hitting the same pair row share one gather slot
            # (the mixed tiles' dual L/R columns handle the two dsts)
            n_e = len(s)
            order = np.lexsort((par, prow, blk))
            s, blk, dloc, par, prow = (a[order] for a in
                                       (s, blk, dloc, par, prow))
            gkey = blk * cfg.PAIRS + prow
            gid = np.concatenate([[0], np.cumsum(gkey[1:] != gkey[:-1])])
            gcounts = np.bincount(gid)
            gstart = np.concatenate([[0], np.cumsum(gcounts)[:-1]])
            # per-(blk,prow) group: evens come first; j = rank within parity
            idx_in_g = np.arange(n_e) - gstart[gid]
            gp = np.bincount(gid * 2 + par, minlength=2 * (len(gcounts)))
            ne_in_g = gp[0::2][gid]
            no_in_g = gp[1::2][gid]
            j = np.where(par == 0, idx_in_g, idx_in_g - ne_in_g)
            m = np.minimum(ne_in_g, no_in_g)
            fused = j < m
            # fused-pair index within block: k-th fused-even pairs with the
            # k-th fused-odd (identical (blk,prow,j) enumeration order)
            frank = np.zeros(n_e, np.int64)
            for pv in (0, 1):
                sel = fused & (par == pv)
                bsel = blk[sel]
                cnts = np.bincount(bsel, minlength=cfg.NBH)
                st = np.concatenate([[0], np.cumsum(cnts)[:-1]])
                frank[sel] = np.arange(sel.sum()) - st[bsel]
            countsP = np.bincount(blk[fused & (par == 0)], minlength=cfg.NBH)
            # singles re-ranked within (blk, parity)
            rank = np.zeros(n_e, np.int64)
            for pv in (0, 1):
                sel = (~fused) & (par == pv)
                bsel = blk[sel]
                cnts = np.bincount(bsel, minlength=cfg.NBH)
                st = np.concatenate([[0], np.cumsum(cnts)[:-1]])
                rank[sel] = np.arange(sel.sum()) - st[bsel]
                if pv == 0:
                    countsE = cnts
                else:
                    countsO = cnts
            cores.append({
                "g": g, "h": h, "dinv": dinv, "src": s,
                "blk": blk, "dloc": dloc, "par": par, "prow": prow,
                "rank": rank, "fused": fused, "frank": frank,
                "countsE": countsE, "countsO": countsO, "countsP": countsP,
            })
    borders = []
    NCH, CH = cfg.NCHUNK, cfg.CHUNK
    capsE = np.zeros((NCH, CH), np.int64)   # full single-E tiles (floor)
    capsO = np.zeros((NCH, CH), np.int64)
    nEs, nOs, nPs = [], [], []
    for core in cores:
        tot = core["countsE"] + core["countsO"] + core["countsP"]
        border = np.argsort(-tot, kind="stable")
        borders.append(border)
        nE = core["countsE"][border].reshape(NCH, CH)
        nO = core["countsO"][border].reshape(NCH, CH)
        nP = core["countsP"][border].reshape(NCH, CH)
        nEs.append(nE)
        nOs.append(nO)
        nPs.append(nP)
        capsE = np.maximum(capsE, nE // 128)
        capsO = np.maximum(capsO, nO // 128)
    # mixed tiles hold fused pairs plus each parity's overflow beyond the
    # full tiles (one gather tile, two matmul passes with L/R columns)
    capsM = np.zeros((NCH, CH), np.int64)
    for nE, nO, nP in zip(nEs, nOs, nPs):
        tails = (nP + np.maximum(0, nE - 128 * capsE)
                 + np.maximum(0, nO - 128 * capsO))
        capsM = np.maximum(capsM, (tails + 127) // 128)
    capsE_t = tuple(tuple(int(x) for x in r) for r in capsE)
    capsO_t = tuple(tuple(int(x) for x in r) for r in capsO)
    capsM_t = tuple(tuple(int(x) for x in r) for r in capsM)
    return cores, borders, capsE_t, capsO_t, capsM_t


def build_core_arrays(cfg, cores, borders, i, capsE, capsO, capsM, lay):
    """idx1/idx2 (wrapped int16) + dstl (bf16) for core i."""
    core = cores[i]
    g = core["g"]
    border = borders[i]
    inv = np.empty(cfg.NBH, np.int64)
    inv[border] = np.arange(cfg.NBH)
    capsE_a = np.asarray(capsE)
    capsO_a = np.asarray(capsO)
    capsM_a = np.asarray(capsM)
    blk, dloc, par, prow, rank = (core[k] for k in
                                  ("blk", "dloc", "par", "prow", "rank"))
    fused, frank = core["fused"], core["frank"]
    pos = inv[blk]
    c = pos >> 2
    b = pos & 3
    capE_cb = capsE_a[c, b]
    capO_cb = capsO_a[c, b]
    capM_cb = capsM_a[c, b]
    # full-tile singles vs mixed-region slots (fused pairs first, then
    # E-tails, then O-tails)
    full = (~fused) & np.where(par == 0, rank < 128 * capE_cb,
                               rank < 128 * capO_cb)
    nEb = core["countsE"][blk]
    P_b = core["countsP"][blk]
    tailE_cnt = np.maximum(0, nEb - 128 * capE_cb)
    tail_idx = np.where(par == 0, P_b + rank - 128 * capE_cb,
                        P_b + tailE_cnt + rank - 128 * capO_cb)
    mix_idx = np.where(fused, frank, tail_idx)
    # gather tile within block and slot row
    gt = np.where(full,
                  np.where(par == 0, rank >> 7, capE_cb + (rank >> 7)),
                  capE_cb + capO_cb + (mix_idx >> 7))
    srow = np.where(full, rank & 127, mix_idx & 127)
    # S/dstl logical column within block (odd parity uses the R column set)
    scol = np.where(full,
                    np.where(par == 0, rank >> 7, capE_cb + (rank >> 7)),
                    capE_cb + capO_cb + (mix_idx >> 7)
                    + np.where(par == 0, 0, capM_cb))
    tbase = np.asarray(lay["tbase"])
    colbase = np.asarray(lay["colbase"])
    slotoff = np.asarray(lay["slotoff"])
    slot = slotoff[c] + (tbase[c, b] + gt) * 128 + srow
    idx1 = np.zeros(lay["total_slots"], np.int16)
    idx1[slot] = prow.astype(np.int16)
    # idx2: position of src's pair row in the piecewise/sorted t2 layout
    inv_of = []
    for hs in range(2):
        bo = borders[2 * g + hs]
        io = np.empty(cfg.NBH, np.int64)
        io[bo] = np.arange(cfg.NBH)
        inv_of.append(io)
    s = core["src"]
    hs = s // cfg.HALF
    srel = s - hs * cfg.HALF
    nb_nat = srel >> 7
    pos_o = np.where(hs == 0, inv_of[0][nb_nat], inv_of[1][nb_nat])
    c2 = pos_o >> 2
    rowo = pos_o * 64 + ((srel & 127) >> 1)
    bounds = _piece_bounds(cfg)
    piece_of_chunk = np.zeros(cfg.NCHUNK, np.int64)
    for p in range(NPIECE):
        piece_of_chunk[bounds[p]: bounds[p + 1]] = p
    pieces = lay["pieces"]
    inb = np.array([pc["inb"] for pc in pieces])
    rows = np.array([pc["rows"] for pc in pieces])
    outb = np.array([pc["outb"] for pc in pieces])
    pc = piece_of_chunk[c2]
    row2 = outb[pc] + hs * rows[pc] + (rowo - inb[pc])
    idx2 = np.zeros(lay["total_slots"], np.int16)
    idx2[slot] = row2.astype(np.int16)
    dstl = np.full((128, lay["totcols"]), -1.0, np.float32)
    dstl[srow, colbase[c, b] + scol] = dloc
    return _wrap16(idx1), _wrap16(idx2), dstl.astype(BF_NP)


def _iota_arr(lay):
    cols = np.empty(lay["iota_cols"], np.float32)
    for v in lay["variants"]:
        o = lay["iota_off"][v]
        cols[o: o + 128 * v] = np.repeat(np.arange(128, dtype=np.float32), v)
    return np.tile(cols, (128, 1)).astype(BF_NP)


# ---------------------------------------------------------------------------
# Device kernel
# ---------------------------------------------------------------------------
def build_kernel(cfg, capsE, capsO, capsM):
    lay = derive_layout(cfg, capsE, capsO, capsM)
    NCH, CH = cfg.NCHUNK, cfg.CHUNK
    cap2, scols, T, tbase, colbase = (lay[k] for k in
                                      ("cap2", "scols", "T", "tbase",
                                       "colbase"))
    slotoff, totcols = lay["slotoff"], lay["totcols"]
    TOT = lay["total_slots"]
    J16 = TOT // 16
    iota_off, iota_cols = lay["iota_off"], lay["iota_cols"]
    pieces = lay["pieces"]
    bounds = _piece_bounds(cfg)
    nc = bacc.Bacc(target_bir_lowering=False)

    t1p_in = nc.dram_tensor("t1p", [cfg.PAIRS, 128], BF16, kind="ExternalInput")
    t1own_in = nc.dram_tensor("t1own", [128, cfg.NBH * 64], BF16,
                              kind="ExternalInput")
    idx1_in = nc.dram_tensor("idx1", [128, J16], I16, kind="ExternalInput")
    idx2_in = nc.dram_tensor("idx2", [128, J16], I16, kind="ExternalInput")
    dstl_in = nc.dram_tensor("dstl", [128, totcols], BF16,
                             kind="ExternalInput")
    iota_in = nc.dram_tensor("iota", [128, iota_cols], BF16,
                             kind="ExternalInput")
    dinv_in = nc.dram_tensor("dinv", [128, cfg.NBH], F32, kind="ExternalInput")
    w1_in = nc.dram_tensor("w1", [64, 128], F32, kind="ExternalInput")
    b1_in = nc.dram_tensor("b1", [128, 1], F32, kind="ExternalInput")
    b1p1_in = nc.dram_tensor("b1p1", [128, 1], F32, kind="ExternalInput")
    w2_in = nc.dram_tensor("w2", [128, 64], F32, kind="ExternalInput")
    b2b_in = nc.dram_tensor("b2b", [128, CH * 64], F32, kind="ExternalInput")
    oh_out = nc.dram_tensor("oh", [cfg.HALF, 64], F32, kind="ExternalOutput")
    t2pin = nc.dram_tensor("t2pin", [cfg.PHALF, 128], BF16)
    t2pout = nc.dram_tensor("t2pout", [cfg.PAIRS, 128], BF16)

    with (
        nc.sbuf_tensor("dstl_sb", [128, totcols], BF16) as dstl_sb,
        nc.sbuf_tensor("iota_sb", [128, iota_cols], BF16) as iota_sb,
        nc.sbuf_tensor("dinv_sb", [128, cfg.NBH], F32) as dinv_sb,
        nc.sbuf_tensor("t1own_sb", [128, cfg.NBH * 64], BF16) as t1own_sb,
        nc.sbuf_tensor("t2own_sb", [128, cfg.NBH * 64], BF16) as t2own_sb,
        nc.sbuf_tensor("identb", [128, 128], BF16) as identb,
        nc.sbuf_tensor("w1bf", [64, 128], BF16) as w1bf,
        nc.sbuf_tensor("w2bf", [128, 64], BF16) as w2bf,
        nc.sbuf_tensor("b1sb", [128, 1], F32) as b1sb,
        nc.sbuf_tensor("b1p1sb", [128, 1], F32) as b1p1sb,
        nc.sbuf_tensor("b2sb", [128, CH * 64], F32) as b2sb,
        nc.semaphore("wsem") as wsem,
        nc.semaphore("ccsem") as ccsem,
    ):
        with TileContext(nc) as tc:
            with tc.tile_pool(name="pre", bufs=2) as pre:
                make_identity(nc, identb[:])
                nc.sync.dma_start(out=dstl_sb[:], in_=dstl_in[:])
                nc.sync.dma_start(out=iota_sb[:], in_=iota_in[:])
                nc.sync.dma_start(out=dinv_sb[:], in_=dinv_in[:])
                nc.sync.dma_start(out=t1own_sb[:], in_=t1own_in[:])
                nc.sync.dma_start(out=b1sb[:], in_=b1_in[:])
                nc.sync.dma_start(out=b1p1sb[:], in_=b1p1_in[:])
                nc.sync.dma_start(out=b2sb[:], in_=b2b_in[:])
                wt = pre.tile([64, 128], F32, tag="w1")
                nc.sync.dma_start(out=wt[:], in_=w1_in[:])
                nc.vector.tensor_copy(out=w1bf[:], in_=wt[:])
                wt2 = pre.tile([128, 64], F32, tag="w2")
                nc.sync.dma_start(out=wt2[:], in_=w2_in[:])
                nc.vector.tensor_copy(out=w2bf[:], in_=wt2[:])

        from contextlib import ExitStack
        with TileContext(nc) as tc:
            with ExitStack() as stack:
                idxp = stack.enter_context(tc.tile_pool(name="idxp", bufs=3))
                msgp = stack.enter_context(tc.tile_pool(name="msgp", bufs=2))
                spool = stack.enter_context(tc.tile_pool(name="sp", bufs=3))
                aggpool = stack.enter_context(
                    tc.tile_pool(name="aggp", bufs=2, space="PSUM"))
                tppool = stack.enter_context(
                    tc.tile_pool(name="tpp", bufs=1, space="PSUM"))
                h1pool = stack.enter_context(
                    tc.tile_pool(name="h1p", bufs=2, space="PSUM"))
                zpool = stack.enter_context(
                    tc.tile_pool(name="zpp", bufs=1, space="PSUM"))
                t2ppool = stack.enter_context(
                    tc.tile_pool(name="t2pp", bufs=2, space="PSUM"))
                finp = stack.enter_context(tc.tile_pool(name="fin", bufs=2))
                finp2 = stack.enter_context(tc.tile_pool(name="fin2", bufs=2))
                stgp = stack.enter_context(tc.tile_pool(name="stg", bufs=2))

                regs = {}
                for v in sorted(set(T)):
                    regs[v] = nc.gpsimd.to_reg(v * 128)

                def chunk_agg(c, table, idx_dram):
                    Tc = T[c]
                    idx_t = idxp.tile([128, Tc * 128 // 16], I16)
                    nc.sync.dma_start(
                        out=idx_t[:],
                        in_=idx_dram[:, slotoff[c] // 16:
                                     slotoff[c] // 16 + Tc * 128 // 16])
                    msg = msgp.tile([128, Tc * 128], BF16)
                    nc.gpsimd.dma_gather(
                        out_ap=msg[:].rearrange("p (t e) -> p t e", e=128),
                        in_ap=table[0: cfg.PAIRS, :],
                        idxs_ap=idx_t[:],
                        num_idxs=Tc * 128,
                        num_idxs_reg=regs[Tc],
                        elem_size=128,
                        single_packet=False,
                    )
                    aggP = aggpool.tile([128, CH * 64], F32)
                    for b in range(CH):
                        ks = scols[c][b]
                        kE = capsE[c][b]
                        kO = capsO[c][b]
                        kM = capsM[c][b]
                        io = iota_off[ks]
                        S = spool.tile([128, 128 * ks], BF16)
                        Sv = S[:].rearrange("p (v t) -> p v t", t=ks)
                        nc.vector.tensor_tensor(
                            out=Sv,
                            in0=iota_sb[:, io: io + 128 * ks]
                                .rearrange("p (v t) -> p v t", t=ks),
                            in1=dstl_sb[:, colbase[c][b]: colbase[c][b] + ks]
                                .to_broadcast([128, ks, 128])
                                .rearrange("p t v -> p v t"),
                            op=ALU.is_equal,
                        )
                        for ln in range(ks):
                            # logical col -> (gather tile, rhs half): full E,
                            # full O, mixed L-pass, mixed R-pass
                            if ln < kE + kO + kM:
                                gt = ln
                                off = 0 if (ln < kE or ln >= kE + kO) else 64
                            else:
                                gt = ln - kM
                                off = 64
                            j = tbase[c][b] + gt
                            nc.tensor.matmul(
                                out=aggP[:, b * 64: (b + 1) * 64],
                                lhsT=Sv[:, :, ln],
                                rhs=msg[:, j * 128 + off: j * 128 + off + 64],
                                start=(ln == 0),
                                stop=(ln == ks - 1),
                            )
                    return aggP

                def finish_l1(c, aggP):
                    # u1 = agg + t1 (f32), scale by dinv in f32 via ACT,
                    # single bf16 rounding at aggV before the transposes
                    u1 = finp.tile([128, CH * 64], F32, tag="u1")
                    nc.vector.tensor_tensor(
                        out=u1[:], in0=aggP[:],
                        in1=t1own_sb[:, c * CH * 64: (c + 1) * CH * 64],
                        op=ALU.add)
                    aggV = finp.tile([128, CH * 64], BF16, tag="aggV")
                    tP = tppool.tile([64, CH * 128], BF16, tag="tp")
                    for b in range(CH):
                        gb = c * CH + b
                        nc.scalar.activation(
                            aggV[:, b * 64: (b + 1) * 64],
                            u1[:, b * 64: (b + 1) * 64],
                            AF.Copy, scale=dinv_sb[:, gb: gb + 1])
                        nc.tensor.transpose(
                            out=tP[:, b * 128: (b + 1) * 128],
                            in_=aggV[:, b * 64: (b + 1) * 64],
                            identity=identb[:])
                    aggT = finp.tile([64, CH * 128], BF16, tag="aggT")
                    nc.scalar.activation(aggT[:], tP[:], AF.Copy)
                    h1P = h1pool.tile([128, CH * 128], F32, tag="h1p")
                    nc.tensor.matmul(out=h1P[:], lhsT=w1bf[:], rhs=aggT[:],
                                     start=True, stop=True)
                    m = finp2.tile([128, CH * 128], F32, tag="m")
                    nc.vector.tensor_scalar(
                        out=m[:], in0=h1P[:], scalar1=b1sb[:, 0:1],
                        scalar2=0.0, op0=ALU.add, op1=ALU.min)
                    x1 = finp2.tile([128, CH * 128], F32, tag="x1")
                    nc.vector.tensor_scalar(
                        out=x1[:], in0=h1P[:], scalar1=b1p1sb[:, 0:1],
                        scalar2=None, op0=ALU.add)
                    ex = finp2.tile([128, CH * 128], F32, tag="ex")
                    nc.scalar.activation(ex[:], m[:], AF.Exp)
                    fmx = finp2.tile([128, CH * 128], F32, tag="fmx")
                    nc.vector.tensor_tensor(out=fmx[:], in0=ex[:], in1=x1[:],
                                            op=ALU.max)
                    h1f = finp2.tile([128, CH * 128], BF16, tag="h1f")
                    nc.vector.tensor_scalar(
                        out=h1f[:], in0=fmx[:], scalar1=-1.0, scalar2=None,
                        op0=ALU.add)
                    zP = zpool.tile([64, CH * 128], F32, tag="zp")
                    nc.tensor.matmul(out=zP[:], lhsT=w2bf[:], rhs=h1f[:],
                                     start=True, stop=True)
                    zsb = finp.tile([64, CH * 128], BF16, tag="zsb")
                    nc.scalar.activation(zsb[:], zP[:], AF.Copy)
                    stage = stgp.tile([128, CH * 64], BF16, tag="stage")
                    t2P = t2ppool.tile([128, CH * 64], BF16, tag="t2P")
                    last_act = None
                    for b in range(CH):
                        gb = c * CH + b
                        nc.tensor.transpose(
                            out=t2P[:, b * 64: (b + 1) * 64],
                            in_=zsb[:, b * 128: (b + 1) * 128],
                            identity=identb[:64, :64])
                        last_act = nc.scalar.activation(
                            stage[:, b * 64: (b + 1) * 64],
                            t2P[:, b * 64: (b + 1) * 64],
                            AF.Copy, scale=dinv_sb[:, gb: gb + 1])
                    cp = nc.vector.tensor_copy(
                        out=t2own_sb[:, c * CH * 64: (c + 1) * CH * 64],
                        in_=stage[:])
                    t2own_copies.append(cp)
                    nc.sync.dma_start(
                        out=t2pin[c * CH * 64: (c + 1) * CH * 64, :]
                        .rearrange("(b q) (r e) -> (q r) b e",
                                   q=64, r=2, e=64),
                        in_=stage[:].rearrange("p (b e) -> p b e", e=64),
                    )

                def emit_piece(p):
                    # no manual sems: the shadow tracker orders the
                    # collective after the t2pin region writes and the L2
                    # gathers after the collective outputs
                    pc = pieces[p]
                    nc.gpsimd.collective_compute(
                        "AllGather", ALU.bypass,
                        replica_groups=[[0, 1], [2, 3], [4, 5], [6, 7]],
                        ins=[t2pin[pc["inb"]: pc["inb"] + pc["rows"], :]
                             .opt()],
                        outs=[t2pout[pc["outb"]:
                                     pc["outb"] + 2 * pc["rows"], :]
                              .opt()],
                    )

                # ---------------- Layer 1 (pieces interleaved) ------------
                t2own_copies = []
                next_piece = 0
                for c in range(NCH):
                    aggP = chunk_agg(c, t1p_in, idx1_in)
                    finish_l1(c, aggP)
                    # emit piece p two chunks after its last input chunk so
                    # the Pool engine has gathers in flight while it waits
                    if (next_piece < NPIECE - 1
                            and c == bounds[next_piece + 1] + 1):
                        emit_piece(next_piece)
                        next_piece += 1
                for p in range(next_piece, NPIECE):
                    emit_piece(p)

                # ---------------- Layer 2 ----------------
                for c in range(NCH):
                    aggP = chunk_agg(c, t2pout, idx2_in)
                    u = finp.tile([128, CH * 64], F32, tag="u")
                    uadd = nc.vector.tensor_tensor(
                        out=u[:], in0=aggP[:],
                        in1=t2own_sb[:, c * CH * 64: (c + 1) * CH * 64],
                        op=ALU.add)
                    # raw-sbuf RAW hazard: order the L2 self-loop read after
                    # the L1 writer of the same t2own region explicitly
                    add_dep_helper(uadd.ins, t2own_copies[c].ins,
                                   reason="L2 self-loop reads t2own chunk")
                    y = finp2.tile([128, CH * 64], F32, tag="y")
                    for b in range(CH):
                        gb = c * CH + b
                        nc.vector.tensor_scalar_mul(
                            y[:, b * 64: (b + 1) * 64],
                            u[:, b * 64: (b + 1) * 64],
                            dinv_sb[:, gb: gb + 1])
                    yb = finp2.tile([128, CH * 64], F32, tag="yb")
                    nc.vector.tensor_tensor(out=yb[:], in0=y[:], in1=b2sb[:],
                                            op=ALU.add)
                    m2 = finp2.tile([128, CH * 64], F32, tag="m2")
                    nc.vector.tensor_scalar(
                        out=m2[:], in0=yb[:], scalar1=0.0, scalar2=None,
                        op0=ALU.min)
                    x12 = finp2.tile([128, CH * 64], F32, tag="x12")
                    nc.vector.tensor_scalar(
                        out=x12[:], in0=yb[:], scalar1=1.0, scalar2=None,
                        op0=ALU.add)
                    e2 = finp2.tile([128, CH * 64], F32, tag="e2")
                    nc.scalar.activation(e2[:], m2[:], AF.Exp)
                    f2 = finp2.tile([128, CH * 64], F32, tag="f2")
                    nc.vector.tensor_tensor(out=f2[:], in0=e2[:], in1=x12[:],
                                            op=ALU.max)
                    stage2 = stgp.tile([128, CH * 64], F32, tag="stage2")
                    nc.vector.tensor_scalar(
                        out=stage2[:], in0=f2[:], scalar1=-1.0, scalar2=None,
                        op0=ALU.add)
                    nc.sync.dma_start(
                        out=oh_out[c * CH * 128: (c + 1) * CH * 128, :]
                        .rearrange("(b p) e -> p b e", p=128),
                        in_=stage2[:].rearrange("p (b e) -> p b e", e=64),
                    )
    nc.finalize()
    return nc


# ---------------------------------------------------------------------------
# Driver
# ---------------------------------------------------------------------------
_NC_CACHE = {}
_PREP_CACHE = {}
LAST_TIMES = {}
_LAST_CAPS = None


def _get_nc(cfg, capsE, capsO, capsM):
    key = (cfg.N, cfg.E, capsE, capsO, capsM)
    if key not in _NC_CACHE:
        _NC_CACHE[key] = build_kernel(cfg, capsE, capsO, capsM)
    return _NC_CACHE[key]


def run(cfg, x, edge_index, W1, b1, W2, b2, spmd_kwargs=None):
    global _LAST_CAPS
    spmd_kwargs = spmd_kwargs or {}
    x = np.asarray(x, np.float32)
    W1 = np.asarray(W1, np.float32)
    b1 = np.asarray(b1, np.float32)
    W2 = np.asarray(W2, np.float32)
    b2 = np.asarray(b2, np.float32)

    import hashlib
    ekey = hashlib.sha1(np.ascontiguousarray(edge_index)).hexdigest()
    if ekey in _PREP_CACHE:
        cores, borders, capsE, capsO, capsM, lay, core_arr = _PREP_CACHE[ekey]
    else:
        cores, borders, capsE, capsO, capsM = preprocess(cfg, edge_index)
        lay = derive_layout(cfg, capsE, capsO, capsM)
        core_arr = [build_core_arrays(cfg, cores, borders, i, capsE, capsO,
                                      capsM, lay) for i in range(len(cores))]
        _PREP_CACHE[ekey] = (cores, borders, capsE, capsO, capsM, lay,
                             core_arr)
    _LAST_CAPS = (capsE, capsO, capsM)
    nc = _get_nc(cfg, capsE, capsO, capsM)
    iota = _iota_arr(lay)

    in_maps = []
    for i, core in enumerate(cores):
        g, h = core["g"], core["h"]
        border = borders[i]
        idx1w, idx2w, dstl = core_arr[i]
        dinv = core["dinv"]
        t1 = np.zeros((cfg.NPAD, 64), np.float32)
        t1[: cfg.N] = x[g]
        t1 *= dinv[:, None]
        t1p = np.ascontiguousarray(t1.reshape(cfg.PAIRS, 128)).astype(BF_NP)
        lo = h * cfg.HALF
        t1h = t1[lo: lo + cfg.HALF].reshape(cfg.NBH, 128, 64)
        t1own = np.ascontiguousarray(
            t1h[border].transpose(1, 0, 2).reshape(128, cfg.NBH * 64)
        ).astype(BF_NP)
        dinv_own = np.ascontiguousarray(
            dinv[lo: lo + cfg.HALF].reshape(cfg.NBH, 128)[border].T)
        in_maps.append({
            "t1p": t1p,
            "t1own": t1own,
            "idx1": idx1w,
            "idx2": idx2w,
            "dstl": np.ascontiguousarray(dstl),
            "iota": iota,
            "dinv": dinv_own,
            "w1": np.ascontiguousarray(W1[g]),
            "b1": np.ascontiguousarray(b1[g].reshape(128, 1)),
            "b1p1": np.ascontiguousarray(b1[g].reshape(128, 1) + 1.0),
            "w2": np.ascontiguousarray(W2[g]),
            "b2b": np.ascontiguousarray(
                np.tile(b2[g], (128, cfg.CHUNK)).astype(np.float32)),
        })
    import time as _time
    _t = _time.monotonic()
    res = run_bass_kernel_spmd(nc, in_maps, core_ids=list(range(8)),
                               **spmd_kwargs)
    LAST_TIMES["launch_wall_s"] = _time.monotonic() - _t

    out = np.empty((cfg.G * cfg.N, 64), np.float32)
    for g in range(cfg.G):
        for h in range(2):
            i = 2 * g + h
            oh = res.results[i]["oh"].reshape(cfg.NBH, 128, 64)
            inv = np.empty(cfg.NBH, np.int64)
            inv[borders[i]] = np.arange(cfg.NBH)
            nat = oh[inv].reshape(cfg.HALF, 64)
            lo = g * cfg.N + h * cfg.HALF
            n_rows = min(cfg.HALF, cfg.N - h * cfg.HALF)
            out[lo: lo + n_rows] = nat[:n_rows]
    return out, res


def kernel(x, edge_index, W1, b1, W2, b2):
    out, _ = run(CFG, x, edge_index, W1, b1, W2, b2)
    return out


# revision 6
# speedup vs baseline: 1.0899x; 1.0057x over previous
"""Multi-graph 2-layer GCN on 8 Trainium2 NeuronCores — fused single launch, v3.

v3 over v2:
- Per-core dst blocks are sorted by edge count and packed into chunk
  positions with per-position tile caps (max over cores), cutting gather
  padding from ~19% to ~3%.
- The t2-half AllGather is split into 4 pieces interleaved into the L1
  chunk stream so most of the exchange hides behind L1 gathers.
- Layer-2 gathers use a second index table (idx2) addressing the
  piecewise/sorted t2 table layout; layer-1 indices stay natural.

See kernel_v2 docstring for the base design (bf16 pair-row gather tables,
parity tiles, one-hot S' matmuls with packed-bf16 DVE builds, dinv folded
into PE transposes, ELU = max(x+1, exp(min(x,0))) - 1).
"""

import sys

try:
    import concourse.bass as bass  # noqa: F401
except ImportError:
    sys.path.insert(0, "/opt/trn_rl_repo")
    import concourse.bass as bass

import numpy as np
import ml_dtypes

import concourse.tile as tile_mod  # noqa: F401
from concourse import bacc
import concourse.mybir as mybir
from concourse.bass_utils import run_bass_kernel_spmd
from concourse.tile import TileContext, add_dep_helper
from concourse.masks import make_identity

AF = mybir.ActivationFunctionType
ALU = mybir.AluOpType
F32 = mybir.dt.float32
BF16 = mybir.dt.bfloat16
I16 = mybir.dt.int16

BF_NP = ml_dtypes.bfloat16


def _patched_drain_and_barrier(self, tick_clock, wait_clock):
    from bass_rust import ScopedClock

    probe = self.nc.sync.nop()
    wait_clock.add_sem_waits(probe.ins, ScopedClock({None: tick_clock.global_clock}))
    si = probe.ins.sync_info
    waits = list(si.on_wait) if si and si.on_wait else []
    if si is not None:
        si.on_wait = waits[:1]
    for w in waits[1:]:
        n = self.nc.sync.nop()
        nsi = n.ins.sync_info
        if nsi is None:
            n.ins.sync_info = mybir.SyncInfo(on_wait=[w], on_update=[])
        else:
            nsi.on_wait = [w]
    self.nc.sync.drain()
    self.nc.all_engine_barrier()
    popped = self.nc._tile_sem_poison_stack.pop()
    assert popped is self._sem_poison
    self.nc.clear_and_free_semaphores(list(self.sems.allocated().values()))
    self.nc.all_engine_barrier()


TileContext._drain_and_barrier = _patched_drain_and_barrier

_orig_add_instruction = TileContext._add_instruction
_waitsplit_counter = [0]


def _patched_add_instruction(self, inst):
    """walrus rejects instructions carrying >1 sem wait; hoist excess waits
    onto same-engine nops inserted immediately before the instruction."""
    si = inst.sync_info
    if (si is not None and si.on_wait and len(si.on_wait) > 1
            and inst.engine != mybir.EngineType.Unassigned):
        waits = list(si.on_wait)
        si.on_wait = waits[-1:]
        for w in waits[:-1]:
            _waitsplit_counter[0] += 1
            nop = mybir.InstNoOp(
                name=f"I-wsplit-{_waitsplit_counter[0]}", ins=[], outs=[])
            nop.engine = inst.engine
            nop.sync_info = mybir.SyncInfo(on_wait=[w], on_update=[])
            _orig_add_instruction(self, nop)
    _orig_add_instruction(self, inst)


TileContext._add_instruction = _patched_add_instruction


# ---------------------------------------------------------------------------
# Config
# ---------------------------------------------------------------------------
class Cfg:
    def __init__(self, G, N, E, F_IN, HID, OUT, chunk=4):
        self.G, self.N, self.E = G, N, E
        self.F_IN, self.HID, self.OUT = F_IN, HID, OUT
        assert F_IN == OUT == 64 and HID == 128
        self.NB = (N + 255) // 256 * 2
        self.NPAD = self.NB * 128
        self.NBH = self.NB // 2
        self.HALF = self.NBH * 128
        self.PAIRS = self.NPAD // 2
        self.PHALF = self.HALF // 2
        self.CHUNK = chunk
        assert self.NBH % chunk == 0
        self.NCHUNK = self.NBH // chunk


CFG = Cfg(G=4, N=50000, E=800000, F_IN=64, HID=128, OUT=64, chunk=4)
NPIECE = 7


def _piece_bounds(cfg):
    n = cfg.NCHUNK
    step = n // NPIECE
    return [p * step for p in range(NPIECE)] + [n]


# ---------------------------------------------------------------------------
# Layout derivation shared by host packing and device program
# ---------------------------------------------------------------------------
def derive_layout(cfg, capsE, capsO, capsM):
    """caps*: [NCHUNK][CHUNK] ints (full-E, full-O, mixed-tail tiles).
    Gather tiles per block: E+O+M; S/dstl logical columns: E+O+2M (each
    mixed tile is swept twice, with left and right rhs halves)."""
    NCH, CH = cfg.NCHUNK, cfg.CHUNK
    cap2 = [[capsE[c][b] + capsO[c][b] + capsM[c][b] for b in range(CH)]
            for c in range(NCH)]
    scols = [[capsE[c][b] + capsO[c][b] + 2 * capsM[c][b] for b in range(CH)]
             for c in range(NCH)]
    T = [sum(cap2[c]) for c in range(NCH)]
    tbase = [[0] * CH for _ in range(NCH)]
    for c in range(NCH):
        for b in range(1, CH):
            tbase[c][b] = tbase[c][b - 1] + cap2[c][b - 1]
    colbase = [[0] * CH for _ in range(NCH)]
    acc = 0
    for c in range(NCH):
        for b in range(CH):
            colbase[c][b] = acc
            acc += scols[c][b]
    totcols = acc
    slotoff = [0] * NCH
    for c in range(1, NCH):
        slotoff[c] = slotoff[c - 1] + T[c - 1] * 128
    total_slots = slotoff[-1] + T[-1] * 128
    variants = sorted({scols[c][b] for c in range(NCH) for b in range(CH)})
    iota_off = {}
    acc = 0
    for v in variants:
        iota_off[v] = acc
        acc += 128 * v
    iota_cols = acc
    bounds = _piece_bounds(cfg)
    pieces = []
    outb = 0
    for p in range(NPIECE):
        c0, c1 = bounds[p], bounds[p + 1]
        rows = (c1 - c0) * CH * 64
        pieces.append(dict(c0=c0, c1=c1, inb=c0 * CH * 64, rows=rows,
                           outb=outb))
        outb += 2 * rows
    return dict(cap2=cap2, scols=scols, T=T, tbase=tbase, colbase=colbase,
                totcols=totcols, slotoff=slotoff, total_slots=total_slots,
                variants=variants, iota_off=iota_off, iota_cols=iota_cols,
                pieces=pieces)


# ---------------------------------------------------------------------------
# Host-side preprocessing
# ---------------------------------------------------------------------------
def _wrap16(flat_i16):
    s = flat_i16.shape[0]
    assert s % 16 == 0
    w = flat_i16.reshape(s // 16, 16).T
    return np.tile(w, (8, 1))


def preprocess(cfg, edge_index):
    cores = []
    for g in range(cfg.G):
        src_g = np.asarray(edge_index[g, 0], np.int64)
        dst_g = np.asarray(edge_index[g, 1], np.int64)
        deg = np.bincount(dst_g, minlength=cfg.NPAD).astype(np.float64) + 1.0
        dinv = (1.0 / np.sqrt(deg)).astype(np.float32)
        for h in range(2):
            lo, hi = h * cfg.HALF, (h + 1) * cfg.HALF
            sel = (dst_g >= lo) & (dst_g < hi)
            s = src_g[sel]
            d = dst_g[sel] - lo
            blk = d >> 7
            dloc = d & 127
            par = s & 1
            prow = s >> 1
            # fuse L/R collisions: an even- and an odd-parity edge of the
            # same block hitting the same pair row share one gather slot
            # (the mixed tiles' dual L/R columns handle the two dsts)
            n_e = len(s)
            order = np.lexsort((par, prow, blk))
            s, blk, dloc, par, prow = (a[order] for a in
                                       (s, blk, dloc, par, prow))
            gkey = blk * cfg.PAIRS + prow
            gid = np.concatenate([[0], np.cumsum(gkey[1:] != gkey[:-1])])
            gcounts = np.bincount(gid)
            gstart = np.concatenate([[0], np.cumsum(gcounts)[:-1]])
            # per-(blk,prow) group: evens come first; j = rank within parity
            idx_in_g = np.arange(n_e) - gstart[gid]
            gp = np.bincount(gid * 2 + par, minlength=2 * (len(gcounts)))
            ne_in_g = gp[0::2][gid]
            no_in_g = gp[1::2][gid]
            j = np.where(par == 0, idx_in_g, idx_in_g - ne_in_g)
            m = np.minimum(ne_in_g, no_in_g)
            fused = j < m
            # fused-pair index within block: k-th fused-even pairs with the
            # k-th fused-odd (identical (blk,prow,j) enumeration order)
            frank = np.zeros(n_e, np.int64)
            for pv in (0, 1):
                sel = fused & (par == pv)
                bsel = blk[sel]
                cnts = np.bincount(bsel, minlength=cfg.NBH)
                st = np.concatenate([[0], np.cumsum(cnts)[:-1]])
                frank[sel] = np.arange(sel.sum()) - st[bsel]
            countsP = np.bincount(blk[fused & (par == 0)], minlength=cfg.NBH)
            # singles re-ranked within (blk, parity)
            rank = np.zeros(n_e, np.int64)
            for pv in (0, 1):
                sel = (~fused) & (par == pv)
                bsel = blk[sel]
                cnts = np.bincount(bsel, minlength=cfg.NBH)
                st = np.concatenate([[0], np.cumsum(cnts)[:-1]])
                rank[sel] = np.arange(sel.sum()) - st[bsel]
                if pv == 0:
                    countsE = cnts
                else:
                    countsO = cnts
            cores.append({
                "g": g, "h": h, "dinv": dinv, "src": s,
                "blk": blk, "dloc": dloc, "par": par, "prow": prow,
                "rank": rank, "fused": fused, "frank": frank,
                "countsE": countsE, "countsO": countsO, "countsP": countsP,
            })
    borders = []
    NCH, CH = cfg.NCHUNK, cfg.CHUNK
    capsE = np.zeros((NCH, CH), np.int64)   # full single-E tiles (floor)
    capsO = np.zeros((NCH, CH), np.int64)
    nEs, nOs, nPs = [], [], []
    for core in cores:
        tot = core["countsE"] + core["countsO"] + core["countsP"]
        border = np.argsort(-tot, kind="stable")
        borders.append(border)
        nE = core["countsE"][border].reshape(NCH, CH)
        nO = core["countsO"][border].reshape(NCH, CH)
        nP = core["countsP"][border].reshape(NCH, CH)
        nEs.append(nE)
        nOs.append(nO)
        nPs.append(nP)
        capsE = np.maximum(capsE, nE // 128)
        capsO = np.maximum(capsO, nO // 128)
    # mixed tiles hold fused pairs plus each parity's overflow beyond the
    # full tiles (one gather tile, two matmul passes with L/R columns)
    capsM = np.zeros((NCH, CH), np.int64)
    for nE, nO, nP in zip(nEs, nOs, nPs):
        tails = (nP + np.maximum(0, nE - 128 * capsE)
                 + np.maximum(0, nO - 128 * capsO))
        capsM = np.maximum(capsM, (tails + 127) // 128)
    capsE_t = tuple(tuple(int(x) for x in r) for r in capsE)
    capsO_t = tuple(tuple(int(x) for x in r) for r in capsO)
    capsM_t = tuple(tuple(int(x) for x in r) for r in capsM)
    return cores, borders, capsE_t, capsO_t, capsM_t


def build_core_arrays(cfg, cores, borders, i, capsE, capsO, capsM, lay):
    """idx1/idx2 (wrapped int16) + dstl (bf16) for core i."""
    core = cores[i]
    g = core["g"]
    border = borders[i]
    inv = np.empty(cfg.NBH, np.int64)
    inv[border] = np.arange(cfg.NBH)
    capsE_a = np.asarray(capsE)
    capsO_a = np.asarray(capsO)
    capsM_a = np.asarray(capsM)
    blk, dloc, par, prow, rank = (core[k] for k in
                                  ("blk", "dloc", "par", "prow", "rank"))
    fused, frank = core["fused"], core["frank"]
    pos = inv[blk]
    c = pos >> 2
    b = pos & 3
    capE_cb = capsE_a[c, b]
    capO_cb = capsO_a[c, b]
    capM_cb = capsM_a[c, b]
    # full-tile singles vs mixed-region slots (fused pairs first, then
    # E-tails, then O-tails)
    full = (~fused) & np.where(par == 0, rank < 128 * capE_cb,
                               rank < 128 * capO_cb)
    nEb = core["countsE"][blk]
    P_b = core["countsP"][blk]
    tailE_cnt = np.maximum(0, nEb - 128 * capE_cb)
    tail_idx = np.where(par == 0, P_b + rank - 128 * capE_cb,
                        P_b + tailE_cnt + rank - 128 * capO_cb)
    mix_idx = np.where(fused, frank, tail_idx)
    # gather tile within block and slot row
    gt = np.where(full,
                  np.where(par == 0, rank >> 7, capE_cb + (rank >> 7)),
                  capE_cb + capO_cb + (mix_idx >> 7))
    srow = np.where(full, rank & 127, mix_idx & 127)
    # S/dstl logical column within block (odd parity uses the R column set)
    scol = np.where(full,
                    np.where(par == 0, rank >> 7, capE_cb + (rank >> 7)),
                    capE_cb + capO_cb + (mix_idx >> 7)
                    + np.where(par == 0, 0, capM_cb))
    tbase = np.asarray(lay["tbase"])
    colbase = np.asarray(lay["colbase"])
    slotoff = np.asarray(lay["slotoff"])
    slot = slotoff[c] + (tbase[c, b] + gt) * 128 + srow
    idx1 = np.zeros(lay["total_slots"], np.int16)
    idx1[slot] = prow.astype(np.int16)
    # idx2: position of src's pair row in the piecewise/sorted t2 layout
    inv_of = []
    for hs in range(2):
        bo = borders[2 * g + hs]
        io = np.empty(cfg.NBH, np.int64)
        io[bo] = np.arange(cfg.NBH)
        inv_of.append(io)
    s = core["src"]
    hs = s // cfg.HALF
    srel = s - hs * cfg.HALF
    nb_nat = srel >> 7
    pos_o = np.where(hs == 0, inv_of[0][nb_nat], inv_of[1][nb_nat])
    c2 = pos_o >> 2
    rowo = pos_o * 64 + ((srel & 127) >> 1)
    bounds = _piece_bounds(cfg)
    piece_of_chunk = np.zeros(cfg.NCHUNK, np.int64)
    for p in range(NPIECE):
        piece_of_chunk[bounds[p]: bounds[p + 1]] = p
    pieces = lay["pieces"]
    inb = np.array([pc["inb"] for pc in pieces])
    rows = np.array([pc["rows"] for pc in pieces])
    outb = np.array([pc["outb"] for pc in pieces])
    pc = piece_of_chunk[c2]
    row2 = outb[pc] + hs * rows[pc] + (rowo - inb[pc])
    idx2 = np.zeros(lay["total_slots"], np.int16)
    idx2[slot] = row2.astype(np.int16)
    dstl = np.full((128, lay["totcols"]), -1.0, np.float32)
    dstl[srow, colbase[c, b] + scol] = dloc
    return _wrap16(idx1), _wrap16(idx2), dstl.astype(BF_NP)


def _iota_arr(lay):
    cols = np.empty(lay["iota_cols"], np.float32)
    for v in lay["variants"]:
        o = lay["iota_off"][v]
        cols[o: o + 128 * v] = np.repeat(np.arange(128, dtype=np.float32), v)
    return np.tile(cols, (128, 1)).astype(BF_NP)


# ---------------------------------------------------------------------------
# Device kernel
# ---------------------------------------------------------------------------
def build_kernel(cfg, capsE, capsO, capsM):
    lay = derive_layout(cfg, capsE, capsO, capsM)
    NCH, CH = cfg.NCHUNK, cfg.CHUNK
    cap2, scols, T, tbase, colbase = (lay[k] for k in
                                      ("cap2", "scols", "T", "tbase",
                                       "colbase"))
    slotoff, totcols = lay["slotoff"], lay["totcols"]
    TOT = lay["total_slots"]
    J16 = TOT // 16
    iota_off, iota_cols = lay["iota_off"], lay["iota_cols"]
    pieces = lay["pieces"]
    bounds = _piece_bounds(cfg)
    nc = bacc.Bacc(target_bir_lowering=False)

    t1p_in = nc.dram_tensor("t1p", [cfg.PAIRS, 128], BF16, kind="ExternalInput")
    t1own_in = nc.dram_tensor("t1own", [128, cfg.NBH * 64], BF16,
                              kind="ExternalInput")
    idx1_in = nc.dram_tensor("idx1", [128, J16], I16, kind="ExternalInput")
    idx2_in = nc.dram_tensor("idx2", [128, J16], I16, kind="ExternalInput")
    dstl_in = nc.dram_tensor("dstl", [128, totcols], BF16,
                             kind="ExternalInput")
    iota_in = nc.dram_tensor("iota", [128, iota_cols], BF16,
                             kind="ExternalInput")
    dinv_in = nc.dram_tensor("dinv", [128, cfg.NBH], F32, kind="ExternalInput")
    w1_in = nc.dram_tensor("w1", [64, 128], F32, kind="ExternalInput")
    b1_in = nc.dram_tensor("b1", [128, 1], F32, kind="ExternalInput")
    b1p1_in = nc.dram_tensor("b1p1", [128, 1], F32, kind="ExternalInput")
    w2_in = nc.dram_tensor("w2", [128, 64], F32, kind="ExternalInput")
    b2b_in = nc.dram_tensor("b2b", [128, CH * 64], F32, kind="ExternalInput")
    oh_out = nc.dram_tensor("oh", [cfg.HALF, 64], F32, kind="ExternalOutput")
    t2pin = nc.dram_tensor("t2pin", [cfg.PHALF, 128], BF16)
    t2pout = nc.dram_tensor("t2pout", [cfg.PAIRS, 128], BF16)

    with (
        nc.sbuf_tensor("dstl_sb", [128, totcols], BF16) as dstl_sb,
        nc.sbuf_tensor("iota_sb", [128, iota_cols], BF16) as iota_sb,
        nc.sbuf_tensor("dinv_sb", [128, cfg.NBH], F32) as dinv_sb,
        nc.sbuf_tensor("t1own_sb", [128, cfg.NBH * 64], BF16) as t1own_sb,
        nc.sbuf_tensor("t2own_sb", [128, cfg.NBH * 64], BF16) as t2own_sb,
        nc.sbuf_tensor("identb", [128, 128], BF16) as identb,
        nc.sbuf_tensor("w1bf", [64, 128], BF16) as w1bf,
        nc.sbuf_tensor("w2bf", [128, 64], BF16) as w2bf,
        nc.sbuf_tensor("b1sb", [128, 1], F32) as b1sb,
        nc.sbuf_tensor("b1p1sb", [128, 1], F32) as b1p1sb,
        nc.sbuf_tensor("b2sb", [128, CH * 64], F32) as b2sb,
        nc.semaphore("wsem") as wsem,
        nc.semaphore("ccsem") as ccsem,
    ):
        with TileContext(nc) as tc:
            with tc.tile_pool(name="pre", bufs=2) as pre:
                make_identity(nc, identb[:])
                nc.sync.dma_start(out=dstl_sb[:], in_=dstl_in[:])
                nc.sync.dma_start(out=iota_sb[:], in_=iota_in[:])
                nc.sync.dma_start(out=dinv_sb[:], in_=dinv_in[:])
                nc.sync.dma_start(out=t1own_sb[:], in_=t1own_in[:])
                nc.sync.dma_start(out=b1sb[:], in_=b1_in[:])
                nc.sync.dma_start(out=b1p1sb[:], in_=b1p1_in[:])
                nc.sync.dma_start(out=b2sb[:], in_=b2b_in[:])
                wt = pre.tile([64, 128], F32, tag="w1")
                nc.sync.dma_start(out=wt[:], in_=w1_in[:])
                nc.vector.tensor_copy(out=w1bf[:], in_=wt[:])
                wt2 = pre.tile([128, 64], F32, tag="w2")
                nc.sync.dma_start(out=wt2[:], in_=w2_in[:])
                nc.vector.tensor_copy(out=w2bf[:], in_=wt2[:])

        from contextlib import ExitStack
        with TileContext(nc) as tc:
            with ExitStack() as stack:
                idxp = stack.enter_context(tc.tile_pool(name="idxp", bufs=3))
                msgp = stack.enter_context(tc.tile_pool(name="msgp", bufs=2))
                spool = stack.enter_context(tc.tile_pool(name="sp", bufs=3))
                aggpool = stack.enter_context(
                    tc.tile_pool(name="aggp", bufs=2, space="PSUM"))
                tppool = stack.enter_context(
                    tc.tile_pool(name="tpp", bufs=1, space="PSUM"))
                h1pool = stack.enter_context(
                    tc.tile_pool(name="h1p", bufs=2, space="PSUM"))
                zpool = stack.enter_context(
                    tc.tile_pool(name="zpp", bufs=1, space="PSUM"))
                t2ppool = stack.enter_context(
                    tc.tile_pool(name="t2pp", bufs=2, space="PSUM"))
                finp = stack.enter_context(tc.tile_pool(name="fin", bufs=2))
                finp2 = stack.enter_context(tc.tile_pool(name="fin2", bufs=2))
                stgp = stack.enter_context(tc.tile_pool(name="stg", bufs=2))

                regs = {}
                for v in sorted(set(T)):
                    regs[v] = nc.gpsimd.to_reg(v * 128)

                def chunk_agg(c, table, idx_dram):
                    Tc = T[c]
                    idx_t = idxp.tile([128, Tc * 128 // 16], I16)
                    nc.sync.dma_start(
                        out=idx_t[:],
                        in_=idx_dram[:, slotoff[c] // 16:
                                     slotoff[c] // 16 + Tc * 128 // 16])
                    msg = msgp.tile([128, Tc * 128], BF16)
                    nc.gpsimd.dma_gather(
                        out_ap=msg[:].rearrange("p (t e) -> p t e", e=128),
                        in_ap=table[0: cfg.PAIRS, :],
                        idxs_ap=idx_t[:],
                        num_idxs=Tc * 128,
                        num_idxs_reg=regs[Tc],
                        elem_size=128,
                        single_packet=False,
                    )
                    aggP = aggpool.tile([128, CH * 64], F32)
                    for b in range(CH):
                        ks = scols[c][b]
                        kE = capsE[c][b]
                        kO = capsO[c][b]
                        kM = capsM[c][b]
                        io = iota_off[ks]
                        S = spool.tile([128, 128 * ks], BF16)
                        Sv = S[:].rearrange("p (v t) -> p v t", t=ks)
                        nc.vector.tensor_tensor(
                            out=Sv,
                            in0=iota_sb[:, io: io + 128 * ks]
                                .rearrange("p (v t) -> p v t", t=ks),
                            in1=dstl_sb[:, colbase[c][b]: colbase[c][b] + ks]
                                .to_broadcast([128, ks, 128])
                                .rearrange("p t v -> p v t"),
                            op=ALU.is_equal,
                        )
                        for ln in range(ks):
                            # logical col -> (gather tile, rhs half): full E,
                            # full O, mixed L-pass, mixed R-pass
                            if ln < kE + kO + kM:
                                gt = ln
                                off = 0 if (ln < kE or ln >= kE + kO) else 64
                            else:
                                gt = ln - kM
                                off = 64
                            j = tbase[c][b] + gt
                            nc.tensor.matmul(
                                out=aggP[:, b * 64: (b + 1) * 64],
                                lhsT=Sv[:, :, ln],
                                rhs=msg[:, j * 128 + off: j * 128 + off + 64],
                                start=(ln == 0),
                                stop=(ln == ks - 1),
                            )
                    return aggP

                def finish_l1(c, aggP):
                    # u1 = agg + t1 (f32), scale by dinv in f32 via ACT,
                    # single bf16 rounding at aggV before the transposes
                    u1 = finp.tile([128, CH * 64], F32, tag="u1")
                    nc.vector.tensor_tensor(
                        out=u1[:], in0=aggP[:],
                        in1=t1own_sb[:, c * CH * 64: (c + 1) * CH * 64],
                        op=ALU.add)
                    aggV = finp.tile([128, CH * 64], BF16, tag="aggV")
                    tP = tppool.tile([64, CH * 128], BF16, tag="tp")
                    for b in range(CH):
                        gb = c * CH + b
                        nc.scalar.activation(
                            aggV[:, b * 64: (b + 1) * 64],
                            u1[:, b * 64: (b + 1) * 64],
                            AF.Copy, scale=dinv_sb[:, gb: gb + 1])
                        nc.tensor.transpose(
                            out=tP[:, b * 128: (b + 1) * 128],
                            in_=aggV[:, b * 64: (b + 1) * 64],
                            identity=identb[:])
                    aggT = finp.tile([64, CH * 128], BF16, tag="aggT")
                    nc.scalar.activation(aggT[:], tP[:], AF.Copy)
                    h1P = h1pool.tile([128, CH * 128], F32, tag="h1p")
                    nc.tensor.matmul(out=h1P[:], lhsT=w1bf[:], rhs=aggT[:],
                                     start=True, stop=True)
                    m = finp2.tile([128, CH * 128], F32, tag="m")
                    nc.vector.tensor_scalar(
                        out=m[:], in0=h1P[:], scalar1=b1sb[:, 0:1],
                        scalar2=0.0, op0=ALU.add, op1=ALU.min)
                    x1 = finp2.tile([128, CH * 128], F32, tag="x1")
                    nc.vector.tensor_scalar(
                        out=x1[:], in0=h1P[:], scalar1=b1p1sb[:, 0:1],
                        scalar2=None, op0=ALU.add)
                    ex = finp2.tile([128, CH * 128], F32, tag="ex")
                    nc.scalar.activation(ex[:], m[:], AF.Exp)
                    fmx = finp2.tile([128, CH * 128], F32, tag="fmx")
                    nc.vector.tensor_tensor(out=fmx[:], in0=ex[:], in1=x1[:],
                                            op=ALU.max)
                    h1f = finp2.tile([128, CH * 128], BF16, tag="h1f")
                    nc.vector.tensor_scalar(
                        out=h1f[:], in0=fmx[:], scalar1=-1.0, scalar2=None,
                        op0=ALU.add)
                    zP = zpool.tile([64, CH * 128], F32, tag="zp")
                    nc.tensor.matmul(out=zP[:], lhsT=w2bf[:], rhs=h1f[:],
                                     start=True, stop=True)
                    zsb = finp.tile([64, CH * 128], BF16, tag="zsb")
                    nc.scalar.activation(zsb[:], zP[:], AF.Copy)
                    stage = stgp.tile([128, CH * 64], BF16, tag="stage")
                    t2P = t2ppool.tile([128, CH * 64], BF16, tag="t2P")
                    last_act = None
                    for b in range(CH):
                        gb = c * CH + b
                        nc.tensor.transpose(
                            out=t2P[:, b * 64: (b + 1) * 64],
                            in_=zsb[:, b * 128: (b + 1) * 128],
                            identity=identb[:64, :64])
                        last_act = nc.scalar.activation(
                            stage[:, b * 64: (b + 1) * 64],
                            t2P[:, b * 64: (b + 1) * 64],
                            AF.Copy, scale=dinv_sb[:, gb: gb + 1])
                    cp = nc.vector.tensor_copy(
                        out=t2own_sb[:, c * CH * 64: (c + 1) * CH * 64],
                        in_=stage[:])
                    t2own_copies.append(cp)
                    nc.sync.dma_start(
                        out=t2pin[c * CH * 64: (c + 1) * CH * 64, :]
                        .rearrange("(b q) (r e) -> (q r) b e",
                                   q=64, r=2, e=64),
                        in_=stage[:].rearrange("p (b e) -> p b e", e=64),
                    )

                def emit_piece(p):
                    # no manual sems: the shadow tracker orders the
                    # collective after the t2pin region writes and the L2
                    # gathers after the collective outputs
                    pc = pieces[p]
                    nc.gpsimd.collective_compute(
                        "AllGather", ALU.bypass,
                        replica_groups=[[0, 1], [2, 3], [4, 5], [6, 7]],
                        ins=[t2pin[pc["inb"]: pc["inb"] + pc["rows"], :]
                             .opt()],
                        outs=[t2pout[pc["outb"]:
                                     pc["outb"] + 2 * pc["rows"], :]
                              .opt()],
                    )

                # ---------------- Layer 1 (pieces interleaved) ------------
                t2own_copies = []
                next_piece = 0
                for c in range(NCH):
                    aggP = chunk_agg(c, t1p_in, idx1_in)
                    finish_l1(c, aggP)
                    # emit piece p two chunks after its last input chunk so
                    # the Pool engine has gathers in flight while it waits
                    if (next_piece < NPIECE - 1
                            and c == bounds[next_piece + 1] + 1):
                        emit_piece(next_piece)
                        next_piece += 1
                for p in range(next_piece, NPIECE):
                    emit_piece(p)

                # ---------------- Layer 2 ----------------
                for c in range(NCH):
                    aggP = chunk_agg(c, t2pout, idx2_in)
                    u = finp.tile([128, CH * 64], F32, tag="u")
                    uadd = nc.vector.tensor_tensor(
                        out=u[:], in0=aggP[:],
                        in1=t2own_sb[:, c * CH * 64: (c + 1) * CH * 64],
                        op=ALU.add)
                    # raw-sbuf RAW hazard: order the L2 self-loop read after
                    # the L1 writer of the same t2own region explicitly
                    add_dep_helper(uadd.ins, t2own_copies[c].ins,
                                   reason="L2 self-loop reads t2own chunk")
                    y = finp2.tile([128, CH * 64], F32, tag="y")
                    for b in range(CH):
                        gb = c * CH + b
                        nc.vector.tensor_scalar_mul(
                            y[:, b * 64: (b + 1) * 64],
                            u[:, b * 64: (b + 1) * 64],
                            dinv_sb[:, gb: gb + 1])
                    yb = finp2.tile([128, CH * 64], F32, tag="yb")
                    nc.vector.tensor_tensor(out=yb[:], in0=y[:], in1=b2sb[:],
                                            op=ALU.add)
                    m2 = finp2.tile([128, CH * 64], F32, tag="m2")
                    nc.vector.tensor_scalar(
                        out=m2[:], in0=yb[:], scalar1=0.0, scalar2=None,
                        op0=ALU.min)
                    x12 = finp2.tile([128, CH * 64], F32, tag="x12")
                    nc.vector.tensor_scalar(
                        out=x12[:], in0=yb[:], scalar1=1.0, scalar2=None,
                        op0=ALU.add)
                    e2 = finp2.tile([128, CH * 64], F32, tag="e2")
                    nc.scalar.activation(e2[:], m2[:], AF.Exp)
                    f2 = finp2.tile([128, CH * 64], F32, tag="f2")
                    nc.vector.tensor_tensor(out=f2[:], in0=e2[:], in1=x12[:],
                                            op=ALU.max)
                    stage2 = stgp.tile([128, CH * 64], F32, tag="stage2")
                    nc.vector.tensor_scalar(
                        out=stage2[:], in0=f2[:], scalar1=-1.0, scalar2=None,
                        op0=ALU.add)
                    nc.sync.dma_start(
                        out=oh_out[c * CH * 128: (c + 1) * CH * 128, :]
                        .rearrange("(b p) e -> p b e", p=128),
                        in_=stage2[:].rearrange("p (b e) -> p b e", e=64),
                    )
    nc.finalize()
    return nc


# ---------------------------------------------------------------------------
# Driver
# ---------------------------------------------------------------------------
_NC_CACHE = {}
_PREP_CACHE = {}
LAST_TIMES = {}
_LAST_CAPS = None


def _get_nc(cfg, capsE, capsO, capsM):
    key = (cfg.N, cfg.E, capsE, capsO, capsM)
    if key not in _NC_CACHE:
        _NC_CACHE[key] = build_kernel(cfg, capsE, capsO, capsM)
    return _NC_CACHE[key]


def run(cfg, x, edge_index, W1, b1, W2, b2, spmd_kwargs=None):
    global _LAST_CAPS
    spmd_kwargs = spmd_kwargs or {}
    x = np.asarray(x, np.float32)
    W1 = np.asarray(W1, np.float32)
    b1 = np.asarray(b1, np.float32)
    W2 = np.asarray(W2, np.float32)
    b2 = np.asarray(b2, np.float32)

    import hashlib
    ekey = hashlib.sha1(np.ascontiguousarray(edge_index)).hexdigest()
    if ekey in _PREP_CACHE:
        cores, borders, capsE, capsO, capsM, lay, core_arr = _PREP_CACHE[ekey]
    else:
        cores, borders, capsE, capsO, capsM = preprocess(cfg, edge_index)
        lay = derive_layout(cfg, capsE, capsO, capsM)
        core_arr = [build_core_arrays(cfg, cores, borders, i, capsE, capsO,
                                      capsM, lay) for i in range(len(cores))]
        _PREP_CACHE[ekey] = (cores, borders, capsE, capsO, capsM, lay,
                             core_arr)
    _LAST_CAPS = (capsE, capsO, capsM)
    nc = _get_nc(cfg, capsE, capsO, capsM)
    iota = _iota_arr(lay)

    in_maps = []
    for i, core in enumerate(cores):
        g, h = core["g"], core["h"]
        border = borders[i]
        idx1w, idx2w, dstl = core_arr[i]
        dinv = core["dinv"]
        t1 = np.zeros((cfg.NPAD, 64), np.float32)
        t1[: cfg.N] = x[g]
        t1 *= dinv[:, None]
        t1p = np.ascontiguousarray(t1.reshape(cfg.PAIRS, 128)).astype(BF_NP)
        lo = h * cfg.HALF
        t1h = t1[lo: lo + cfg.HALF].reshape(cfg.NBH, 128, 64)
        t1own = np.ascontiguousarray(
            t1h[border].transpose(1, 0, 2).reshape(128, cfg.NBH * 64)
        ).astype(BF_NP)
        dinv_own = np.ascontiguousarray(
            dinv[lo: lo + cfg.HALF].reshape(cfg.NBH, 128)[border].T)
        in_maps.append({
            "t1p": t1p,
            "t1own": t1own,
            "idx1": idx1w,
            "idx2": idx2w,
            "dstl": np.ascontiguousarray(dstl),
            "iota": iota,
            "dinv": dinv_own,
            "w1": np.ascontiguousarray(W1[g]),
            "b1": np.ascontiguousarray(b1[g].reshape(128, 1)),
            "b1p1": np.ascontiguousarray(b1[g].reshape(128, 1) + 1.0),
            "w2": np.ascontiguousarray(W2[g]),
            "b2b": np.ascontiguousarray(
                np.tile(b2[g], (128, cfg.CHUNK)).astype(np.float32)),
        })
    import time as _time
    _t = _time.monotonic()
    res = run_bass_kernel_spmd(nc, in_maps, core_ids=list(range(8)),
                               **spmd_kwargs)
    LAST_TIMES["launch_wall_s"] = _time.monotonic() - _t

    out = np.empty((cfg.G * cfg.N, 64), np.float32)
    for g in range(cfg.G):
        for h in range(2):
            i = 2 * g + h
            oh = res.results[i]["oh"].reshape(cfg.NBH, 128, 64)
            inv = np.empty(cfg.NBH, np.int64)
            inv[borders[i]] = np.arange(cfg.NBH)
            nat = oh[inv].reshape(cfg.HALF, 64)
            lo = g * cfg.N + h * cfg.HALF
            n_rows = min(cfg.HALF, cfg.N - h * cfg.HALF)
            out[lo: lo + n_rows] = nat[:n_rows]
    return out, res


def kernel(x, edge_index, W1, b1, W2, b2):
    out, _ = run(CFG, x, edge_index, W1, b1, W2, b2)
    return out


# revision 7
# speedup vs baseline: 1.1055x; 1.0143x over previous
"""Multi-graph 2-layer GCN on 8 Trainium2 NeuronCores — fused single launch, v3.

v3 over v2:
- Per-core dst blocks are sorted by edge count and packed into chunk
  positions with per-position tile caps (max over cores), cutting gather
  padding from ~19% to ~3%.
- The t2-half AllGather is split into 4 pieces interleaved into the L1
  chunk stream so most of the exchange hides behind L1 gathers.
- Layer-2 gathers use a second index table (idx2) addressing the
  piecewise/sorted t2 table layout; layer-1 indices stay natural.

See kernel_v2 docstring for the base design (bf16 pair-row gather tables,
parity tiles, one-hot S' matmuls with packed-bf16 DVE builds, dinv folded
into PE transposes, ELU = max(x+1, exp(min(x,0))) - 1).
"""

import sys

try:
    import concourse.bass as bass  # noqa: F401
except ImportError:
    sys.path.insert(0, "/opt/trn_rl_repo")
    import concourse.bass as bass

import numpy as np
import ml_dtypes

import concourse.tile as tile_mod  # noqa: F401
from concourse import bacc
import concourse.mybir as mybir
from concourse.bass_utils import run_bass_kernel_spmd
from concourse.tile import TileContext, add_dep_helper
from concourse.masks import make_identity

AF = mybir.ActivationFunctionType
ALU = mybir.AluOpType
F32 = mybir.dt.float32
BF16 = mybir.dt.bfloat16
I16 = mybir.dt.int16

BF_NP = ml_dtypes.bfloat16


def _patched_drain_and_barrier(self, tick_clock, wait_clock):
    from bass_rust import ScopedClock

    probe = self.nc.sync.nop()
    wait_clock.add_sem_waits(probe.ins, ScopedClock({None: tick_clock.global_clock}))
    si = probe.ins.sync_info
    waits = list(si.on_wait) if si and si.on_wait else []
    if si is not None:
        si.on_wait = waits[:1]
    for w in waits[1:]:
        n = self.nc.sync.nop()
        nsi = n.ins.sync_info
        if nsi is None:
            n.ins.sync_info = mybir.SyncInfo(on_wait=[w], on_update=[])
        else:
            nsi.on_wait = [w]
    self.nc.sync.drain()
    self.nc.all_engine_barrier()
    popped = self.nc._tile_sem_poison_stack.pop()
    assert popped is self._sem_poison
    self.nc.clear_and_free_semaphores(list(self.sems.allocated().values()))
    self.nc.all_engine_barrier()


TileContext._drain_and_barrier = _patched_drain_and_barrier

_orig_add_instruction = TileContext._add_instruction
_waitsplit_counter = [0]


def _patched_add_instruction(self, inst):
    """walrus rejects instructions carrying >1 sem wait; hoist excess waits
    onto same-engine nops inserted immediately before the instruction."""
    si = inst.sync_info
    if (si is not None and si.on_wait and len(si.on_wait) > 1
            and inst.engine != mybir.EngineType.Unassigned):
        waits = list(si.on_wait)
        si.on_wait = waits[-1:]
        for w in waits[:-1]:
            _waitsplit_counter[0] += 1
            nop = mybir.InstNoOp(
                name=f"I-wsplit-{_waitsplit_counter[0]}", ins=[], outs=[])
            nop.engine = inst.engine
            nop.sync_info = mybir.SyncInfo(on_wait=[w], on_update=[])
            _orig_add_instruction(self, nop)
    _orig_add_instruction(self, inst)


TileContext._add_instruction = _patched_add_instruction


# ---------------------------------------------------------------------------
# Config
# ---------------------------------------------------------------------------
class Cfg:
    def __init__(self, G, N, E, F_IN, HID, OUT, chunk=4):
        self.G, self.N, self.E = G, N, E
        self.F_IN, self.HID, self.OUT = F_IN, HID, OUT
        assert F_IN == OUT == 64 and HID == 128
        self.NB = (N + 255) // 256 * 2
        self.NPAD = self.NB * 128
        self.NBH = self.NB // 2
        self.HALF = self.NBH * 128
        self.PAIRS = self.NPAD // 2
        self.PHALF = self.HALF // 2
        self.CHUNK = chunk
        assert self.NBH % chunk == 0
        self.NCHUNK = self.NBH // chunk


CFG = Cfg(G=4, N=50000, E=800000, F_IN=64, HID=128, OUT=64, chunk=4)
NPIECE = 7


def _piece_bounds(cfg):
    # tapered: big pieces early (fully hidden behind L1 gathers), tiny
    # final piece so the serial L1->L2 exchange exposure is minimal
    return [0, 9, 18, 27, 35, 42, 47, 49]


# ---------------------------------------------------------------------------
# Layout derivation shared by host packing and device program
# ---------------------------------------------------------------------------
def derive_layout(cfg, capsE, capsO, capsM):
    """caps*: [NCHUNK][CHUNK] ints (full-E, full-O, mixed-tail tiles).
    Gather tiles per block: E+O+M; S/dstl logical columns: E+O+2M (each
    mixed tile is swept twice, with left and right rhs halves)."""
    NCH, CH = cfg.NCHUNK, cfg.CHUNK
    cap2 = [[capsE[c][b] + capsO[c][b] + capsM[c][b] for b in range(CH)]
            for c in range(NCH)]
    scols = [[capsE[c][b] + capsO[c][b] + 2 * capsM[c][b] for b in range(CH)]
             for c in range(NCH)]
    T = [sum(cap2[c]) for c in range(NCH)]
    tbase = [[0] * CH for _ in range(NCH)]
    for c in range(NCH):
        for b in range(1, CH):
            tbase[c][b] = tbase[c][b - 1] + cap2[c][b - 1]
    colbase = [[0] * CH for _ in range(NCH)]
    acc = 0
    for c in range(NCH):
        for b in range(CH):
            colbase[c][b] = acc
            acc += scols[c][b]
    totcols = acc
    slotoff = [0] * NCH
    for c in range(1, NCH):
        slotoff[c] = slotoff[c - 1] + T[c - 1] * 128
    total_slots = slotoff[-1] + T[-1] * 128
    variants = sorted({scols[c][b] for c in range(NCH) for b in range(CH)})
    iota_off = {}
    acc = 0
    for v in variants:
        iota_off[v] = acc
        acc += 128 * v
    iota_cols = acc
    bounds = _piece_bounds(cfg)
    pieces = []
    outb = 0
    for p in range(NPIECE):
        c0, c1 = bounds[p], bounds[p + 1]
        rows = (c1 - c0) * CH * 64
        pieces.append(dict(c0=c0, c1=c1, inb=c0 * CH * 64, rows=rows,
                           outb=outb))
        outb += 2 * rows
    return dict(cap2=cap2, scols=scols, T=T, tbase=tbase, colbase=colbase,
                totcols=totcols, slotoff=slotoff, total_slots=total_slots,
                variants=variants, iota_off=iota_off, iota_cols=iota_cols,
                pieces=pieces)


# ---------------------------------------------------------------------------
# Host-side preprocessing
# ---------------------------------------------------------------------------
def _wrap16(flat_i16):
    s = flat_i16.shape[0]
    assert s % 16 == 0
    w = flat_i16.reshape(s // 16, 16).T
    return np.tile(w, (8, 1))


def preprocess(cfg, edge_index):
    cores = []
    for g in range(cfg.G):
        src_g = np.asarray(edge_index[g, 0], np.int64)
        dst_g = np.asarray(edge_index[g, 1], np.int64)
        deg = np.bincount(dst_g, minlength=cfg.NPAD).astype(np.float64) + 1.0
        dinv = (1.0 / np.sqrt(deg)).astype(np.float32)
        for h in range(2):
            lo, hi = h * cfg.HALF, (h + 1) * cfg.HALF
            sel = (dst_g >= lo) & (dst_g < hi)
            s = src_g[sel]
            d = dst_g[sel] - lo
            blk = d >> 7
            dloc = d & 127
            par = s & 1
            prow = s >> 1
            # fuse L/R collisions: an even- and an odd-parity edge of the
            # same block hitting the same pair row share one gather slot
            # (the mixed tiles' dual L/R columns handle the two dsts)
            n_e = len(s)
            order = np.lexsort((par, prow, blk))
            s, blk, dloc, par, prow = (a[order] for a in
                                       (s, blk, dloc, par, prow))
            gkey = blk * cfg.PAIRS + prow
            gid = np.concatenate([[0], np.cumsum(gkey[1:] != gkey[:-1])])
            gcounts = np.bincount(gid)
            gstart = np.concatenate([[0], np.cumsum(gcounts)[:-1]])
            # per-(blk,prow) group: evens come first; j = rank within parity
            idx_in_g = np.arange(n_e) - gstart[gid]
            gp = np.bincount(gid * 2 + par, minlength=2 * (len(gcounts)))
            ne_in_g = gp[0::2][gid]
            no_in_g = gp[1::2][gid]
            j = np.where(par == 0, idx_in_g, idx_in_g - ne_in_g)
            m = np.minimum(ne_in_g, no_in_g)
            fused = j < m
            # fused-pair index within block: k-th fused-even pairs with the
            # k-th fused-odd (identical (blk,prow,j) enumeration order)
            frank = np.zeros(n_e, np.int64)
            for pv in (0, 1):
                sel = fused & (par == pv)
                bsel = blk[sel]
                cnts = np.bincount(bsel, minlength=cfg.NBH)
                st = np.concatenate([[0], np.cumsum(cnts)[:-1]])
                frank[sel] = np.arange(sel.sum()) - st[bsel]
            countsP = np.bincount(blk[fused & (par == 0)], minlength=cfg.NBH)
            # singles re-ranked within (blk, parity)
            rank = np.zeros(n_e, np.int64)
            for pv in (0, 1):
                sel = (~fused) & (par == pv)
                bsel = blk[sel]
                cnts = np.bincount(bsel, minlength=cfg.NBH)
                st = np.concatenate([[0], np.cumsum(cnts)[:-1]])
                rank[sel] = np.arange(sel.sum()) - st[bsel]
                if pv == 0:
                    countsE = cnts
                else:
                    countsO = cnts
            cores.append({
                "g": g, "h": h, "dinv": dinv, "src": s,
                "blk": blk, "dloc": dloc, "par": par, "prow": prow,
                "rank": rank, "fused": fused, "frank": frank,
                "countsE": countsE, "countsO": countsO, "countsP": countsP,
            })
    borders = []
    NCH, CH = cfg.NCHUNK, cfg.CHUNK
    capsE = np.zeros((NCH, CH), np.int64)   # full single-E tiles (floor)
    capsO = np.zeros((NCH, CH), np.int64)
    nEs, nOs, nPs = [], [], []
    for core in cores:
        tot = core["countsE"] + core["countsO"] + core["countsP"]
        border = np.argsort(-tot, kind="stable")
        borders.append(border)
        nE = core["countsE"][border].reshape(NCH, CH)
        nO = core["countsO"][border].reshape(NCH, CH)
        nP = core["countsP"][border].reshape(NCH, CH)
        nEs.append(nE)
        nOs.append(nO)
        nPs.append(nP)
        capsE = np.maximum(capsE, nE // 128)
        capsO = np.maximum(capsO, nO // 128)
    # mixed tiles hold fused pairs plus each parity's overflow beyond the
    # full tiles (one gather tile, two matmul passes with L/R columns)
    capsM = np.zeros((NCH, CH), np.int64)
    for nE, nO, nP in zip(nEs, nOs, nPs):
        tails = (nP + np.maximum(0, nE - 128 * capsE)
                 + np.maximum(0, nO - 128 * capsO))
        capsM = np.maximum(capsM, (tails + 127) // 128)
    capsE_t = tuple(tuple(int(x) for x in r) for r in capsE)
    capsO_t = tuple(tuple(int(x) for x in r) for r in capsO)
    capsM_t = tuple(tuple(int(x) for x in r) for r in capsM)
    return cores, borders, capsE_t, capsO_t, capsM_t


def build_core_arrays(cfg, cores, borders, i, capsE, capsO, capsM, lay):
    """idx1/idx2 (wrapped int16) + dstl (bf16) for core i."""
    core = cores[i]
    g = core["g"]
    border = borders[i]
    inv = np.empty(cfg.NBH, np.int64)
    inv[border] = np.arange(cfg.NBH)
    capsE_a = np.asarray(capsE)
    capsO_a = np.asarray(capsO)
    capsM_a = np.asarray(capsM)
    blk, dloc, par, prow, rank = (core[k] for k in
                                  ("blk", "dloc", "par", "prow", "rank"))
    fused, frank = core["fused"], core["frank"]
    pos = inv[blk]
    c = pos >> 2
    b = pos & 3
    capE_cb = capsE_a[c, b]
    capO_cb = capsO_a[c, b]
    capM_cb = capsM_a[c, b]
    # full-tile singles vs mixed-region slots (fused pairs first, then
    # E-tails, then O-tails)
    full = (~fused) & np.where(par == 0, rank < 128 * capE_cb,
                               rank < 128 * capO_cb)
    nEb = core["countsE"][blk]
    P_b = core["countsP"][blk]
    tailE_cnt = np.maximum(0, nEb - 128 * capE_cb)
    tail_idx = np.where(par == 0, P_b + rank - 128 * capE_cb,
                        P_b + tailE_cnt + rank - 128 * capO_cb)
    mix_idx = np.where(fused, frank, tail_idx)
    # gather tile within block and slot row
    gt = np.where(full,
                  np.where(par == 0, rank >> 7, capE_cb + (rank >> 7)),
                  capE_cb + capO_cb + (mix_idx >> 7))
    srow = np.where(full, rank & 127, mix_idx & 127)
    # S/dstl logical column within block (odd parity uses the R column set)
    scol = np.where(full,
                    np.where(par == 0, rank >> 7, capE_cb + (rank >> 7)),
                    capE_cb + capO_cb + (mix_idx >> 7)
                    + np.where(par == 0, 0, capM_cb))
    tbase = np.asarray(lay["tbase"])
    colbase = np.asarray(lay["colbase"])
    slotoff = np.asarray(lay["slotoff"])
    slot = slotoff[c] + (tbase[c, b] + gt) * 128 + srow
    idx1 = np.zeros(lay["total_slots"], np.int16)
    idx1[slot] = prow.astype(np.int16)
    # idx2: position of src's pair row in the piecewise/sorted t2 layout
    inv_of = []
    for hs in range(2):
        bo = borders[2 * g + hs]
        io = np.empty(cfg.NBH, np.int64)
        io[bo] = np.arange(cfg.NBH)
        inv_of.append(io)
    s = core["src"]
    hs = s // cfg.HALF
    srel = s - hs * cfg.HALF
    nb_nat = srel >> 7
    pos_o = np.where(hs == 0, inv_of[0][nb_nat], inv_of[1][nb_nat])
    c2 = pos_o >> 2
    rowo = pos_o * 64 + ((srel & 127) >> 1)
    bounds = _piece_bounds(cfg)
    piece_of_chunk = np.zeros(cfg.NCHUNK, np.int64)
    for p in range(NPIECE):
        piece_of_chunk[bounds[p]: bounds[p + 1]] = p
    pieces = lay["pieces"]
    inb = np.array([pc["inb"] for pc in pieces])
    rows = np.array([pc["rows"] for pc in pieces])
    outb = np.array([pc["outb"] for pc in pieces])
    pc = piece_of_chunk[c2]
    row2 = outb[pc] + hs * rows[pc] + (rowo - inb[pc])
    idx2 = np.zeros(lay["total_slots"], np.int16)
    idx2[slot] = row2.astype(np.int16)
    dstl = np.full((128, lay["totcols"]), -1.0, np.float32)
    dstl[srow, colbase[c, b] + scol] = dloc
    return _wrap16(idx1), _wrap16(idx2), dstl.astype(BF_NP)


def _iota_arr(lay):
    cols = np.empty(lay["iota_cols"], np.float32)
    for v in lay["variants"]:
        o = lay["iota_off"][v]
        cols[o: o + 128 * v] = np.repeat(np.arange(128, dtype=np.float32), v)
    return np.tile(cols, (128, 1)).astype(BF_NP)


# ---------------------------------------------------------------------------
# Device kernel
# ---------------------------------------------------------------------------
def build_kernel(cfg, capsE, capsO, capsM):
    lay = derive_layout(cfg, capsE, capsO, capsM)
    NCH, CH = cfg.NCHUNK, cfg.CHUNK
    cap2, scols, T, tbase, colbase = (lay[k] for k in
                                      ("cap2", "scols", "T", "tbase",
                                       "colbase"))
    slotoff, totcols = lay["slotoff"], lay["totcols"]
    TOT = lay["total_slots"]
    J16 = TOT // 16
    iota_off, iota_cols = lay["iota_off"], lay["iota_cols"]
    pieces = lay["pieces"]
    bounds = _piece_bounds(cfg)
    nc = bacc.Bacc(target_bir_lowering=False)

    t1p_in = nc.dram_tensor("t1p", [cfg.PAIRS, 128], BF16, kind="ExternalInput")
    t1own_in = nc.dram_tensor("t1own", [128, cfg.NBH * 64], BF16,
                              kind="ExternalInput")
    idx1_in = nc.dram_tensor("idx1", [128, J16], I16, kind="ExternalInput")
    idx2_in = nc.dram_tensor("idx2", [128, J16], I16, kind="ExternalInput")
    dstl_in = nc.dram_tensor("dstl", [128, totcols], BF16,
                             kind="ExternalInput")
    iota_in = nc.dram_tensor("iota", [128, iota_cols], BF16,
                             kind="ExternalInput")
    dinv_in = nc.dram_tensor("dinv", [128, cfg.NBH], F32, kind="ExternalInput")
    w1_in = nc.dram_tensor("w1", [64, 128], F32, kind="ExternalInput")
    b1_in = nc.dram_tensor("b1", [128, 1], F32, kind="ExternalInput")
    b1p1_in = nc.dram_tensor("b1p1", [128, 1], F32, kind="ExternalInput")
    w2_in = nc.dram_tensor("w2", [128, 64], F32, kind="ExternalInput")
    b2b_in = nc.dram_tensor("b2b", [128, CH * 64], F32, kind="ExternalInput")
    oh_out = nc.dram_tensor("oh", [cfg.HALF, 64], F32, kind="ExternalOutput")
    t2pin = nc.dram_tensor("t2pin", [cfg.PHALF, 128], BF16)
    t2pout = nc.dram_tensor("t2pout", [cfg.PAIRS, 128], BF16)

    with (
        nc.sbuf_tensor("dstl_sb", [128, totcols], BF16) as dstl_sb,
        nc.sbuf_tensor("iota_sb", [128, iota_cols], BF16) as iota_sb,
        nc.sbuf_tensor("dinv_sb", [128, cfg.NBH], F32) as dinv_sb,
        nc.sbuf_tensor("t1own_sb", [128, cfg.NBH * 64], BF16) as t1own_sb,
        nc.sbuf_tensor("t2own_sb", [128, cfg.NBH * 64], BF16) as t2own_sb,
        nc.sbuf_tensor("identb", [128, 128], BF16) as identb,
        nc.sbuf_tensor("w1bf", [64, 128], BF16) as w1bf,
        nc.sbuf_tensor("w2bf", [128, 64], BF16) as w2bf,
        nc.sbuf_tensor("b1sb", [128, 1], F32) as b1sb,
        nc.sbuf_tensor("b1p1sb", [128, 1], F32) as b1p1sb,
        nc.sbuf_tensor("b2sb", [128, CH * 64], F32) as b2sb,
        nc.semaphore("wsem") as wsem,
        nc.semaphore("ccsem") as ccsem,
    ):
        with TileContext(nc) as tc:
            with tc.tile_pool(name="pre", bufs=2) as pre:
                make_identity(nc, identb[:])
                nc.sync.dma_start(out=dstl_sb[:], in_=dstl_in[:])
                nc.sync.dma_start(out=iota_sb[:], in_=iota_in[:])
                nc.sync.dma_start(out=dinv_sb[:], in_=dinv_in[:])
                nc.sync.dma_start(out=t1own_sb[:], in_=t1own_in[:])
                nc.sync.dma_start(out=b1sb[:], in_=b1_in[:])
                nc.sync.dma_start(out=b1p1sb[:], in_=b1p1_in[:])
                nc.sync.dma_start(out=b2sb[:], in_=b2b_in[:])
                wt = pre.tile([64, 128], F32, tag="w1")
                nc.sync.dma_start(out=wt[:], in_=w1_in[:])
                nc.vector.tensor_copy(out=w1bf[:], in_=wt[:])
                wt2 = pre.tile([128, 64], F32, tag="w2")
                nc.sync.dma_start(out=wt2[:], in_=w2_in[:])
                nc.vector.tensor_copy(out=w2bf[:], in_=wt2[:])

        from contextlib import ExitStack
        with TileContext(nc) as tc:
            with ExitStack() as stack:
                idxp = stack.enter_context(tc.tile_pool(name="idxp", bufs=3))
                msgp = stack.enter_context(tc.tile_pool(name="msgp", bufs=2))
                spool = stack.enter_context(tc.tile_pool(name="sp", bufs=3))
                aggpool = stack.enter_context(
                    tc.tile_pool(name="aggp", bufs=2, space="PSUM"))
                tppool = stack.enter_context(
                    tc.tile_pool(name="tpp", bufs=1, space="PSUM"))
                h1pool = stack.enter_context(
                    tc.tile_pool(name="h1p", bufs=2, space="PSUM"))
                zpool = stack.enter_context(
                    tc.tile_pool(name="zpp", bufs=1, space="PSUM"))
                t2ppool = stack.enter_context(
                    tc.tile_pool(name="t2pp", bufs=2, space="PSUM"))
                finp = stack.enter_context(tc.tile_pool(name="fin", bufs=2))
                finp2 = stack.enter_context(tc.tile_pool(name="fin2", bufs=2))
                stgp = stack.enter_context(tc.tile_pool(name="stg", bufs=2))

                regs = {}
                for v in sorted(set(T)):
                    regs[v] = nc.gpsimd.to_reg(v * 128)

                def chunk_agg(c, table, idx_dram):
                    Tc = T[c]
                    idx_t = idxp.tile([128, Tc * 128 // 16], I16)
                    nc.sync.dma_start(
                        out=idx_t[:],
                        in_=idx_dram[:, slotoff[c] // 16:
                                     slotoff[c] // 16 + Tc * 128 // 16])
                    msg = msgp.tile([128, Tc * 128], BF16)
                    nc.gpsimd.dma_gather(
                        out_ap=msg[:].rearrange("p (t e) -> p t e", e=128),
                        in_ap=table[0: cfg.PAIRS, :],
                        idxs_ap=idx_t[:],
                        num_idxs=Tc * 128,
                        num_idxs_reg=regs[Tc],
                        elem_size=128,
                        single_packet=False,
                    )
                    aggP = aggpool.tile([128, CH * 64], F32)
                    for b in range(CH):
                        ks = scols[c][b]
                        kE = capsE[c][b]
                        kO = capsO[c][b]
                        kM = capsM[c][b]
                        io = iota_off[ks]
                        S = spool.tile([128, 128 * ks], BF16)
                        Sv = S[:].rearrange("p (v t) -> p v t", t=ks)
                        nc.vector.tensor_tensor(
                            out=Sv,
                            in0=iota_sb[:, io: io + 128 * ks]
                                .rearrange("p (v t) -> p v t", t=ks),
                            in1=dstl_sb[:, colbase[c][b]: colbase[c][b] + ks]
                                .to_broadcast([128, ks, 128])
                                .rearrange("p t v -> p v t"),
                            op=ALU.is_equal,
                        )
                        for ln in range(ks):
                            # logical col -> (gather tile, rhs half): full E,
                            # full O, mixed L-pass, mixed R-pass
                            if ln < kE + kO + kM:
                                gt = ln
                                off = 0 if (ln < kE or ln >= kE + kO) else 64
                            else:
                                gt = ln - kM
                                off = 64
                            j = tbase[c][b] + gt
                            nc.tensor.matmul(
                                out=aggP[:, b * 64: (b + 1) * 64],
                                lhsT=Sv[:, :, ln],
                                rhs=msg[:, j * 128 + off: j * 128 + off + 64],
                                start=(ln == 0),
                                stop=(ln == ks - 1),
                            )
                    return aggP

                def finish_l1(c, aggP):
                    # u1 = agg + t1 (f32), scale by dinv in f32 via ACT,
                    # single bf16 rounding at aggV before the transposes
                    u1 = finp.tile([128, CH * 64], F32, tag="u1")
                    nc.vector.tensor_tensor(
                        out=u1[:], in0=aggP[:],
                        in1=t1own_sb[:, c * CH * 64: (c + 1) * CH * 64],
                        op=ALU.add)
                    aggV = finp.tile([128, CH * 64], BF16, tag="aggV")
                    tP = tppool.tile([64, CH * 128], BF16, tag="tp")
                    for b in range(CH):
                        gb = c * CH + b
                        nc.scalar.activation(
                            aggV[:, b * 64: (b + 1) * 64],
                            u1[:, b * 64: (b + 1) * 64],
                            AF.Copy, scale=dinv_sb[:, gb: gb + 1])
                        nc.tensor.transpose(
                            out=tP[:, b * 128: (b + 1) * 128],
                            in_=aggV[:, b * 64: (b + 1) * 64],
                            identity=identb[:])
                    aggT = finp.tile([64, CH * 128], BF16, tag="aggT")
                    nc.scalar.activation(aggT[:], tP[:], AF.Copy)
                    h1P = h1pool.tile([128, CH * 128], F32, tag="h1p")
                    nc.tensor.matmul(out=h1P[:], lhsT=w1bf[:], rhs=aggT[:],
                                     start=True, stop=True)
                    m = finp2.tile([128, CH * 128], F32, tag="m")
                    nc.vector.tensor_scalar(
                        out=m[:], in0=h1P[:], scalar1=b1sb[:, 0:1],
                        scalar2=0.0, op0=ALU.add, op1=ALU.min)
                    x1 = finp2.tile([128, CH * 128], F32, tag="x1")
                    nc.vector.tensor_scalar(
                        out=x1[:], in0=h1P[:], scalar1=b1p1sb[:, 0:1],
                        scalar2=None, op0=ALU.add)
                    ex = finp2.tile([128, CH * 128], F32, tag="ex")
                    nc.scalar.activation(ex[:], m[:], AF.Exp)
                    fmx = finp2.tile([128, CH * 128], F32, tag="fmx")
                    nc.vector.tensor_tensor(out=fmx[:], in0=ex[:], in1=x1[:],
                                            op=ALU.max)
                    h1f = finp2.tile([128, CH * 128], BF16, tag="h1f")
                    nc.vector.tensor_scalar(
                        out=h1f[:], in0=fmx[:], scalar1=-1.0, scalar2=None,
                        op0=ALU.add)
                    zP = zpool.tile([64, CH * 128], F32, tag="zp")
                    nc.tensor.matmul(out=zP[:], lhsT=w2bf[:], rhs=h1f[:],
                                     start=True, stop=True)
                    zsb = finp.tile([64, CH * 128], BF16, tag="zsb")
                    nc.scalar.activation(zsb[:], zP[:], AF.Copy)
                    stage = stgp.tile([128, CH * 64], BF16, tag="stage")
                    t2P = t2ppool.tile([128, CH * 64], BF16, tag="t2P")
                    last_act = None
                    for b in range(CH):
                        gb = c * CH + b
                        nc.tensor.transpose(
                            out=t2P[:, b * 64: (b + 1) * 64],
                            in_=zsb[:, b * 128: (b + 1) * 128],
                            identity=identb[:64, :64])
                        last_act = nc.scalar.activation(
                            stage[:, b * 64: (b + 1) * 64],
                            t2P[:, b * 64: (b + 1) * 64],
                            AF.Copy, scale=dinv_sb[:, gb: gb + 1])
                    cp = nc.vector.tensor_copy(
                        out=t2own_sb[:, c * CH * 64: (c + 1) * CH * 64],
                        in_=stage[:])
                    t2own_copies.append(cp)
                    nc.sync.dma_start(
                        out=t2pin[c * CH * 64: (c + 1) * CH * 64, :]
                        .rearrange("(b q) (r e) -> (q r) b e",
                                   q=64, r=2, e=64),
                        in_=stage[:].rearrange("p (b e) -> p b e", e=64),
                    )

                def emit_piece(p):
                    # no manual sems: the shadow tracker orders the
                    # collective after the t2pin region writes and the L2
                    # gathers after the collective outputs
                    pc = pieces[p]
                    nc.gpsimd.collective_compute(
                        "AllGather", ALU.bypass,
                        replica_groups=[[0, 1], [2, 3], [4, 5], [6, 7]],
                        ins=[t2pin[pc["inb"]: pc["inb"] + pc["rows"], :]
                             .opt()],
                        outs=[t2pout[pc["outb"]:
                                     pc["outb"] + 2 * pc["rows"], :]
                              .opt()],
                    )

                # ---------------- Layer 1 (pieces interleaved) ------------
                t2own_copies = []
                next_piece = 0
                for c in range(NCH):
                    aggP = chunk_agg(c, t1p_in, idx1_in)
                    finish_l1(c, aggP)
                    # emit piece p two chunks after its last input chunk so
                    # the Pool engine has gathers in flight while it waits
                    if (next_piece < NPIECE - 1
                            and c == bounds[next_piece + 1] + 1):
                        emit_piece(next_piece)
                        next_piece += 1
                for p in range(next_piece, NPIECE):
                    emit_piece(p)

                # ---------------- Layer 2 ----------------
                for c in range(NCH):
                    aggP = chunk_agg(c, t2pout, idx2_in)
                    u = finp.tile([128, CH * 64], F32, tag="u")
                    uadd = nc.vector.tensor_tensor(
                        out=u[:], in0=aggP[:],
                        in1=t2own_sb[:, c * CH * 64: (c + 1) * CH * 64],
                        op=ALU.add)
                    # raw-sbuf RAW hazard: order the L2 self-loop read after
                    # the L1 writer of the same t2own region explicitly
                    add_dep_helper(uadd.ins, t2own_copies[c].ins,
                                   reason="L2 self-loop reads t2own chunk")
                    y = finp2.tile([128, CH * 64], F32, tag="y")
                    for b in range(CH):
                        gb = c * CH + b
                        nc.vector.tensor_scalar_mul(
                            y[:, b * 64: (b + 1) * 64],
                            u[:, b * 64: (b + 1) * 64],
                            dinv_sb[:, gb: gb + 1])
                    yb = finp2.tile([128, CH * 64], F32, tag="yb")
                    nc.vector.tensor_tensor(out=yb[:], in0=y[:], in1=b2sb[:],
                                            op=ALU.add)
                    m2 = finp2.tile([128, CH * 64], F32, tag="m2")
                    nc.vector.tensor_scalar(
                        out=m2[:], in0=yb[:], scalar1=0.0, scalar2=None,
                        op0=ALU.min)
                    x12 = finp2.tile([128, CH * 64], F32, tag="x12")
                    nc.vector.tensor_scalar(
                        out=x12[:], in0=yb[:], scalar1=1.0, scalar2=None,
                        op0=ALU.add)
                    e2 = finp2.tile([128, CH * 64], F32, tag="e2")
                    nc.scalar.activation(e2[:], m2[:], AF.Exp)
                    f2 = finp2.tile([128, CH * 64], F32, tag="f2")
                    nc.vector.tensor_tensor(out=f2[:], in0=e2[:], in1=x12[:],
                                            op=ALU.max)
                    stage2 = stgp.tile([128, CH * 64], F32, tag="stage2")
                    nc.vector.tensor_scalar(
                        out=stage2[:], in0=f2[:], scalar1=-1.0, scalar2=None,
                        op0=ALU.add)
                    nc.sync.dma_start(
                        out=oh_out[c * CH * 128: (c + 1) * CH * 128, :]
                        .rearrange("(b p) e -> p b e", p=128),
                        in_=stage2[:].rearrange("p (b e) -> p b e", e=64),
                    )
    nc.finalize()
    return nc


# ---------------------------------------------------------------------------
# Driver
# ---------------------------------------------------------------------------
_NC_CACHE = {}
_PREP_CACHE = {}
LAST_TIMES = {}
_LAST_CAPS = None


def _get_nc(cfg, capsE, capsO, capsM):
    key = (cfg.N, cfg.E, capsE, capsO, capsM)
    if key not in _NC_CACHE:
        _NC_CACHE[key] = build_kernel(cfg, capsE, capsO, capsM)
    return _NC_CACHE[key]


def run(cfg, x, edge_index, W1, b1, W2, b2, spmd_kwargs=None):
    global _LAST_CAPS
    spmd_kwargs = spmd_kwargs or {}
    x = np.asarray(x, np.float32)
    W1 = np.asarray(W1, np.float32)
    b1 = np.asarray(b1, np.float32)
    W2 = np.asarray(W2, np.float32)
    b2 = np.asarray(b2, np.float32)

    import hashlib
    ekey = hashlib.sha1(np.ascontiguousarray(edge_index)).hexdigest()
    if ekey in _PREP_CACHE:
        cores, borders, capsE, capsO, capsM, lay, core_arr = _PREP_CACHE[ekey]
    else:
        cores, borders, capsE, capsO, capsM = preprocess(cfg, edge_index)
        lay = derive_layout(cfg, capsE, capsO, capsM)
        core_arr = [build_core_arrays(cfg, cores, borders, i, capsE, capsO,
                                      capsM, lay) for i in range(len(cores))]
        _PREP_CACHE[ekey] = (cores, borders, capsE, capsO, capsM, lay,
                             core_arr)
    _LAST_CAPS = (capsE, capsO, capsM)
    nc = _get_nc(cfg, capsE, capsO, capsM)
    iota = _iota_arr(lay)

    in_maps = []
    for i, core in enumerate(cores):
        g, h = core["g"], core["h"]
        border = borders[i]
        idx1w, idx2w, dstl = core_arr[i]
        dinv = core["dinv"]
        t1 = np.zeros((cfg.NPAD, 64), np.float32)
        t1[: cfg.N] = x[g]
        t1 *= dinv[:, None]
        t1p = np.ascontiguousarray(t1.reshape(cfg.PAIRS, 128)).astype(BF_NP)
        lo = h * cfg.HALF
        t1h = t1[lo: lo + cfg.HALF].reshape(cfg.NBH, 128, 64)
        t1own = np.ascontiguousarray(
            t1h[border].transpose(1, 0, 2).reshape(128, cfg.NBH * 64)
        ).astype(BF_NP)
        dinv_own = np.ascontiguousarray(
            dinv[lo: lo + cfg.HALF].reshape(cfg.NBH, 128)[border].T)
        in_maps.append({
            "t1p": t1p,
            "t1own": t1own,
            "idx1": idx1w,
            "idx2": idx2w,
            "dstl": np.ascontiguousarray(dstl),
            "iota": iota,
            "dinv": dinv_own,
            "w1": np.ascontiguousarray(W1[g]),
            "b1": np.ascontiguousarray(b1[g].reshape(128, 1)),
            "b1p1": np.ascontiguousarray(b1[g].reshape(128, 1) + 1.0),
            "w2": np.ascontiguousarray(W2[g]),
            "b2b": np.ascontiguousarray(
                np.tile(b2[g], (128, cfg.CHUNK)).astype(np.float32)),
        })
    import time as _time
    _t = _time.monotonic()
    res = run_bass_kernel_spmd(nc, in_maps, core_ids=list(range(8)),
                               **spmd_kwargs)
    LAST_TIMES["launch_wall_s"] = _time.monotonic() - _t

    out = np.empty((cfg.G * cfg.N, 64), np.float32)
    for g in range(cfg.G):
        for h in range(2):
            i = 2 * g + h
            oh = res.results[i]["oh"].reshape(cfg.NBH, 128, 64)
            inv = np.empty(cfg.NBH, np.int64)
            inv[borders[i]] = np.arange(cfg.NBH)
            nat = oh[inv].reshape(cfg.HALF, 64)
            lo = g * cfg.N + h * cfg.HALF
            n_rows = min(cfg.HALF, cfg.N - h * cfg.HALF)
            out[lo: lo + n_rows] = nat[:n_rows]
    return out, res


def kernel(x, edge_index, W1, b1, W2, b2):
    out, _ = run(CFG, x, edge_index, W1, b1, W2, b2)
    return out
